# revision 1
# baseline (speedup 1.0000x reference)
"""CoconBlock forward on 8 Trainium2 NeuronCores.

Sharding: core c = (b, g) with b = c // 4 (batch), g = c % 4 (tensor-parallel
rank). Within each batch group of 4 cores:
  - attention QKV / context-KV weights column-sharded by head group (3 heads),
  - W_proj row-sharded, partial outputs AllReduced,
  - MLP W_fc column-sharded / W_mproj row-sharded, partial outputs (with the
    residual and bias pre-folded as +h2/4 + b_mproj/4 per core) ReduceScattered
    so each core lands exactly its 192-feature slice of the final output.

All on-device activations are feature-on-partition (f32 has no DMA transpose;
this layout makes every matmul transpose-free). LayerNorm reductions over the
feature (partition) axis run on the PE via ones-vector matmuls; the affine
(gamma/beta) is folded into the following weight matrix on the host. Engine
balance: exp/gelu/sqrt on ACT, every psum->sbuf copy/bias-add on DVE.

Attention uses the 128-aligned causal structure: with queries padded to 1152
and keys (256 context + 1025 self) padded to 1408, a (query-block, key-tile)
pair is fully allowed, partially masked by a shifted-triangle slice of one
master mask, or skipped entirely. Scores for a 512-query block are exp'ed in
one ACT op per key tile; the softmax denominator rides the attend matmul as a
ones-column appended to V (partition 64), and 1/den is broadcast to partitions
0..63 with a K=1 PE outer product.
"""

import sys

sys.path.insert(0, "/opt/trn_rl_repo")

import ml_dtypes
import numpy as np

import concourse.bass as bass
import concourse.bacc as bacc
import concourse.mybir as mybir
import concourse.tile as tile
from concourse.bass_utils import run_bass_kernel_spmd

F32 = mybir.dt.float32
AF = mybir.ActivationFunctionType
ALU = mybir.AluOpType
ts, ds = bass.ts, bass.ds

D = 768
DH = 64
S = 1024
SC = 256
TOK = S + 1            # 1025 (sos + x)
TOKP = 1152            # 9 * 128
NT = TOKP // 128       # 9
KEYSP = 1408           # 11 * 128
NK = KEYSP // 128      # 11
KO = D // 128          # 6 feature sub-tiles
TP = 4
FG = 192               # features per core in head-sharded tensors (3 heads)
HG = 3                 # heads per core
EPS = 1e-5
N_CORES = 8
GROUPS = [[0, 1, 2, 3], [4, 5, 6, 7]]
BLKS = [(0, 512), (512, 512), (1024, 128)]  # token blocks (start, len)


BF16 = mybir.dt.bfloat16


def _emit_ln(nc, pers, trans, psum, src, dst, onec_sb, oner_sb, eps_sb):
    """dst = (src - mean) * rsqrt(var + eps), stats over the feature axis."""
    m_row = trans.tile([1, TOKP], F32, tag="m_row", bufs=2)
    s_row = trans.tile([1, TOKP], F32, tag="s_row", bufs=2)
    for bs, bl in BLKS:
        sl = ds(bs, bl)
        ps_m = psum.tile([128, 512], F32, tag="sm", bufs=2)
        ps_s = psum.tile([128, 512], F32, tag="sm", bufs=2)
        for o in range(KO):
            sq = trans.tile([128, 512], BF16, tag="sq", bufs=2)
            nc.vector.tensor_mul(sq[:, :bl], src[:, o, sl], src[:, o, sl])
            nc.tensor.matmul(
                ps_m[0:1, :bl], onec_sb, src[:, o, sl],
                start=(o == 0), stop=(o == KO - 1),
            )
            nc.tensor.matmul(
                ps_s[0:1, :bl], onec_sb, sq[:, :bl],
                start=(o == 0), stop=(o == KO - 1),
            )
        nc.vector.tensor_scalar_mul(m_row[:, sl], ps_m[0:1, :bl], 1.0 / D)
        nc.vector.tensor_scalar_mul(s_row[:, sl], ps_s[0:1, :bl], 1.0 / D)
    # var = E[x^2] - mean^2 ; rstd = 1/sqrt(var + eps) ; mr = mean * rstd
    m2 = trans.tile([1, TOKP], F32, tag="lntmp", bufs=2)
    nc.vector.tensor_mul(m2[:], m_row[:], m_row[:])
    nc.vector.tensor_tensor(s_row[:], s_row[:], m2[:], ALU.subtract)
    nc.scalar.activation(s_row[:], s_row[:], AF.Sqrt, bias=eps_sb)
    nc.vector.reciprocal(s_row[:], s_row[:])
    nc.vector.tensor_mul(m_row[:], m_row[:], s_row[:])

    # broadcast rstd and mean*rstd across partitions (bf16 outer products;
    # bf16 rounding of rstd only scales the normalized values by ~0.4%)
    rs16 = trans.tile([1, TOKP], BF16, tag="rs16", bufs=2)
    mr16 = trans.tile([1, TOKP], BF16, tag="mr16", bufs=2)
    nc.vector.tensor_copy(rs16[:], s_row[:])
    nc.vector.tensor_copy(mr16[:], m_row[:])
    for bs, bl in BLKS:
        sl = ds(bs, bl)
        ps_b = psum.tile([128, 512], F32, tag="sm", bufs=2)
        nc.tensor.matmul(ps_b[:, :bl], oner_sb[0:1, 0:128], rs16[:, sl],
                         start=True, stop=True)
        for o in range(KO):
            # dst = src * rstd only; the -mean*rstd term is folded into the
            # consuming matmuls as a rank-1 correction (colsum(W) x mr)
            nc.vector.tensor_mul(dst[:, o, sl], src[:, o, sl], ps_b[:, :bl])
    return mr16


def build_program(sim_collectives=True, gelu_fn=None, debug_taps=False):
    if gelu_fn is None:
        gelu_fn = AF.Gelu_apprx_tanh
    nc = bacc.Bacc(None, num_devices=N_CORES)

    # ---- DRAM I/O ----
    hT_d = nc.dram_tensor("hT", [D, TOKP], BF16, kind="ExternalInput")
    ctxT_d = nc.dram_tensor("ctxT", [D, SC], BF16, kind="ExternalInput")
    wq_d = nc.dram_tensor("wq", [D, FG], BF16, kind="ExternalInput")
    wk_d = nc.dram_tensor("wk", [D, FG], BF16, kind="ExternalInput")
    wv_d = nc.dram_tensor("wv", [D, FG], BF16, kind="ExternalInput")
    wrk_d = nc.dram_tensor("wrk", [D, FG], BF16, kind="ExternalInput")
    wrv_d = nc.dram_tensor("wrv", [D, FG], BF16, kind="ExternalInput")
    bq_d = nc.dram_tensor("bq", [FG, 1], F32, kind="ExternalInput")
    bk_d = nc.dram_tensor("bk", [FG, 1], F32, kind="ExternalInput")
    brk_d = nc.dram_tensor("brk", [FG, 1], F32, kind="ExternalInput")
    bv_d = nc.dram_tensor("bv_r", [1, FG], BF16, kind="ExternalInput")
    brv_d = nc.dram_tensor("brv_r", [1, FG], BF16, kind="ExternalInput")
    wp0_d = nc.dram_tensor("wp0", [DH, D], BF16, kind="ExternalInput")
    wp1_d = nc.dram_tensor("wp1", [DH, D], BF16, kind="ExternalInput")
    wp2_d = nc.dram_tensor("wp2", [DH, D], BF16, kind="ExternalInput")
    bp4_d = nc.dram_tensor("bp4", [D, 1], F32, kind="ExternalInput")
    wfc_d = nc.dram_tensor("wfc", [D, D], BF16, kind="ExternalInput")
    bfc_d = nc.dram_tensor("bfc", [D, 1], F32, kind="ExternalInput")
    wmp_d = nc.dram_tensor("wmp", [D, D], BF16, kind="ExternalInput")
    bmp16_d = nc.dram_tensor("bmp16_r", [1, D], BF16, kind="ExternalInput")
    mask_d = nc.dram_tensor("mask", [128, 896], BF16, kind="ExternalInput")
    onec_d = nc.dram_tensor("onec", [128, 1], BF16, kind="ExternalInput")
    oner_d = nc.dram_tensor("oner", [1, 512], F32)
    crow_d = nc.dram_tensor("crow", [1, 1344], BF16, kind="ExternalInput")
    oner16_d = nc.dram_tensor("oner16", [1, 512], BF16, kind="ExternalInput")
    out_d = nc.dram_tensor("out", [FG, TOKP], BF16, kind="ExternalOutput")

    with tile.TileContext(nc) as tc, \
         tc.tile_pool(name="pers", bufs=1) as pers, \
         tc.tile_pool(name="trans", bufs=2) as trans, \
         tc.tile_pool(name="psum", bufs=1, space="PSUM") as psum, \
         tc.tile_pool(name="dram", bufs=1, space="DRAM") as dram:

        # ---- resident SBUF tensors ----
        h_main = pers.tile([128, KO, TOKP], BF16, tag="h_main")
        ctx = pers.tile([128, KO, SC], BF16, tag="ctx")
        wp_sb = [pers.tile([DH, D], BF16, tag=f"wp{h}", name=f"wp{h}")
                 for h in range(HG)]
        kf01 = pers.tile([128, KEYSP], BF16, tag="kf01")
        kf2 = pers.tile([64, KEYSP], BF16, tag="kf2")
        q01 = pers.tile([128, TOKP], BF16, tag="q01")
        q2 = pers.tile([64, TOKP], BF16, tag="q2")
        vsb = pers.tile([128, NK, HG, DH + 1], BF16, tag="vsb")
        a_sb = [pers.tile([DH, TOKP], BF16, tag=f"a{h}", name=f"a{h}")
                for h in range(HG)]
        gl = pers.tile([128, KO, TOKP], BF16, tag="gl")
        mask_sb = pers.tile([128, 896], BF16, tag="mask")
        cst = pers.tile([128, 20], F32, tag="cst")
        bq01, bq2 = cst[:, 0:1], cst[0:64, 1:2]
        bk01, bk2 = cst[:, 2:3], cst[0:64, 3:4]
        brk01, brk2 = cst[:, 4:5], cst[0:64, 5:6]
        bp4, bfc = cst[:, 6:12], cst[:, 12:18]
        eps_sb = cst[0:1, 19:20]
        rows = pers.tile([1, 1664], F32, tag="rows")
        ones_t = pers.tile([128, DH], BF16, tag="ones_t")
        oner = rows[:, 1152:1664]
        rows16 = pers.tile([1, 1664], BF16, tag="rows16")
        crow = pers.tile([1, 1344], BF16, tag="crow")
        bv_r, brv_r = rows16[:, 0:FG], rows16[:, FG:2 * FG]
        oner16, bmp16_r = rows16[:, 384:896], rows16[:, 896:896 + D]
        cst16 = pers.tile([128, 2], BF16, tag="cst16")
        onec = cst16[:, 0:1]

        # ---- constant / activation input DMAs ----
        pin = lambda t: t.rearrange("(o p) n -> p o n", p=128)
        nc.sync.dma_start(out=h_main[:], in_=pin(hT_d))
        nc.sync.dma_start(out=ctx[:], in_=pin(ctxT_d))
        for h, wpd in enumerate([wp0_d, wp1_d, wp2_d]):
            nc.sync.dma_start(out=wp_sb[h][:], in_=wpd[:])
        nc.sync.dma_start(out=mask_sb[:], in_=mask_d[:])
        nc.sync.dma_start(out=onec, in_=onec_d[:])
        nc.sync.dma_start(out=oner, in_=oner_d[:])
        nc.sync.dma_start(out=oner16, in_=oner16_d[:])
        nc.sync.dma_start(out=crow, in_=crow_d[:])
        nc.sync.dma_start(out=bq01, in_=bq_d[0:128, :])
        nc.sync.dma_start(out=bq2, in_=bq_d[128:FG, :])
        nc.sync.dma_start(out=bk01, in_=bk_d[0:128, :])
        nc.sync.dma_start(out=bk2, in_=bk_d[128:FG, :])
        nc.sync.dma_start(out=brk01, in_=brk_d[0:128, :])
        nc.sync.dma_start(out=brk2, in_=brk_d[128:FG, :])
        nc.sync.dma_start(out=bv_r, in_=bv_d[:])
        nc.sync.dma_start(out=brv_r, in_=brv_d[:])
        nc.sync.dma_start(out=bp4, in_=bp4_d.rearrange("(o p) 1 -> p o", p=128))
        nc.sync.dma_start(out=bfc, in_=bfc_d.rearrange("(o p) 1 -> p o", p=128))
        nc.sync.dma_start(out=bmp16_r, in_=bmp16_d[:])
        nc.vector.memset(eps_sb, EPS)
        nc.vector.memset(ones_t[:], 1.0)
        nc.vector.memset(kf01[:], 0.0)
        nc.vector.memset(kf2[:], 0.0)
        nc.vector.memset(vsb[:], 0.0)

        # ---- LN1 (normalize only; affine folded into wq/wk/wv) ----
        hl = pers.tile([128, KO, TOKP], BF16, tag="ln_out")
        mr1 = _emit_ln(nc, pers, trans, psum, h_main, hl, onec, oner16, eps_sb)

        def load_w(dram_t):
            w = pers.tile([128, KO, FG], BF16, tag="wqkv", bufs=2)
            nc.sync.dma_start(out=w[:], in_=pin(dram_t))
            return w

        # ---- QKV projections (feature-major q/k; token-major v) ----
        def qk_proj(w_sb, b01, b2, dst01, dst2, dst_off, src, src_len,
                    corr=None, mr=None):
            # dst[mi] = w[:, :, mi].T @ src + bias, written at dst col offset
            for dst_t, bias_t, m0, ml in [(dst01, b01, 0, 128),
                                          (dst2, b2, 128, 64)]:
                for bs, bl in [(s, l) for (s, l) in BLKS if s < src_len]:
                    bl = min(bl, src_len - bs)
                    ps = psum.tile([128, 512], F32, tag="sm", bufs=2)
                    for o in range(KO):
                        nc.tensor.matmul(
                            ps[:ml, :bl], w_sb[:, o, ds(m0, ml)],
                            src[:, o, ds(bs, bl)],
                            start=(o == 0),
                            stop=(corr is None and o == KO - 1),
                        )
                    if corr is not None:
                        nc.tensor.matmul(
                            ps[:ml, :bl], crow[0:1, ds(corr + m0, ml)],
                            mr[0:1, ds(bs, bl)], start=False, stop=True,
                        )
                    if dst_off + bs == SC + 1024:
                        # only self token 1024 (key col 1280) is real
                        nc.vector.tensor_scalar_add(
                            dst_t[:ml, 1280:1281], ps[:ml, 0:1],
                            bias_t[:ml, :])
                    else:
                        nc.vector.tensor_scalar_add(
                            dst_t[:ml, ds(dst_off + bs, bl)], ps[:ml, :bl],
                            bias_t[:ml, :])

        def v_proj(w_sb, b_row, src, n_tiles, kt_base, corr=None, mr=None):
            # V[token, feat] = src.T @ w + 1 (x) bias  (token-major output)
            for tt in range(n_tiles):
                ps = psum.tile([128, 512], F32, tag="sm", bufs=2)
                for o in range(KO):
                    nc.tensor.matmul(
                        ps[:, :FG], src[:, o, ts(tt, 128)], w_sb[:, o, :],
                        start=(o == 0), stop=False,
                    )
                if corr is not None:
                    nc.tensor.matmul(ps[:, :FG], mr[0:1, ts(tt, 128)],
                                     crow[0:1, ds(corr, FG)],
                                     start=False, stop=False)
                nc.tensor.matmul(ps[:, :FG], oner16[0:1, 0:128], b_row,
                                 start=False, stop=True)
                kt = kt_base + tt
                rws = 1 if kt == NK - 1 else 128
                nc.vector.tensor_copy(
                    vsb[:rws, kt, :, 0:DH],
                    ps[:rws, 0:FG].rearrange("p (h d) -> p h d", h=HG))

        # q: all padded tokens; k(self): keys 256..1280; kc: keys 0..255
        qk_proj(load_w(wq_d), bq01, bq2, q01, q2, 0, hl, TOKP,
                corr=0, mr=mr1)
        qk_proj(load_w(wk_d), bk01, bk2, kf01, kf2, SC, hl, TOKP,
                corr=FG, mr=mr1)
        qk_proj(load_w(wrk_d), brk01, brk2, kf01, kf2, 0, ctx, SC,
                corr=None, mr=None)
        v_proj(load_w(wrv_d), brv_r, ctx, 2, 0)   # context V -> key tiles 0..1
        v_proj(load_w(wv_d), bv_r, hl, NT, 2, corr=2 * FG, mr=mr1)
        # denominator ones-column (pad rows of the ones column only land in
        # masked or garbage-query positions)
        nc.vector.memset(vsb[:, :, :, DH:DH + 1], 1.0)

        # ---- attention ----
        # per (head, query block): key tiles in groups share one multi-bank
        # scores psum so each exp ACT op covers up to 3x512 (or the whole
        # 128-wide tail) and the ~1.6us fixed ACT cost is amortized.
        kf_of = [(kf01, 0), (kf01, 64), (kf2, 0)]
        q_of = [(q01, 0), (q01, 64), (q2, 0)]
        for h in range(HG):
            kf_t, kf_o = kf_of[h]
            q_t, q_o = q_of[h]
            for qs, ql in BLKS:
                qb0 = qs // 128                      # first 128-query tile
                last_kt = min((qs + ql - 1) // 128 + 2, NK - 1)
                ps_a = psum.tile([128, 512], F32, tag="sm", bufs=2,
                                 name="ps_a")[0:DH + 1]
                gsz = 3 if ql == 512 else 12         # kt per exp group
                for ktg in range(0, last_kt + 1, gsz):
                    kts = range(ktg, min(ktg + gsz, last_kt + 1))
                    ps_s = psum.tile([128, 1536], F32, tag="big3", bufs=2,
                                     name="ps_s")
                    for j, kt in enumerate(kts):
                        nc.tensor.matmul(
                            ps_s[:, ds(j * ql, ql)],
                            kf_t[kf_o:kf_o + DH, ts(kt, 128)],
                            q_t[q_o:q_o + DH, ds(qs, ql)],
                            start=True, stop=True,
                        )
                    expt = trans.tile([128, 1536], BF16, tag="expt", bufs=3)
                    nw = len(kts) * ql
                    nc.scalar.activation(expt[:, :nw], ps_s[:, :nw], AF.Exp,
                                         scale=0.125)
                    for j, kt in enumerate(kts):
                        if kt >= qb0 + 2:  # shifted-triangle mask
                            dlt = kt - qb0 - 2
                            nc.vector.tensor_mul(
                                expt[:, ds(j * ql, ql)],
                                expt[:, ds(j * ql, ql)],
                                mask_sb[:, ds(384 - 128 * dlt, ql)])
                        nc.tensor.matmul(
                            ps_a[:, :ql], vsb[:, kt, h, :],
                            expt[:, ds(j * ql, ql)],
                            start=(kt == 0), stop=(kt == last_kt),
                        )
                # normalize: a = num * (1/den); den sits at partition 64,
                # broadcast to partitions 0..63 via a K=1 PE outer product
                rec = trans.tile([DH + 1, 512], F32, tag="rec", bufs=2)
                r16 = trans.tile([DH + 1, 512], BF16, tag="r16", bufs=2)
                nc.vector.reciprocal(rec[DH:DH + 1, :ql], ps_a[DH:DH + 1, :ql])
                nc.vector.tensor_copy(r16[DH:DH + 1, :ql], rec[DH:DH + 1, :ql])
                ps_r = psum.tile([128, 512], F32, tag="sm", bufs=2,
                                 name="ps_r")[0:DH]
                nc.tensor.matmul(ps_r[:, :ql], ones_t[DH:DH + 1, :],
                                 r16[DH:DH + 1, :ql], start=True, stop=True)
                nc.vector.tensor_copy(rec[0:DH, :ql], ps_r[:, :ql])
                nc.vector.tensor_mul(a_sb[h][:, ds(qs, ql)],
                                     ps_a[0:DH, :ql], rec[0:DH, :ql])

        # ---- attention output projection (row-sharded) + AllReduce ----
        dar_in = dram.tile([KO, 128, TOKP], BF16)
        dar_out = dram.tile([KO, 128, TOKP], BF16)
        for mo in range(KO):
            for bs, bl in BLKS:
                ps = psum.tile([128, 512], F32, tag="sm", bufs=2)
                for h in range(HG):
                    nc.tensor.matmul(ps[:, :bl], wp_sb[h][:, ts(mo, 128)],
                                     a_sb[h][:, ds(bs, bl)],
                                     start=(h == 0), stop=(h == HG - 1))
                armo = trans.tile([128, 512], BF16, tag="armo", bufs=2)
                nc.vector.tensor_scalar_add(armo[:, :bl], ps[:, :bl],
                                            bp4[:, mo:mo + 1])
                nc.sync.dma_start(dar_in[mo, :, ds(bs, bl)], armo[:, :bl])
        if sim_collectives:
            nc.gpsimd.collective_compute(
                "AllReduce", ALU.add, replica_groups=GROUPS,
                ins=[dar_in.opt()], outs=[dar_out.opt()],
            )
        else:
            nc.gpsimd.dma_start(dar_out[:], dar_in[:])
        # residual: h_main <- h_main + allreduced proj output (in place)
        for mo in range(KO):
            for bs, bl in BLKS:
                h2a = trans.tile([128, 512], BF16, tag="h2a", bufs=2)
                nc.sync.dma_start(h2a[:, :bl], dar_out[mo, :, ds(bs, bl)])
                nc.vector.tensor_add(h_main[:, mo, ds(bs, bl)],
                                     h_main[:, mo, ds(bs, bl)], h2a[:, :bl])

        # ---- LN2 + MLP ----
        z0 = pers.tile([128, KO, TOKP], BF16, tag="ln_out")
        mr2 = _emit_ln(nc, pers, trans, psum, h_main, z0, onec, oner16, eps_sb)

        for mg in range(3):  # stream W_fc in thirds of the output dim
            wfc_sb = trans.tile([128, KO, 256], BF16, tag="wbig", bufs=2)
            nc.sync.dma_start(out=wfc_sb[:],
                              in_=pin(wfc_d)[:, :, ds(mg * 256, 256)])
            for mi in range(2):
                mo = mg * 2 + mi
                ps_g = psum.tile([128, 1536], F32, tag="big3", bufs=2)
                for bs, bl in BLKS:
                    for o in range(KO):
                        nc.tensor.matmul(
                            ps_g[:, ds(bs, bl)], wfc_sb[:, o, ts(mi, 128)],
                            z0[:, o, ds(bs, bl)],
                            start=(o == 0), stop=False,
                        )
                    nc.tensor.matmul(
                        ps_g[:, ds(bs, bl)],
                        crow[0:1, ds(3 * FG + mo * 128, 128)],
                        mr2[0:1, ds(bs, bl)], start=False, stop=True,
                    )
                nc.scalar.activation(gl[:, mo, :], ps_g[:, :TOKP], gelu_fn,
                                     bias=bfc[:, mo:mo + 1])

        drs_in = dram.tile([KO, 128, TOKP], BF16)
        drs_out = dram.tile([FG, TOKP], BF16)
        for mg in range(3):
            wmp_sb = trans.tile([128, KO, 256], BF16, tag="wbig", bufs=2)
            nc.sync.dma_start(out=wmp_sb[:],
                              in_=pin(wmp_d)[:, :, ds(mg * 256, 256)])
            for mi in range(2):
                mo = mg * 2 + mi
                ps_mp = psum.tile([128, 1536], F32, tag="big3", bufs=2)
                for bs, bl in BLKS:
                    for o in range(KO):
                        nc.tensor.matmul(
                            ps_mp[:, ds(bs, bl)], wmp_sb[:, o, ts(mi, 128)],
                            gl[:, o, ds(bs, bl)],
                            start=(o == 0), stop=False,
                        )
                    nc.tensor.matmul(
                        ps_mp[:, ds(bs, bl)], bmp16_r[0:1, ts(mo, 128)],
                        oner16[0:1, :bl], start=False, stop=True,
                    )
                mpart = trans.tile([128, TOKP], BF16, tag="mpart", bufs=2)
                # mpart = h_main/4 + (mproj partial + b_mproj/4)
                nc.vector.scalar_tensor_tensor(
                    out=mpart[:], in0=h_main[:, mo, :], scalar=0.25,
                    in1=ps_mp[:, :TOKP], op0=ALU.mult, op1=ALU.add)
                nc.sync.dma_start(drs_in[mo], mpart[:])
        if sim_collectives:
            nc.gpsimd.collective_compute(
                "ReduceScatter", ALU.add, replica_groups=GROUPS,
                ins=[drs_in.opt()], outs=[drs_out.opt()],
            )
        else:
            nc.gpsimd.dma_start(drs_out[0:128, :], drs_in[0, :, :])
            nc.gpsimd.dma_start(drs_out[128:FG, :], drs_in[1, 0:64, :])
        nc.sync.dma_start(out_d[:], drs_out[:])
        if debug_taps:
            for nm, t in [("dbg_hl", hl), ("dbg_q01", q01), ("dbg_kf01", kf01),
                          ("dbg_vsb", vsb), ("dbg_a0", a_sb[0]),
                          ("dbg_h", h_main), ("dbg_z0", z0), ("dbg_gl", gl)]:
                dt_ = t.dtype
                shp = list(t.shape)
                d = nc.dram_tensor(nm, shp, dt_, kind="ExternalOutput")
                nc.sync.dma_start(d[:], t[:])

    nc.compile()
    return nc


_NC_CACHE = None


def _get_program():
    global _NC_CACHE
    if _NC_CACHE is None:
        _NC_CACHE = build_program()
    return _NC_CACHE


def make_in_maps(inputs):
    f = lambda a: np.asarray(a, dtype=np.float32)
    x = f(inputs["x"])
    context_seq = f(inputs["context_seq"])
    sos_h = f(inputs["sos_h"])
    g1, b1 = f(inputs["ln1_g"]), f(inputs["ln1_b"])
    W_attn, b_attn = f(inputs["W_attn"]), f(inputs["b_attn"])
    W_ref, b_ref = f(inputs["W_ref"]), f(inputs["b_ref"])
    W_proj, b_proj = f(inputs["W_proj"]), f(inputs["b_proj"])
    g2, b2 = f(inputs["ln2_g"]), f(inputs["ln2_b"])
    W_fc, b_fc = f(inputs["W_fc"]), f(inputs["b_fc"])
    W_mproj, b_mproj = f(inputs["W_mproj"]), f(inputs["b_mproj"])

    # master causal mask: mask[p, c] = 1 iff p <= c - 384
    cix = np.arange(896)[None, :]
    pix = np.arange(128)[:, None]
    mask = (pix <= cix - 384).astype(np.float32)
    onec = np.ones((128, 1), np.float32)
    oner = np.ones((1, 512), np.float32)

    wfc_g = W_fc * g2[:, None]
    bfc_full = b2 @ W_fc + b_fc

    in_maps = []
    for core in range(N_CORES):
        b, g = core // TP, core % TP
        h = np.concatenate([sos_h[None, :], x[b]], axis=0)  # [1025, 768]
        hT = np.zeros((D, TOKP), ml_dtypes.bfloat16)
        hT[:, :TOK] = h.T.astype(ml_dtypes.bfloat16)
        qsl = slice(FG * g, FG * (g + 1))
        ksl = slice(D + FG * g, D + FG * (g + 1))
        vsl = slice(2 * D + FG * g, 2 * D + FG * (g + 1))
        rks = slice(FG * g, FG * (g + 1))
        rvs = slice(D + FG * g, D + FG * (g + 1))
        mcols = slice(D * g, D * (g + 1))    # W_fc column slice (768 per core)
        wp_slab = W_proj[FG * g:FG * (g + 1), :]
        bf = ml_dtypes.bfloat16
        in_maps.append({
            "hT": hT,
            "ctxT": np.ascontiguousarray(context_seq[b].T).astype(bf),
            "wq": np.ascontiguousarray(W_attn[:, qsl] * g1[:, None]).astype(bf),
            "wk": np.ascontiguousarray(W_attn[:, ksl] * g1[:, None]).astype(bf),
            "wv": np.ascontiguousarray(W_attn[:, vsl] * g1[:, None]).astype(bf),
            "wrk": np.ascontiguousarray(W_ref[:, rks]).astype(bf),
            "wrv": np.ascontiguousarray(W_ref[:, rvs]).astype(bf),
            "bq": (b1 @ W_attn[:, qsl] + b_attn[qsl]).reshape(FG, 1),
            "bk": (b1 @ W_attn[:, ksl] + b_attn[ksl]).reshape(FG, 1),
            "brk": b_ref[rks].reshape(FG, 1),
            "bv_r": (b1 @ W_attn[:, vsl] + b_attn[vsl]).reshape(1, FG).astype(bf),
            "brv_r": b_ref[rvs].reshape(1, FG).astype(bf),
            "wp0": np.ascontiguousarray(wp_slab[0:64, :]).astype(bf),
            "wp1": np.ascontiguousarray(wp_slab[64:128, :]).astype(bf),
            "wp2": np.ascontiguousarray(wp_slab[128:192, :]).astype(bf),
            "bp4": (b_proj / TP).reshape(D, 1),
            "wfc": np.ascontiguousarray(wfc_g[:, mcols]).astype(bf),
            "bfc": bfc_full[mcols].reshape(D, 1),
            "wmp": np.ascontiguousarray(W_mproj[mcols, :]).astype(bf),
            "bmp16_r": (b_mproj / TP).reshape(1, D).astype(bf),
            "mask": mask.astype(bf),
            "crow": np.concatenate([
                -(W_attn[:, qsl] * g1[:, None]).sum(0),
                -(W_attn[:, ksl] * g1[:, None]).sum(0),
                -(W_attn[:, vsl] * g1[:, None]).sum(0),
                -wfc_g[:, mcols].sum(0),
            ]).reshape(1, 1344).astype(bf),
            "onec": onec.astype(bf),
            "oner": oner,
            "oner16": oner.astype(bf),
        })
    return in_maps


def assemble_output(results, B=2):
    out = np.empty((B, S, D), np.float32)
    for b in range(B):
        parts = [np.asarray(results[TP * b + g]["out"], np.float32)
                 for g in range(TP)]
        full = np.concatenate(parts, axis=0)  # [768, 1152]
        out[b] = full[:, 1:TOK].T
    return out


def kernel(**inputs):
    nc = _get_program()
    in_maps = make_in_maps(inputs)
    res = run_bass_kernel_spmd(nc, in_maps, list(range(N_CORES)))
    return assemble_output(res.results, B=np.asarray(inputs["x"]).shape[0])


if __name__ == "__main__":
    import reference
    ins = reference.setup_inputs()
    ins = {k: np.asarray(v) for k, v in ins.items()}
    got = kernel(**ins)
    exp = np.asarray(reference.reference(**ins))
    err = np.abs(got - exp).max() / np.abs(exp).max()
    print("max abs err:", np.abs(got - exp).max(), "rel:", err)



# revision 12
# speedup vs baseline: 1.4956x; 1.4956x over previous
"""CoconBlock forward on 8 Trainium2 NeuronCores.

Sharding: core c = (b, g) with b = c // 4 (batch), g = c % 4 (tensor-parallel
rank). Within each batch group of 4 cores:
  - attention QKV / context-KV weights column-sharded by head group (3 heads),
  - W_proj row-sharded, partial outputs AllReduced per token block,
  - MLP W_fc column-sharded / W_mproj row-sharded, partial outputs (with the
    residual and bias pre-folded as +h/4 + b_mproj/4 per core) ReduceScattered
    so each core lands exactly its 192-feature slice of the final output.

All on-device activations are feature-on-partition (f32 has no DMA transpose;
this layout makes every matmul transpose-free). Token axis is processed in
blocks (0,512),(512,512),(1024,1) -- no padded-token compute. LayerNorm
reductions over the feature (partition) axis run on the PE via a [128,128]
ones matmul that leaves the statistics replicated across all partitions (PE
matmul cost depends only on the output free size, so replication is free);
the normalize multiplies are then cheap all-SBUF bf16 DVE ops and no
broadcast matmul is needed. The LN affine (gamma/beta) is folded into the
following weight matrix on the host; the -mean*rstd term is folded into the
consuming matmuls as a rank-1 correction (colsum(W) x mr).

Engine balance: every PSUM->SBUF move that carries a per-partition bias runs
on ACT (Identity with bias AP); exp/gelu/sqrt/square on ACT; masks, rstd
reciprocal, residual scalar_tensor_tensor folds on DVE.

Attention uses the 128-aligned causal structure: with keys (256 context +
1025 self) padded to 1408, a (query-block, key-tile) pair is fully allowed,
partially masked by a shifted-triangle slice of one master mask, or skipped.
Scores for a query block are exp'ed in up-to-2-key-tile groups out of a
[128,1024] PSUM slab; the softmax denominator rides the attend matmul as a
ones-column appended to V (partition 64), and 1/den is broadcast to
partitions 0..63 with a K=1 PE outer product.

The attention output projection and its AllReduce run per token block,
pipelined against the remaining attention blocks; the residual is pre-folded
(h/4 per core) so the AllReduce result IS the new h, DMAed straight back
into h_main.
"""

import sys

sys.path.insert(0, "/opt/trn_rl_repo")

import ml_dtypes
import numpy as np

import concourse.bass as bass
import concourse.bacc as bacc
import concourse.mybir as mybir
import concourse.tile as tile
from concourse.bass_utils import run_bass_kernel_spmd

F32 = mybir.dt.float32
AF = mybir.ActivationFunctionType
ALU = mybir.AluOpType
ts, ds = bass.ts, bass.ds

D = 768
DH = 64
S = 1024
SC = 256
TOK = S + 1            # 1025 (sos + x)
TOKP = 1152            # tile column capacity (only 0..1024 computed)
KEYSP = 1408           # 11 * 128
NK = KEYSP // 128      # 11
KO = D // 128          # 6 feature sub-tiles
TP = 4
FG = 192               # features per core in head-sharded tensors (3 heads)
HG = 3                 # heads per core
EPS = 1e-5
N_CORES = 8
GROUPS = [[0, 1, 2, 3], [4, 5, 6, 7]]
BLKS = [(0, 512), (512, 512), (1024, 1)]  # token blocks (start, len)

BF16 = mybir.dt.bfloat16


def build_program(sim_collectives=True, gelu_fn=None, debug_taps=False):
    if gelu_fn is None:
        gelu_fn = AF.Gelu_apprx_tanh
    nc = bacc.Bacc(None, num_devices=N_CORES)

    # ---- DRAM I/O ----
    hT_d = nc.dram_tensor("hT", [D, TOKP], BF16, kind="ExternalInput")
    ctxT_d = nc.dram_tensor("ctxT", [D, SC], BF16, kind="ExternalInput")
    wqkv_d = nc.dram_tensor("wqkv", [D, 5 * FG], BF16, kind="ExternalInput")
    wp_d = nc.dram_tensor("wp", [DH, 3 * D], BF16, kind="ExternalInput")
    wfc_d = nc.dram_tensor("wfc", [D, D], BF16, kind="ExternalInput")
    wmp_d = nc.dram_tensor("wmp", [D, D], BF16, kind="ExternalInput")
    cst_d = nc.dram_tensor("cst", [128, 25], F32, kind="ExternalInput")
    rows16_d = nc.dram_tensor("rows16", [1, 1856], BF16, kind="ExternalInput")
    maskp_d = nc.dram_tensor("maskp", [128, 1088], BF16, kind="ExternalInput")
    out_d = nc.dram_tensor("out", [FG, TOK], BF16, kind="ExternalOutput")

    with tile.TileContext(nc) as tc, \
         tc.tile_pool(name="pers", bufs=1) as pers, \
         tc.tile_pool(name="trans", bufs=2) as trans, \
         tc.tile_pool(name="psum", bufs=1, space="PSUM") as psum, \
         tc.tile_pool(name="dram", bufs=1, space="DRAM") as dram:

        # ---- resident SBUF tensors ----
        h_main = pers.tile([128, KO, TOKP], BF16, tag="h_main")
        ctx = pers.tile([128, KO, SC], BF16, tag="ctx")
        wqkv = pers.tile([128, KO, 5 * FG], BF16, tag="wqkv")
        wp_sb = pers.tile([DH, HG, D], BF16, tag="wp")
        wfc_sb = pers.tile([128, KO, D], BF16, tag="wfc")
        wmp_sb = pers.tile([128, KO, D], BF16, tag="wmp")
        kf01 = pers.tile([128, KEYSP], BF16, tag="kf01")
        kf2 = pers.tile([64, KEYSP], BF16, tag="kf2")
        q01 = pers.tile([128, TOKP], BF16, tag="q01")
        q2 = pers.tile([64, TOKP], BF16, tag="q2")
        vsb = pers.tile([128, NK, HG, DH + 1], BF16, tag="vsb")
        a_sb = [pers.tile([DH, TOKP], BF16, tag=f"a{h}", name=f"a{h}")
                for h in range(HG)]
        gl = pers.tile([128, KO, TOKP], BF16, tag="gl")
        maskp = pers.tile([128, 1088], BF16, tag="maskp")
        mask_sb = maskp[:, 0:896]
        onesq = maskp[:, 896:1024]          # [128,128] ones (LN stats lhsT)
        ones64 = maskp[64:65, 1024:1088]    # [1,64] ones at partition 64
        cst = pers.tile([128, 25], F32, tag="cst")
        bq01, bq2 = cst[:, 0:1], cst[0:64, 1:2]
        bk01, bk2 = cst[:, 2:3], cst[0:64, 3:4]
        brk01, brk2 = cst[:, 4:5], cst[0:64, 5:6]
        bp4, bfc, bmp4 = cst[:, 6:12], cst[:, 12:18], cst[:, 18:24]
        eps_c = cst[:, 24:25]
        rows16 = pers.tile([1, 1856], BF16, tag="rows16")
        bv_r, brv_r = rows16[:, 0:FG], rows16[:, FG:2 * FG]
        oner16 = rows16[:, 384:512]
        crow = rows16[:, 512:1856]
        mr1h = pers.tile([1, TOKP], BF16, tag="mr1h")
        mr2h = pers.tile([1, TOKP], BF16, tag="mr2h")

        # ---- input DMAs (hT per block first so LN1 starts early) ----
        pin = lambda t: t.rearrange("(o p) n -> p o n", p=128)
        for bs, bl in BLKS:
            nc.sync.dma_start(out=h_main[:, :, ds(bs, bl)],
                              in_=pin(hT_d)[:, :, ds(bs, bl)])
        nc.sync.dma_start(out=maskp[:], in_=maskp_d[:])
        nc.sync.dma_start(out=cst[:], in_=cst_d[:])
        nc.sync.dma_start(out=rows16[:], in_=rows16_d[:])
        nc.sync.dma_start(out=ctx[:], in_=pin(ctxT_d))
        nc.sync.dma_start(out=wqkv[:], in_=pin(wqkv_d))
        nc.sync.dma_start(out=wp_sb[:], in_=wp_d.rearrange("p (h n) -> p h n",
                                                           h=HG))
        nc.sync.dma_start(out=wfc_sb[:], in_=pin(wfc_d))
        nc.sync.dma_start(out=wmp_sb[:], in_=pin(wmp_d))
        nc.vector.memset(kf01[:, ds(SC + TOK, KEYSP - SC - TOK)], 0.0)
        nc.vector.memset(kf2[:, ds(SC + TOK, KEYSP - SC - TOK)], 0.0)
        nc.vector.memset(vsb[:, NK - 1, :, :], 0.0)
        nc.vector.memset(vsb[:, :, :, DH:DH + 1], 1.0)

        # ---- LayerNorm (normalize only; affine folded into next weights) ---
        def emit_ln(src, dst, mr_b16):
            """dst = src * rsqrt(var+eps); mr rows = mean*rstd (for the rank-1
            -mean correction in consuming matmuls). Stats land replicated on
            all 128 partitions via the onesq matmul, so everything downstream
            is a cheap elementwise op."""
            for bs, bl in BLKS:
                sl = ds(bs, bl)
                ps_m = psum.tile([128, 512], F32, tag="pa", bufs=2)
                ps_s = psum.tile([128, 512], F32, tag="pj", bufs=2)
                for o in range(KO):
                    sq = trans.tile([128, 512], BF16, tag="sq", bufs=2)
                    nc.vector.tensor_mul(sq[:, :bl], src[:, o, sl],
                                         src[:, o, sl])
                    nc.tensor.matmul(ps_m[:, :bl], onesq, src[:, o, sl],
                                     start=(o == 0), stop=(o == KO - 1))
                    nc.tensor.matmul(ps_s[:, :bl], onesq, sq[:, :bl],
                                     start=(o == 0), stop=(o == KO - 1))
                m2 = trans.tile([128, 512], F32, tag="m2", bufs=2)
                nc.scalar.activation(m2[:, :bl], ps_m[:, :bl], AF.Square,
                                     scale=1.0 / D)
                var = trans.tile([128, 512], F32, tag="var", bufs=2)
                nc.vector.scalar_tensor_tensor(
                    out=var[:, :bl], in0=ps_s[:, :bl], scalar=1.0 / D,
                    in1=m2[:, :bl], op0=ALU.mult, op1=ALU.subtract)
                sd = trans.tile([128, 512], F32, tag="sd", bufs=2)
                nc.scalar.activation(sd[:, :bl], var[:, :bl], AF.Sqrt,
                                     bias=eps_c)
                rstd = trans.tile([128, 512], BF16, tag="rstd", bufs=2)
                with nc.allow_low_precision(reason="bf16 rstd: ~0.4% scale"):
                    nc.vector.reciprocal(rstd[:, :bl], sd[:, :bl])
                nc.vector.scalar_tensor_tensor(
                    out=mr_b16[:, sl], in0=ps_m[0:1, :bl], scalar=1.0 / D,
                    in1=rstd[0:1, :bl], op0=ALU.mult, op1=ALU.mult)
                for o in range(KO):
                    nc.vector.tensor_mul(dst[:, o, sl], src[:, o, sl],
                                         rstd[:, :bl])

        hl = pers.tile([128, KO, TOKP], BF16, tag="ln_out")
        emit_ln(h_main, hl, mr1h)

        # ---- QKV projections (feature-major q/k; token-major v) ----
        def qk_proj(wcol, b01, b2, dst01, dst2, dst_off, src, src_blks,
                    corr, mr):
            # dst[mi] = w[:, :, mi].T @ src + bias, written at dst col offset
            for dst_t, bias_t, m0, ml in [(dst01, b01, 0, 128),
                                          (dst2, b2, 128, 64)]:
                for bs, bl in src_blks:
                    ps = psum.tile([128, 512], F32, tag="pa", bufs=2)
                    for o in range(KO):
                        nc.tensor.matmul(
                            ps[:ml, :bl], wqkv[:, o, ds(wcol + m0, ml)],
                            src[:, o, ds(bs, bl)],
                            start=(o == 0),
                            stop=(corr is None and o == KO - 1),
                        )
                    if corr is not None:
                        nc.tensor.matmul(
                            ps[:ml, :bl], crow[0:1, ds(corr + m0, ml)],
                            mr[0:1, ds(bs, bl)], start=False, stop=True,
                        )
                    nc.scalar.activation(
                        dst_t[:ml, ds(dst_off + bs, bl)], ps[:ml, :bl],
                        AF.Identity, bias=bias_t)

        def v_proj(wcol, b_row, src, n_tiles, kt_base, corr):
            # V[token, feat] = src.T @ w + 1 (x) bias  (token-major output)
            for tt in range(n_tiles):
                tw = 1 if kt_base + tt == NK - 1 else 128  # real tokens
                tsl = ds(tt * 128, tw)
                ps = psum.tile([128, 512], F32, tag="pj", bufs=2)
                for o in range(KO):
                    nc.tensor.matmul(
                        ps[:tw, :FG], src[:, o, tsl], wqkv[:, o,
                                                           ds(wcol, FG)],
                        start=(o == 0), stop=False,
                    )
                if corr is not None:
                    nc.tensor.matmul(ps[:tw, :FG], mr1h[0:1, tsl],
                                     crow[0:1, ds(corr, FG)],
                                     start=False, stop=False)
                nc.tensor.matmul(ps[:tw, :FG], oner16[0:1, 0:tw], b_row,
                                 start=False, stop=True)
                nc.vector.tensor_copy(
                    vsb[:tw, kt_base + tt, :, 0:DH],
                    ps[:tw, 0:FG].rearrange("p (h d) -> p h d", h=HG))

        # q: all tokens; k(self): keys 256..1280; kc: keys 0..255
        qk_proj(0, bq01, bq2, q01, q2, 0, hl, BLKS, 0, mr1h)
        qk_proj(FG, bk01, bk2, kf01, kf2, SC, hl, BLKS, FG, mr1h)
        qk_proj(3 * FG, brk01, brk2, kf01, kf2, 0, ctx, [(0, 256)],
                None, None)
        v_proj(4 * FG, brv_r, ctx, 2, 0, None)     # context V -> key tiles 0,1
        v_proj(2 * FG, bv_r, hl, NK - 2, 2, 2 * FG)

        # ---- attention + per-block proj/AllReduce pipeline ----
        kf_of = [(kf01, 0), (kf01, 64), (kf2, 0)]
        q_of = [(q01, 0), (q01, 64), (q2, 0)]
        dar_in, dar_out = [], []
        for bi, (bs, bl) in enumerate(BLKS):
            dar_in.append(dram.tile([128, KO, bl], BF16,
                                    name=f"dar_in{bi}"))
            dar_out.append(dram.tile([128, KO, bl], BF16,
                                     name=f"dar_out{bi}"))

        for bi, (qs, ql) in enumerate(BLKS):
            qb0 = qs // 128                      # first 128-query tile
            last_kt = min((qs + ql - 1) // 128 + 2, NK - 1)
            gsz = 2 if ql > 1 else NK            # key tiles per exp group
            for h in range(HG):
                kf_t, kf_o = kf_of[h]
                q_t, q_o = q_of[h]
                ps_a = psum.tile([128, 512], F32, tag="pa", bufs=2,
                                 name="ps_a")[0:DH + 1]
                for ktg in range(0, last_kt + 1, gsz):
                    kts = range(ktg, min(ktg + gsz, last_kt + 1))
                    ps_s = psum.tile([128, 1024], F32, tag="sc", bufs=2,
                                     name="ps_s")
                    for j, kt in enumerate(kts):
                        nc.tensor.matmul(
                            ps_s[:, ds(j * ql, ql)],
                            kf_t[kf_o:kf_o + DH, ts(kt, 128)],
                            q_t[q_o:q_o + DH, ds(qs, ql)],
                            start=True, stop=True,
                        )
                    expt = trans.tile([128, 1024], BF16, tag="expt", bufs=3)
                    nw = len(kts) * ql
                    nc.scalar.activation(expt[:, :nw], ps_s[:, :nw], AF.Exp,
                                         scale=0.125)
                    for j, kt in enumerate(kts):
                        if kt >= qb0 + 2:  # shifted-triangle mask
                            dlt = kt - qb0 - 2
                            nc.vector.tensor_mul(
                                expt[:, ds(j * ql, ql)],
                                expt[:, ds(j * ql, ql)],
                                mask_sb[:, ds(384 - 128 * dlt, ql)])
                        nc.tensor.matmul(
                            ps_a[:, :ql], vsb[:, kt, h, :],
                            expt[:, ds(j * ql, ql)],
                            start=(kt == 0), stop=(kt == last_kt),
                        )
                # normalize: a = num * (1/den); den sits at partition 64,
                # broadcast to partitions 0..63 via a K=1 PE outer product
                r16 = trans.tile([128, 512], BF16, tag="r16", bufs=2)
                with nc.allow_low_precision(reason="bf16 1/den: ~0.4%"):
                    nc.vector.reciprocal(r16[DH:DH + 1, :ql],
                                         ps_a[DH:DH + 1, :ql])
                ps_r = psum.tile([128, 512], F32, tag="pj", bufs=2,
                                 name="ps_r")[0:DH]
                nc.tensor.matmul(ps_r[:, :ql], ones64,
                                 r16[DH:DH + 1, :ql], start=True, stop=True)
                recs = trans.tile([128, 512], BF16, tag="recs", bufs=2)
                nc.scalar.activation(recs[0:DH, :ql], ps_r[:, :ql],
                                     AF.Identity)
                nc.vector.tensor_mul(a_sb[h][:, ds(qs, ql)],
                                     ps_a[0:DH, :ql], recs[0:DH, :ql])

            # attention output projection for this token block (row-sharded);
            # residual pre-folded as +h/4 so the AllReduce output IS new h.
            armo = trans.tile([128, KO, 512], BF16, tag="armo", bufs=2)
            for mo in range(KO):
                ps = psum.tile([128, 512], F32, tag="pj", bufs=2)
                for h in range(HG):
                    nc.tensor.matmul(ps[:, :ql], wp_sb[:, h, ts(mo, 128)],
                                     a_sb[h][:, ds(qs, ql)],
                                     start=(h == 0), stop=(h == HG - 1))
                ptmp = trans.tile([128, 512], BF16, tag="ptmp", bufs=2)
                nc.scalar.activation(ptmp[:, :ql], ps[:, :ql], AF.Identity,
                                     bias=bp4[:, mo:mo + 1])
                nc.vector.scalar_tensor_tensor(
                    out=armo[:, mo, :ql], in0=h_main[:, mo, ds(qs, ql)],
                    scalar=0.25, in1=ptmp[:, :ql], op0=ALU.mult, op1=ALU.add)
            nc.sync.dma_start(dar_in[bi][:], armo[:, :, :ql])
            if sim_collectives:
                nc.gpsimd.collective_compute(
                    "AllReduce", ALU.add, replica_groups=GROUPS,
                    ins=[dar_in[bi].opt()], outs=[dar_out[bi].opt()],
                )
            else:
                nc.gpsimd.dma_start(dar_out[bi][:], dar_in[bi][:])
            # h_main <- allreduced (h + attn out), in place
            nc.gpsimd.dma_start(h_main[:, :, ds(qs, ql)], dar_out[bi][:])

        # ---- LN2 + MLP ----
        z0 = pers.tile([128, KO, TOKP], BF16, tag="ln_out")
        emit_ln(h_main, z0, mr2h)

        for bs, bl in BLKS:
            for mo in range(KO):
                ps = psum.tile([128, 512], F32, tag="pa", bufs=2)
                for o in range(KO):
                    nc.tensor.matmul(
                        ps[:, :bl], wfc_sb[:, o, ts(mo, 128)],
                        z0[:, o, ds(bs, bl)],
                        start=(o == 0), stop=False,
                    )
                nc.tensor.matmul(
                    ps[:, :bl], crow[0:1, ds(3 * FG + mo * 128, 128)],
                    mr2h[0:1, ds(bs, bl)], start=False, stop=True,
                )
                nc.scalar.activation(gl[:, mo, ds(bs, bl)], ps[:, :bl],
                                     gelu_fn, bias=bfc[:, mo:mo + 1])

        drs_in = [dram.tile([KO, 128, bl], BF16, name=f"drs_in{i}")
                  for i, (_, bl) in enumerate(BLKS)]
        drs_out = [dram.tile([FG, bl], BF16, name=f"drs_out{i}")
                   for i, (_, bl) in enumerate(BLKS)]
        for bi, (bs, bl) in enumerate(BLKS):
            mpart = trans.tile([128, KO, 512], BF16, tag="armo", bufs=2)
            for mo in range(KO):
                ps = psum.tile([128, 512], F32, tag="pj", bufs=2)
                for o in range(KO):
                    nc.tensor.matmul(
                        ps[:, :bl], wmp_sb[:, o, ts(mo, 128)],
                        gl[:, o, ds(bs, bl)],
                        start=(o == 0), stop=(o == KO - 1),
                    )
                mtmp = trans.tile([128, 512], BF16, tag="ptmp", bufs=2)
                nc.scalar.activation(mtmp[:, :bl], ps[:, :bl], AF.Identity,
                                     bias=bmp4[:, mo:mo + 1])
                # mpart = h_main/4 + (mproj partial + b_mproj/4)
                nc.vector.scalar_tensor_tensor(
                    out=mpart[:, mo, :bl], in0=h_main[:, mo, ds(bs, bl)],
                    scalar=0.25, in1=mtmp[:, :bl], op0=ALU.mult, op1=ALU.add)
            nc.sync.dma_start(
                drs_in[bi].rearrange("o p n -> p o n"), mpart[:, :, :bl])
            if sim_collectives:
                nc.gpsimd.collective_compute(
                    "ReduceScatter", ALU.add, replica_groups=GROUPS,
                    ins=[drs_in[bi].opt()], outs=[drs_out[bi].opt()],
                )
                nc.sync.dma_start(out_d[:, ds(bs, bl)], drs_out[bi][:])
            else:
                nc.gpsimd.dma_start(out_d[0:128, ds(bs, bl)],
                                    drs_in[bi][0])
                nc.gpsimd.dma_start(out_d[128:FG, ds(bs, bl)],
                                    drs_in[bi][1, 0:64])
        if debug_taps:
            for nm, t in [("dbg_hl", hl), ("dbg_q01", q01),
                          ("dbg_kf01", kf01), ("dbg_vsb", vsb),
                          ("dbg_a0", a_sb[0]), ("dbg_h", h_main),
                          ("dbg_z0", z0), ("dbg_gl", gl)]:
                d = nc.dram_tensor(nm, list(t.shape), t.dtype,
                                   kind="ExternalOutput")
                nc.sync.dma_start(d[:], t[:])

    nc.compile()
    return nc


_NC_CACHE = None


def _get_program():
    global _NC_CACHE
    if _NC_CACHE is None:
        _NC_CACHE = build_program()
    return _NC_CACHE


def make_in_maps(inputs):
    f = lambda a: np.asarray(a, dtype=np.float32)
    bf = ml_dtypes.bfloat16
    x = f(inputs["x"])
    context_seq = f(inputs["context_seq"])
    sos_h = f(inputs["sos_h"])
    g1, b1 = f(inputs["ln1_g"]), f(inputs["ln1_b"])
    W_attn, b_attn = f(inputs["W_attn"]), f(inputs["b_attn"])
    W_ref, b_ref = f(inputs["W_ref"]), f(inputs["b_ref"])
    W_proj, b_proj = f(inputs["W_proj"]), f(inputs["b_proj"])
    g2, b2 = f(inputs["ln2_g"]), f(inputs["ln2_b"])
    W_fc, b_fc = f(inputs["W_fc"]), f(inputs["b_fc"])
    W_mproj, b_mproj = f(inputs["W_mproj"]), f(inputs["b_mproj"])

    # master causal mask: mask[p, c] = 1 iff p <= c - 384
    cix = np.arange(896)[None, :]
    pix = np.arange(128)[:, None]
    mask = (pix <= cix - 384).astype(np.float32)
    maskp = np.ones((128, 1088), np.float32)
    maskp[:, 0:896] = mask

    wfc_g = W_fc * g2[:, None]
    bfc_full = b2 @ W_fc + b_fc

    in_maps = []
    for core in range(N_CORES):
        b, g = core // TP, core % TP
        h = np.concatenate([sos_h[None, :], x[b]], axis=0)  # [1025, 768]
        hT = np.zeros((D, TOKP), bf)
        hT[:, :TOK] = h.T.astype(bf)
        qsl = slice(FG * g, FG * (g + 1))
        ksl = slice(D + FG * g, D + FG * (g + 1))
        vsl = slice(2 * D + FG * g, 2 * D + FG * (g + 1))
        rks = slice(FG * g, FG * (g + 1))
        rvs = slice(D + FG * g, D + FG * (g + 1))
        mcols = slice(D * g, D * (g + 1))    # W_fc column slice (768 per core)
        wq = W_attn[:, qsl] * g1[:, None]
        wk = W_attn[:, ksl] * g1[:, None]
        wv = W_attn[:, vsl] * g1[:, None]
        wqkv = np.concatenate([wq, wk, wv, W_ref[:, rks], W_ref[:, rvs]],
                              axis=1)
        wp_slab = W_proj[FG * g:FG * (g + 1), :]   # [192, 768]
        wp = np.concatenate([wp_slab[0:64], wp_slab[64:128],
                             wp_slab[128:192]], axis=1)  # [64, 2304]
        cst = np.zeros((128, 25), np.float32)
        bq = b1 @ W_attn[:, qsl] + b_attn[qsl]
        bk = b1 @ W_attn[:, ksl] + b_attn[ksl]
        cst[:, 0] = bq[0:128]
        cst[0:64, 1] = bq[128:192]
        cst[:, 2] = bk[0:128]
        cst[0:64, 3] = bk[128:192]
        cst[:, 4] = b_ref[rks][0:128]
        cst[0:64, 5] = b_ref[rks][128:192]
        cst[:, 6:12] = (b_proj / TP).reshape(6, 128).T
        cst[:, 12:18] = bfc_full[mcols].reshape(6, 128).T
        cst[:, 18:24] = (b_mproj / TP).reshape(6, 128).T
        cst[:, 24] = EPS
        rows16 = np.zeros((1, 1856), np.float32)
        rows16[0, 0:FG] = b1 @ W_attn[:, vsl] + b_attn[vsl]
        rows16[0, FG:2 * FG] = b_ref[rvs]
        rows16[0, 384:512] = 1.0
        rows16[0, 512:1856] = np.concatenate([
            -wq.sum(0), -wk.sum(0), -wv.sum(0), -wfc_g[:, mcols].sum(0)])
        in_maps.append({
            "hT": hT,
            "ctxT": np.ascontiguousarray(context_seq[b].T).astype(bf),
            "wqkv": np.ascontiguousarray(wqkv).astype(bf),
            "wp": np.ascontiguousarray(wp).astype(bf),
            "wfc": np.ascontiguousarray(wfc_g[:, mcols]).astype(bf),
            "wmp": np.ascontiguousarray(W_mproj[mcols, :]).astype(bf),
            "cst": cst,
            "rows16": rows16.astype(bf),
            "maskp": maskp.astype(bf),
        })
    return in_maps


def assemble_output(results, B=2):
    out = np.empty((B, S, D), np.float32)
    for b in range(B):
        parts = [np.asarray(results[TP * b + g]["out"], np.float32)
                 for g in range(TP)]
        full = np.concatenate(parts, axis=0)  # [768, 1025]
        out[b] = full[:, 1:TOK].T
    return out


def kernel(**inputs):
    nc = _get_program()
    in_maps = make_in_maps(inputs)
    res = run_bass_kernel_spmd(nc, in_maps, list(range(N_CORES)))
    return assemble_output(res.results, B=np.asarray(inputs["x"]).shape[0])


if __name__ == "__main__":
    import reference
    ins = reference.setup_inputs()
    ins = {k: np.asarray(v) for k, v in ins.items()}
    got = kernel(**ins)
    exp = np.asarray(reference.reference(**ins))
    err = np.abs(got - exp).max() / np.abs(exp).max()
    print("max abs err:", np.abs(got - exp).max(), "rel:", err)


# revision 32
# speedup vs baseline: 1.7168x; 1.1479x over previous
"""CoconBlock forward on 8 Trainium2 NeuronCores.

Sharding: core c = (b, g) with b = c // 4 (batch), g = c % 4 (tensor-parallel
rank). Within each batch group of 4 cores:
  - attention QKV / context-KV weights column-sharded by head group (3 heads),
  - W_proj row-sharded, partial outputs AllReduced per token block,
  - MLP W_fc column-sharded / W_mproj row-sharded, partial outputs (with the
    residual and bias pre-folded as +h/4 + b_mproj/4 per core) ReduceScattered
    so each core lands exactly its 192-feature slice of the final output.

All on-device activations are feature-on-partition (f32 has no DMA transpose;
this layout makes every matmul transpose-free). Token axis is processed in
blocks (0,512),(512,512),(1024,1) -- no padded-token compute. LayerNorm
reductions over the feature (partition) axis run on the PE via a [128,128]
ones matmul that leaves the statistics replicated across all partitions (PE
matmul cost depends only on the output free size, so replication is free);
the normalize multiplies are then cheap all-SBUF bf16 DVE ops and no
broadcast matmul is needed. The LN affine (gamma/beta) is folded into the
following weight matrix on the host; the -mean*rstd term is folded into the
consuming matmuls as a rank-1 correction (colsum(W) x mr).

Engine balance: every PSUM->SBUF move that carries a per-partition bias runs
on ACT (Identity with bias AP); exp/gelu/sqrt/square on ACT; masks, rstd
reciprocal, residual scalar_tensor_tensor folds on DVE.

Attention uses the 128-aligned causal structure: with keys (256 context +
1025 self) padded to 1408, a (query-block, key-tile) pair is fully allowed,
partially masked by a shifted-triangle slice of one master mask, or skipped.
Scores for a query block are exp'ed in up-to-2-key-tile groups out of a
[128,1024] PSUM slab; the softmax denominator rides the attend matmul as a
ones-column appended to V (partition 64), and 1/den is broadcast to
partitions 0..63 with a K=1 PE outer product.

The attention output projection and its AllReduce run per token block,
pipelined against the remaining attention blocks; the residual is pre-folded
(h/4 per core) so the AllReduce result IS the new h, DMAed straight back
into h_main.
"""

import sys

sys.path.insert(0, "/opt/trn_rl_repo")

import ml_dtypes
import numpy as np

import concourse.bass as bass
import concourse.bacc as bacc
import concourse.mybir as mybir
import concourse.tile as tile
from concourse.bass_utils import run_bass_kernel_spmd

F32 = mybir.dt.float32
AF = mybir.ActivationFunctionType
ALU = mybir.AluOpType
ts, ds = bass.ts, bass.ds

D = 768
DH = 64
S = 1024
SC = 256
TOK = S + 1            # 1025 (sos + x)
TOKP = 1152            # tile column capacity (only 0..1024 computed)
KEYSP = 1408           # 11 * 128
NK = KEYSP // 128      # 11
KO = D // 128          # 6 feature sub-tiles
TP = 4
FG = 192               # features per core in head-sharded tensors (3 heads)
HG = 3                 # heads per core
EPS = 1e-5
N_CORES = 8
GROUPS = [[0, 1, 2, 3], [4, 5, 6, 7]]
BLKS = [(0, 512), (512, 512), (1024, 1)]  # token blocks (start, len)

BF16 = mybir.dt.bfloat16


def build_program(sim_collectives=True, gelu_fn=None, debug_taps=False):
    if gelu_fn is None:
        gelu_fn = AF.Gelu_apprx_tanh
    nc = bacc.Bacc(None, num_devices=N_CORES)

    # ---- DRAM I/O ----
    hT_d = nc.dram_tensor("hT", [D, TOKP], BF16, kind="ExternalInput")
    ctxT_d = nc.dram_tensor("ctxT", [D, SC], BF16, kind="ExternalInput")
    wqkv_d = nc.dram_tensor("wqkv", [D, 5 * FG], BF16, kind="ExternalInput")
    wp_d = nc.dram_tensor("wp", [DH, 3 * D], BF16, kind="ExternalInput")
    wfc_d = nc.dram_tensor("wfc", [D, D], BF16, kind="ExternalInput")
    wmp_d = nc.dram_tensor("wmp", [D, D], BF16, kind="ExternalInput")
    cst_d = nc.dram_tensor("cst", [128, 25], F32, kind="ExternalInput")
    rows16_d = nc.dram_tensor("rows16", [1, 1856], BF16, kind="ExternalInput")
    maskp_d = nc.dram_tensor("maskp", [128, 1088], BF16, kind="ExternalInput")
    out_d = nc.dram_tensor("out", [FG, TOK], BF16, kind="ExternalOutput")

    with tile.TileContext(nc) as tc, \
         tc.tile_pool(name="pers", bufs=1) as pers, \
         tc.tile_pool(name="trans", bufs=2) as trans, \
         tc.tile_pool(name="psum", bufs=1, space="PSUM") as psum, \
         tc.tile_pool(name="dram", bufs=1, space="DRAM") as dram:

        # ---- resident SBUF tensors ----
        h_main = pers.tile([128, KO, TOKP], BF16, tag="h_main")
        ctx = pers.tile([128, KO, SC], BF16, tag="ctx")
        wqkv = pers.tile([128, KO, 5 * FG], BF16, tag="wqkv")
        wp_sb = pers.tile([DH, HG, D], BF16, tag="wp")
        wfc_sb = pers.tile([128, KO, D], BF16, tag="wfc")
        wmp_sb = pers.tile([128, KO, D], BF16, tag="wmp")
        kf01 = pers.tile([128, KEYSP], BF16, tag="kf01")
        kf2 = pers.tile([64, KEYSP], BF16, tag="kf2")
        q01 = pers.tile([128, TOKP], BF16, tag="q01")
        q2 = pers.tile([64, TOKP], BF16, tag="q2")
        vsb = pers.tile([128, NK, HG, DH + 1], BF16, tag="vsb")
        a_sb = [pers.tile([DH, TOKP], BF16, tag=f"a{h}", name=f"a{h}")
                for h in range(HG)]
        gl = pers.tile([128, KO, TOKP], BF16, tag="gl")
        maskp = pers.tile([128, 1088], BF16, tag="maskp")
        mask_sb = maskp[:, 0:896]
        onesq = maskp[:, 896:1024]          # [128,128] ones (LN stats lhsT)
        ones64 = maskp[64:65, 1024:1088]    # [1,64] ones at partition 64
        cst = pers.tile([128, 25], F32, tag="cst")
        bq01, bq2 = cst[:, 0:1], cst[0:64, 1:2]
        bk01, bk2 = cst[:, 2:3], cst[0:64, 3:4]
        brk01, brk2 = cst[:, 4:5], cst[0:64, 5:6]
        bp4, bfc, bmp4 = cst[:, 6:12], cst[:, 12:18], cst[:, 18:24]
        eps_c = cst[:, 24:25]
        rows16 = pers.tile([1, 1856], BF16, tag="rows16")
        bv_r, brv_r = rows16[:, 0:FG], rows16[:, FG:2 * FG]
        oner16 = rows16[:, 384:512]
        crow = rows16[:, 512:1856]
        mr1h = pers.tile([1, TOKP], BF16, tag="mr1h")
        mr2h = pers.tile([1, TOKP], BF16, tag="mr2h")

        # ---- input DMAs (mask consts + hT block 0 first: LN1 starts early) -
        pin = lambda t: t.rearrange("(o p) n -> p o n", p=128)
        nc.sync.dma_start(out=maskp[:], in_=maskp_d[:])
        nc.sync.dma_start(out=h_main[:, 0:3, ds(0, 512)],
                          in_=pin(hT_d)[:, 0:3, ds(0, 512)])
        nc.sync.dma_start(out=h_main[:, 3:6, ds(0, 512)],
                          in_=pin(hT_d)[:, 3:6, ds(0, 512)])
        nc.sync.dma_start(out=cst[:], in_=cst_d[:])
        for bs, bl in BLKS[1:]:
            nc.sync.dma_start(out=h_main[:, :, ds(bs, bl)],
                              in_=pin(hT_d)[:, :, ds(bs, bl)])
        nc.sync.dma_start(out=rows16[:], in_=rows16_d[:])
        nc.sync.dma_start(out=ctx[:], in_=pin(ctxT_d))
        nc.sync.dma_start(out=wqkv[:], in_=pin(wqkv_d))
        nc.sync.dma_start(out=wp_sb[:], in_=wp_d.rearrange("p (h n) -> p h n",
                                                           h=HG))
        nc.sync.dma_start(out=wfc_sb[:], in_=pin(wfc_d))
        nc.sync.dma_start(out=wmp_sb[:], in_=pin(wmp_d))
        nc.vector.memset(kf01[:, ds(SC + TOK, KEYSP - SC - TOK)], 0.0)
        nc.vector.memset(kf2[:, ds(SC + TOK, KEYSP - SC - TOK)], 0.0)
        nc.vector.memset(vsb[:, NK - 1, :, :], 0.0)
        nc.vector.memset(vsb[:, :, :, DH:DH + 1], 1.0)

        # ---- LayerNorm (normalize only; affine folded into next weights).
        # dst = src * rsqrt(var+eps); mr row = mean*rstd (for the rank-1
        # -mean correction in consuming matmuls). Stats land replicated on
        # all 128 partitions via the onesq matmul, so everything downstream
        # is a cheap elementwise op.
        def ln_stats(src, bs, bl):
            sl = ds(bs, bl)
            ps_m = psum.tile([128, 512], F32, tag="pa", bufs=2)
            ps_s = psum.tile([128, 512], F32, tag="pj", bufs=2)
            for o in range(KO):
                sq = trans.tile([128, 512], BF16, tag="sq", bufs=3)
                nc.vector.tensor_mul(sq[:, :bl], src[:, o, sl],
                                     src[:, o, sl])
                nc.tensor.matmul(ps_m[:, :bl], onesq, src[:, o, sl],
                                 start=(o == 0), stop=(o == KO - 1))
                nc.tensor.matmul(ps_s[:, :bl], onesq, sq[:, :bl],
                                 start=(o == 0), stop=(o == KO - 1))
            return ps_m, ps_s

        def ln_chain(src, dst, mr_b16, bs, bl, ps_m, ps_s):
            sl = ds(bs, bl)
            # the mean row leaves PSUM first so ps_m recycles early
            mrow = trans.tile([1, 512], F32, tag="mrow", bufs=2)
            nc.scalar.activation(mrow[:, :bl], ps_m[0:1, :bl], AF.Identity,
                                 scale=1.0 / D)
            m2 = trans.tile([128, 512], F32, tag="m2", bufs=2)
            nc.scalar.activation(m2[:, :bl], ps_m[:, :bl], AF.Square,
                                 scale=1.0 / D)
            var = trans.tile([128, 512], F32, tag="var", bufs=2)
            nc.vector.scalar_tensor_tensor(
                out=var[:, :bl], in0=ps_s[:, :bl], scalar=1.0 / D,
                in1=m2[:, :bl], op0=ALU.mult, op1=ALU.subtract)
            sd = trans.tile([128, 512], F32, tag="sd", bufs=2)
            nc.scalar.activation(sd[:, :bl], var[:, :bl], AF.Sqrt,
                                 bias=eps_c)
            rstd = trans.tile([128, 512], BF16, tag="rstd", bufs=2)
            with nc.allow_low_precision(reason="bf16 rstd: ~0.4% scale"):
                nc.vector.reciprocal(rstd[:, :bl], sd[:, :bl])
            nc.vector.tensor_mul(mr_b16[:, sl], mrow[:, :bl],
                                 rstd[0:1, :bl])
            for o in range(KO):
                nc.vector.tensor_mul(dst[:, o, sl], src[:, o, sl],
                                     rstd[:, :bl])

        # stats b0, b1 fill the 2-deep psum pools; each chain is emitted
        # right before the allocation that recycles its stats buffers
        hl = pers.tile([128, KO, TOKP], BF16, tag="ln_out")
        s0 = ln_stats(h_main, *BLKS[0])
        s1 = ln_stats(h_main, *BLKS[1])
        ln_chain(h_main, hl, mr1h, *BLKS[0], *s0)
        s2 = ln_stats(h_main, *BLKS[2])
        ln_chain(h_main, hl, mr1h, *BLKS[1], *s1)
        ln_chain(h_main, hl, mr1h, *BLKS[2], *s2)

        # ---- QKV projections (feature-major q/k; token-major v) ----
        def qk_proj(wcol, b01, b2, dst01, dst2, dst_off, src, src_blks,
                    corr, mr):
            # dst[mi] = w[:, :, mi].T @ src + bias, written at dst col offset
            for dst_t, bias_t, m0, ml in [(dst01, b01, 0, 128),
                                          (dst2, b2, 128, 64)]:
                for bs, bl in src_blks:
                    ps = psum.tile([128, 512], F32, tag="pa", bufs=2)
                    for o in range(KO):
                        nc.tensor.matmul(
                            ps[:ml, :bl], wqkv[:, o, ds(wcol + m0, ml)],
                            src[:, o, ds(bs, bl)],
                            start=(o == 0),
                            stop=(corr is None and o == KO - 1),
                        )
                    if corr is not None:
                        nc.tensor.matmul(
                            ps[:ml, :bl], crow[0:1, ds(corr + m0, ml)],
                            mr[0:1, ds(bs, bl)], start=False, stop=True,
                        )
                    nc.scalar.activation(
                        dst_t[:ml, ds(dst_off + bs, bl)], ps[:ml, :bl],
                        AF.Identity, bias=bias_t)

        def v_proj(wcol, b_row, src, n_tiles, kt_base, corr):
            # V[token, feat] = src.T @ w + 1 (x) bias  (token-major output)
            for tt in range(n_tiles):
                tw = 1 if kt_base + tt == NK - 1 else 128  # real tokens
                tsl = ds(tt * 128, tw)
                ps = psum.tile([128, 512], F32, tag="pj", bufs=2)
                for o in range(KO):
                    nc.tensor.matmul(
                        ps[:tw, :FG], src[:, o, tsl], wqkv[:, o,
                                                           ds(wcol, FG)],
                        start=(o == 0), stop=False,
                    )
                if corr is not None:
                    nc.tensor.matmul(ps[:tw, :FG], mr1h[0:1, tsl],
                                     crow[0:1, ds(corr, FG)],
                                     start=False, stop=False)
                nc.tensor.matmul(ps[:tw, :FG], oner16[0:1, 0:tw], b_row,
                                 start=False, stop=True)
                nc.vector.tensor_copy(
                    vsb[:tw, kt_base + tt, :, 0:DH],
                    ps[:tw, 0:FG].rearrange("p (h d) -> p h d", h=HG))

        # q: all tokens; k(self): keys 256..1280; kc: keys 0..255
        qk_proj(0, bq01, bq2, q01, q2, 0, hl, BLKS, 0, mr1h)
        qk_proj(FG, bk01, bk2, kf01, kf2, SC, hl, BLKS, FG, mr1h)
        qk_proj(3 * FG, brk01, brk2, kf01, kf2, 0, ctx, [(0, 256)],
                None, None)
        v_proj(4 * FG, brv_r, ctx, 2, 0, None)     # context V -> key tiles 0,1
        v_proj(2 * FG, bv_r, hl, NK - 2, 2, 2 * FG)

        # ---- attention + per-block proj/AllReduce pipeline ----
        kf_of = [(kf01, 0), (kf01, 64), (kf2, 0)]
        q_of = [(q01, 0), (q01, 64), (q2, 0)]
        dar_in, dar_out = [], []
        for bi, (bs, bl) in enumerate(BLKS):
            dar_in.append(dram.tile([128, KO, bl], BF16,
                                    name=f"dar_in{bi}"))
            dar_out.append(dram.tile([128, KO, bl], BF16,
                                     name=f"dar_out{bi}"))

        for bi, (qs, ql) in enumerate(BLKS):
            qb0 = qs // 128                      # first 128-query tile
            last_kt = min((qs + ql - 1) // 128 + 2, NK - 1)
            # per key tile: column offset of the first not-fully-masked query
            # (query qs+c attends key j iff j <= 256 + qs + c, so tile kt is
            # all-masked for c < 128*(kt-qb0-2) and triangular for the next
            # 128 columns -- those columns are simply not computed, which is
            # exactly equivalent to multiplying by 0)
            def col0(kt):
                return min(max(0, 128 * (kt - qb0 - 2)), ql)
            # greedy-pack key-tile strips into [128,1024] PSUM groups
            strips = [(kt, col0(kt), ql - col0(kt))
                      for kt in range(last_kt + 1) if ql - col0(kt) > 0]
            groups, off = [], 0
            for kt, co, w in strips:
                if not groups or off + w > 1024:
                    groups.append([])
                    off = 0
                groups[-1].append((kt, co, w, off))
                off += w

            def head_scores(h, ps_a):
                kf_t, kf_o = kf_of[h]
                q_t, q_o = q_of[h]
                for g in groups:
                    ps_s = psum.tile([128, 1024], F32, tag="sc", bufs=2,
                                     name="ps_s")
                    for kt, co, w, off in g:
                        nc.tensor.matmul(
                            ps_s[:, ds(off, w)],
                            kf_t[kf_o:kf_o + DH, ts(kt, 128)],
                            q_t[q_o:q_o + DH, ds(qs + co, w)],
                            start=True, stop=True,
                        )
                    expt = trans.tile([128, 1024], BF16, tag="expt", bufs=3)
                    nw = g[-1][3] + g[-1][2]  # last strip's off + w
                    nc.scalar.activation(expt[:, :nw], ps_s[:, :nw], AF.Exp,
                                         scale=0.125)
                    for kt, co, w, off in g:
                        if kt >= qb0 + 2:
                            # triangle starts at strip col 128*dlt - co
                            dlt = kt - qb0 - 2
                            t0 = 128 * dlt - co
                            tw = min(128, w - t0)
                            nc.vector.tensor_mul(
                                expt[:, ds(off + t0, tw)],
                                expt[:, ds(off + t0, tw)],
                                mask_sb[:, ds(384, tw)])
                            if t0 > 0:  # fully-masked cols below the triangle
                                nc.vector.tensor_mul(
                                    expt[:, ds(off, t0)],
                                    expt[:, ds(off, t0)],
                                    mask_sb[:, ds(0, t0)])
                        nc.tensor.matmul(
                            ps_a[:, ds(co, w)], vsb[:, kt, h, :],
                            expt[:, ds(off, w)],
                            start=(kt == 0), stop=(kt == last_kt),
                        )

            def head_norm(h, ps_a):
                # a = num * (1/den); den sits at partition 64, broadcast to
                # partitions 0..63 via a K=1 PE outer product
                r16 = trans.tile([128, 512], BF16, tag="r16", bufs=2)
                with nc.allow_low_precision(reason="bf16 1/den: ~0.4%"):
                    nc.vector.reciprocal(r16[DH:DH + 1, :ql],
                                         ps_a[DH:DH + 1, :ql])
                ps_r = psum.tile([128, 512], F32, tag="pj", bufs=2,
                                 name="ps_r")[0:DH]
                nc.tensor.matmul(ps_r[:, :ql], ones64,
                                 r16[DH:DH + 1, :ql], start=True, stop=True)
                recs = trans.tile([128, 512], BF16, tag="recs", bufs=2)
                nc.scalar.activation(recs[0:DH, :ql], ps_r[:, :ql],
                                     AF.Identity)
                nc.vector.tensor_mul(a_sb[h][:, ds(qs, ql)],
                                     ps_a[0:DH, :ql], recs[0:DH, :ql])

            if ql > 1:
                for h in range(HG):
                    ps_a = psum.tile([128, 512], F32, tag="pa", bufs=2,
                                     name="ps_a")[0:DH + 1]
                    head_scores(h, ps_a)
                    head_norm(h, ps_a)
            else:
                for h in range(HG):
                    ps_a = psum.tile([128, 512], F32, tag="pa", bufs=2,
                                     name="ps_a")[0:DH + 1]
                    head_scores(h, ps_a)
                    head_norm(h, ps_a)

            # attention output projection for this token block (row-sharded);
            # residual pre-folded as +h/4 so the AllReduce output IS new h.
            # high_priority: this chain feeds the AllReduce that LN2 waits on,
            # so its ACT/DVE ops must not queue behind the next block's exps.
            armo = trans.tile([128, KO, 512], BF16, tag="armo", bufs=2)
            with tc.high_priority():
                for mo in range(KO):
                    ps = psum.tile([128, 512], F32, tag="pj", bufs=2)
                    for h in range(HG):
                        nc.tensor.matmul(ps[:, :ql],
                                         wp_sb[:, h, ts(mo, 128)],
                                         a_sb[h][:, ds(qs, ql)],
                                         start=(h == 0), stop=(h == HG - 1))
                    ptmp = trans.tile([128, 512], BF16, tag="ptmp", bufs=2)
                    nc.scalar.activation(ptmp[:, :ql], ps[:, :ql],
                                         AF.Identity, bias=bp4[:, mo:mo + 1])
                    nc.vector.scalar_tensor_tensor(
                        out=armo[:, mo, :ql], in0=h_main[:, mo, ds(qs, ql)],
                        scalar=0.25, in1=ptmp[:, :ql], op0=ALU.mult,
                        op1=ALU.add)
                nc.sync.dma_start(dar_in[bi][:], armo[:, :, :ql])
            if sim_collectives:
                nc.gpsimd.collective_compute(
                    "AllReduce", ALU.add, replica_groups=GROUPS,
                    ins=[dar_in[bi].opt()], outs=[dar_out[bi].opt()],
                )
                # h_main <- allreduced (h + attn out), in place
                nc.gpsimd.dma_start(h_main[:, :, ds(qs, ql)], dar_out[bi][:])
            else:
                # local stand-in: the payload makes the same SBUF->DRAM->SBUF
                # round trip a real AllReduce result would
                nc.gpsimd.dma_start(h_main[:, :, ds(qs, ql)], dar_in[bi][:])

        # ---- LN2 + MLP ----
        z0 = pers.tile([128, KO, TOKP], BF16, tag="ln_out")
        t0 = ln_stats(h_main, *BLKS[0])
        t1 = ln_stats(h_main, *BLKS[1])
        ln_chain(h_main, z0, mr2h, *BLKS[0], *t0)
        t2 = ln_stats(h_main, *BLKS[2])
        ln_chain(h_main, z0, mr2h, *BLKS[1], *t1)
        ln_chain(h_main, z0, mr2h, *BLKS[2], *t2)

        def fc_block(bs, bl):
            for mo in range(KO):
                ps = psum.tile([128, 512], F32, tag="pa", bufs=2)
                for o in range(KO):
                    nc.tensor.matmul(
                        ps[:, :bl], wfc_sb[:, o, ts(mo, 128)],
                        z0[:, o, ds(bs, bl)],
                        start=(o == 0), stop=False,
                    )
                nc.tensor.matmul(
                    ps[:, :bl], crow[0:1, ds(3 * FG + mo * 128, 128)],
                    mr2h[0:1, ds(bs, bl)], start=False, stop=True,
                )
                nc.scalar.activation(gl[:, mo, ds(bs, bl)], ps[:, :bl],
                                     gelu_fn, bias=bfc[:, mo:mo + 1])

        drs_in = [dram.tile([KO, 128, bl], BF16, name=f"drs_in{i}")
                  for i, (_, bl) in enumerate(BLKS)]
        drs_out = [dram.tile([FG, bl], BF16, name=f"drs_out{i}")
                   for i, (_, bl) in enumerate(BLKS)]

        def mp_block(bi):
            bs, bl = BLKS[bi]
            mpart = trans.tile([128, KO, 512], BF16, tag="armo", bufs=2)
            for mo in range(KO):
                ps = psum.tile([128, 512], F32, tag="pj", bufs=2)
                for o in range(KO):
                    nc.tensor.matmul(
                        ps[:, :bl], wmp_sb[:, o, ts(mo, 128)],
                        gl[:, o, ds(bs, bl)],
                        start=(o == 0), stop=(o == KO - 1),
                    )
                mtmp = trans.tile([128, 512], BF16, tag="ptmp", bufs=2)
                with tc.high_priority():
                    nc.scalar.activation(mtmp[:, :bl], ps[:, :bl],
                                         AF.Identity,
                                         bias=bmp4[:, mo:mo + 1])
                    # mpart = h_main/4 + (mproj partial + b_mproj/4)
                    nc.vector.scalar_tensor_tensor(
                        out=mpart[:, mo, :bl], in0=h_main[:, mo, ds(bs, bl)],
                        scalar=0.25, in1=mtmp[:, :bl], op0=ALU.mult,
                        op1=ALU.add)
            if sim_collectives:
                nc.sync.dma_start(
                    drs_in[bi].rearrange("o p n -> p o n"), mpart[:, :, :bl])
                nc.gpsimd.collective_compute(
                    "ReduceScatter", ALU.add, replica_groups=GROUPS,
                    ins=[drs_in[bi].opt()], outs=[drs_out[bi].opt()],
                )
                nc.sync.dma_start(out_d[:, ds(bs, bl)], drs_out[bi][:])
            else:
                # local stand-in for ReduceScatter: this core's output slice
                # comes straight from SBUF (a real RS also lands in out_d)
                nc.gpsimd.dma_start(out_d[0:128, ds(bs, bl)],
                                    mpart[:, 0, :bl])
                nc.gpsimd.dma_start(out_d[128:FG, ds(bs, bl)],
                                    mpart[0:64, 1, :bl])

        for bi in range(3):
            fc_block(*BLKS[bi])
            mp_block(bi)

        if debug_taps:
            for nm, t in [("dbg_hl", hl), ("dbg_q01", q01),
                          ("dbg_kf01", kf01), ("dbg_vsb", vsb),
                          ("dbg_a0", a_sb[0]), ("dbg_a1", a_sb[1]),
                          ("dbg_a2", a_sb[2]), ("dbg_h", h_main),
                          ("dbg_z0", z0), ("dbg_gl", gl),
                          ("dbg_dar_in0", dar_in[0]),
                          ("dbg_dar_out0", dar_out[0])]:
                d = nc.dram_tensor(nm, list(t.shape), t.dtype,
                                   kind="ExternalOutput")
                nc.sync.dma_start(d[:], t[:])

    nc.compile()
    return nc


_NC_CACHE = None


def _get_program():
    global _NC_CACHE
    if _NC_CACHE is None:
        _NC_CACHE = build_program()
    return _NC_CACHE


def make_in_maps(inputs):
    f = lambda a: np.asarray(a, dtype=np.float32)
    bf = ml_dtypes.bfloat16
    x = f(inputs["x"])
    context_seq = f(inputs["context_seq"])
    sos_h = f(inputs["sos_h"])
    g1, b1 = f(inputs["ln1_g"]), f(inputs["ln1_b"])
    W_attn, b_attn = f(inputs["W_attn"]), f(inputs["b_attn"])
    W_ref, b_ref = f(inputs["W_ref"]), f(inputs["b_ref"])
    W_proj, b_proj = f(inputs["W_proj"]), f(inputs["b_proj"])
    g2, b2 = f(inputs["ln2_g"]), f(inputs["ln2_b"])
    W_fc, b_fc = f(inputs["W_fc"]), f(inputs["b_fc"])
    W_mproj, b_mproj = f(inputs["W_mproj"]), f(inputs["b_mproj"])

    # master causal mask: mask[p, c] = 1 iff p <= c - 384
    cix = np.arange(896)[None, :]
    pix = np.arange(128)[:, None]
    mask = (pix <= cix - 384).astype(np.float32)
    maskp = np.ones((128, 1088), np.float32)
    maskp[:, 0:896] = mask

    wfc_g = W_fc * g2[:, None]
    bfc_full = b2 @ W_fc + b_fc

    in_maps = []
    for core in range(N_CORES):
        b, g = core // TP, core % TP
        h = np.concatenate([sos_h[None, :], x[b]], axis=0)  # [1025, 768]
        hT = np.zeros((D, TOKP), bf)
        hT[:, :TOK] = h.T.astype(bf)
        qsl = slice(FG * g, FG * (g + 1))
        ksl = slice(D + FG * g, D + FG * (g + 1))
        vsl = slice(2 * D + FG * g, 2 * D + FG * (g + 1))
        rks = slice(FG * g, FG * (g + 1))
        rvs = slice(D + FG * g, D + FG * (g + 1))
        mcols = slice(D * g, D * (g + 1))    # W_fc column slice (768 per core)
        wq = W_attn[:, qsl] * g1[:, None]
        wk = W_attn[:, ksl] * g1[:, None]
        wv = W_attn[:, vsl] * g1[:, None]
        wqkv = np.concatenate([wq, wk, wv, W_ref[:, rks], W_ref[:, rvs]],
                              axis=1)
        wp_slab = W_proj[FG * g:FG * (g + 1), :]   # [192, 768]
        wp = np.concatenate([wp_slab[0:64], wp_slab[64:128],
                             wp_slab[128:192]], axis=1)  # [64, 2304]
        cst = np.zeros((128, 25), np.float32)
        bq = b1 @ W_attn[:, qsl] + b_attn[qsl]
        bk = b1 @ W_attn[:, ksl] + b_attn[ksl]
        cst[:, 0] = bq[0:128]
        cst[0:64, 1] = bq[128:192]
        cst[:, 2] = bk[0:128]
        cst[0:64, 3] = bk[128:192]
        cst[:, 4] = b_ref[rks][0:128]
        cst[0:64, 5] = b_ref[rks][128:192]
        cst[:, 6:12] = (b_proj / TP).reshape(6, 128).T
        cst[:, 12:18] = bfc_full[mcols].reshape(6, 128).T
        cst[:, 18:24] = (b_mproj / TP).reshape(6, 128).T
        cst[:, 24] = EPS
        rows16 = np.zeros((1, 1856), np.float32)
        rows16[0, 0:FG] = b1 @ W_attn[:, vsl] + b_attn[vsl]
        rows16[0, FG:2 * FG] = b_ref[rvs]
        rows16[0, 384:512] = 1.0
        rows16[0, 512:1856] = np.concatenate([
            -wq.sum(0), -wk.sum(0), -wv.sum(0), -wfc_g[:, mcols].sum(0)])
        in_maps.append({
            "hT": hT,
            "ctxT": np.ascontiguousarray(context_seq[b].T).astype(bf),
            "wqkv": np.ascontiguousarray(wqkv).astype(bf),
            "wp": np.ascontiguousarray(wp).astype(bf),
            "wfc": np.ascontiguousarray(wfc_g[:, mcols]).astype(bf),
            "wmp": np.ascontiguousarray(W_mproj[mcols, :]).astype(bf),
            "cst": cst,
            "rows16": rows16.astype(bf),
            "maskp": maskp.astype(bf),
        })
    return in_maps


def assemble_output(results, B=2):
    out = np.empty((B, S, D), np.float32)
    for b in range(B):
        parts = [np.asarray(results[TP * b + g]["out"], np.float32)
                 for g in range(TP)]
        full = np.concatenate(parts, axis=0)  # [768, 1025]
        out[b] = full[:, 1:TOK].T
    return out


def kernel(**inputs):
    nc = _get_program()
    in_maps = make_in_maps(inputs)
    res = run_bass_kernel_spmd(nc, in_maps, list(range(N_CORES)))
    return assemble_output(res.results, B=np.asarray(inputs["x"]).shape[0])


if __name__ == "__main__":
    import reference
    ins = reference.setup_inputs()
    ins = {k: np.asarray(v) for k, v in ins.items()}
    got = kernel(**ins)
    exp = np.asarray(reference.reference(**ins))
    err = np.abs(got - exp).max() / np.abs(exp).max()
    print("max abs err:", np.abs(got - exp).max(), "rel:", err)


# revision 38
# speedup vs baseline: 1.8097x; 1.0541x over previous
"""CoconBlock forward on 8 Trainium2 NeuronCores.

Sharding: core c = (b, g) with b = c // 4 (batch), g = c % 4 (tensor-parallel
rank). Within each batch group of 4 cores:
  - attention QKV / context-KV weights column-sharded by head group (3 heads),
  - W_proj row-sharded, partial outputs AllReduced per token block,
  - MLP W_fc column-sharded / W_mproj row-sharded, partial outputs (with the
    residual and bias pre-folded as +h/4 + b_mproj/4 per core) ReduceScattered
    so each core lands exactly its 192-feature slice of the final output.

All on-device activations are feature-on-partition (f32 has no DMA transpose;
this layout makes every matmul transpose-free). Token axis is processed in
blocks (0,512),(512,512),(1024,1) -- no padded-token compute. LayerNorm
reductions over the feature (partition) axis run on the PE via a [128,128]
ones matmul that leaves the statistics replicated across all partitions (PE
matmul cost depends only on the output free size, so replication is free);
the normalize multiplies are then cheap all-SBUF bf16 DVE ops and no
broadcast matmul is needed. The LN affine (gamma/beta) is folded into the
following weight matrix on the host; the -mean*rstd term is folded into the
consuming matmuls as a rank-1 correction (colsum(W) x mr).

Engine balance: every PSUM->SBUF move that carries a per-partition bias runs
on ACT (Identity with bias AP); exp/gelu/sqrt/square on ACT; masks, rstd
reciprocal, residual scalar_tensor_tensor folds on DVE.

Attention uses the 128-aligned causal structure: with keys (256 context +
1025 self) padded to 1408, a (query-block, key-tile) pair is fully allowed,
partially masked by a shifted-triangle slice of one master mask, or skipped.
Scores for a query block are exp'ed in up-to-2-key-tile groups out of a
[128,1024] PSUM slab; the softmax denominator rides the attend matmul as a
ones-column appended to V (partition 64), and 1/den is broadcast to
partitions 0..63 with a K=1 PE outer product.

The attention output projection and its AllReduce run per token block,
pipelined against the remaining attention blocks; the residual is pre-folded
(h/4 per core) so the AllReduce result IS the new h, DMAed straight back
into h_main.
"""

import sys

sys.path.insert(0, "/opt/trn_rl_repo")

import ml_dtypes
import numpy as np

import concourse.bass as bass
import concourse.bacc as bacc
import concourse.mybir as mybir
import concourse.tile as tile
from concourse.bass_utils import run_bass_kernel_spmd

F32 = mybir.dt.float32
AF = mybir.ActivationFunctionType
ALU = mybir.AluOpType
ts, ds = bass.ts, bass.ds

D = 768
DH = 64
S = 1024
SC = 256
TOK = S + 1            # 1025 (sos + x)
TOKP = 1152            # tile column capacity (only 0..1024 computed)
KEYSP = 1408           # 11 * 128
NK = KEYSP // 128      # 11
KO = D // 128          # 6 feature sub-tiles
TP = 4
FG = 192               # features per core in head-sharded tensors (3 heads)
HG = 3                 # heads per core
EPS = 1e-5
N_CORES = 8
GROUPS = [[0, 1, 2, 3], [4, 5, 6, 7]]
BLKS = [(0, 512), (512, 512), (1024, 1)]  # token blocks (start, len)

BF16 = mybir.dt.bfloat16
F8 = mybir.dt.float8e4
DR = mybir.MatmulPerfMode.DoubleRow


def build_program(sim_collectives=True, gelu_fn=None, debug_taps=False):
    if gelu_fn is None:
        gelu_fn = AF.Gelu_apprx_tanh
    nc = bacc.Bacc(None, num_devices=N_CORES)

    # ---- DRAM I/O ----
    hT_d = nc.dram_tensor("hT", [D, TOKP], BF16, kind="ExternalInput")
    ctxT_d = nc.dram_tensor("ctxT", [D, SC], F8, kind="ExternalInput")
    wqkv_d = nc.dram_tensor("wqkv", [D, 5 * FG], F8, kind="ExternalInput")
    wp_d = nc.dram_tensor("wp", [DH, 3 * D], F8, kind="ExternalInput")
    wfc_d = nc.dram_tensor("wfc", [D, D], BF16, kind="ExternalInput")
    wmp_d = nc.dram_tensor("wmp", [D, D], BF16, kind="ExternalInput")
    cst_d = nc.dram_tensor("cst", [128, 25], F32, kind="ExternalInput")
    rows16_d = nc.dram_tensor("rows16", [1, 1856], BF16, kind="ExternalInput")
    maskp_d = nc.dram_tensor("maskp", [128, 1088], BF16, kind="ExternalInput")
    out_d = nc.dram_tensor("out", [FG, TOK], BF16, kind="ExternalOutput")

    with tile.TileContext(nc) as tc, \
         tc.tile_pool(name="pers", bufs=1) as pers, \
         tc.tile_pool(name="trans", bufs=2) as trans, \
         tc.tile_pool(name="psum", bufs=1, space="PSUM") as psum, \
         tc.tile_pool(name="dram", bufs=1, space="DRAM") as dram:

        # ---- resident SBUF tensors ----
        h_main = pers.tile([128, KO, TOKP], BF16, tag="h_main")
        ctx = pers.tile([128, KO, SC], F8, tag="ctx")
        wqkv = pers.tile([128, KO, 5 * FG], F8, tag="wqkv")
        wp_sb = pers.tile([DH, HG, D], F8, tag="wp")
        wfc_sb = pers.tile([128, KO, D], BF16, tag="wfc")
        wmp_sb = pers.tile([128, KO, D], BF16, tag="wmp")
        kf01 = pers.tile([128, KEYSP], BF16, tag="kf01")
        kf2 = pers.tile([64, KEYSP], BF16, tag="kf2")
        q01 = pers.tile([128, TOKP], BF16, tag="q01")
        q2 = pers.tile([64, TOKP], BF16, tag="q2")
        vsb = pers.tile([128, NK, HG, DH + 1], BF16, tag="vsb")
        a_all = pers.tile([DH, HG, TOKP], F8, tag="a_all")
        gl = pers.tile([128, KO, TOKP], BF16, tag="gl")
        maskp = pers.tile([128, 1088], BF16, tag="maskp")
        mask_sb = maskp[:, 0:896]
        onesq = maskp[:, 896:1024]          # [128,128] ones (LN stats lhsT)
        ones64 = maskp[64:65, 1024:1088]    # [1,64] ones at partition 64
        cst = pers.tile([128, 25], F32, tag="cst")
        bq01, bq2 = cst[:, 0:1], cst[0:64, 1:2]
        bk01, bk2 = cst[:, 2:3], cst[0:64, 3:4]
        brk01, brk2 = cst[:, 4:5], cst[0:64, 5:6]
        bp4, bfc, bmp4 = cst[:, 6:12], cst[:, 12:18], cst[:, 18:24]
        eps_c = cst[:, 24:25]
        rows16 = pers.tile([1, 1856], BF16, tag="rows16")
        bv_r, brv_r = rows16[:, 0:FG], rows16[:, FG:2 * FG]
        oner16 = rows16[:, 384:512]
        crow = rows16[:, 512:1856]
        mr1h = pers.tile([1, TOKP], BF16, tag="mr1h")
        mr2h = pers.tile([1, TOKP], BF16, tag="mr2h")

        # ---- input DMAs (mask consts + hT block 0 first: LN1 starts early) -
        pin = lambda t: t.rearrange("(o p) n -> p o n", p=128)
        nc.sync.dma_start(out=maskp[:], in_=maskp_d[:])
        nc.sync.dma_start(out=h_main[:, 0:3, ds(0, 512)],
                          in_=pin(hT_d)[:, 0:3, ds(0, 512)])
        nc.sync.dma_start(out=h_main[:, 3:6, ds(0, 512)],
                          in_=pin(hT_d)[:, 3:6, ds(0, 512)])
        nc.sync.dma_start(out=cst[:], in_=cst_d[:])
        for bs, bl in BLKS[1:]:
            nc.sync.dma_start(out=h_main[:, :, ds(bs, bl)],
                              in_=pin(hT_d)[:, :, ds(bs, bl)])
        nc.sync.dma_start(out=rows16[:], in_=rows16_d[:])
        nc.sync.dma_start(out=ctx[:], in_=pin(ctxT_d))
        nc.sync.dma_start(out=wqkv[:], in_=pin(wqkv_d))
        nc.sync.dma_start(out=wp_sb[:], in_=wp_d.rearrange("p (h n) -> p h n",
                                                           h=HG))
        nc.sync.dma_start(out=wfc_sb[:], in_=pin(wfc_d))
        nc.sync.dma_start(out=wmp_sb[:], in_=pin(wmp_d))
        nc.vector.memset(kf01[:, ds(SC + TOK, KEYSP - SC - TOK)], 0.0)
        nc.vector.memset(kf2[:, ds(SC + TOK, KEYSP - SC - TOK)], 0.0)
        nc.vector.memset(vsb[:, NK - 1, :, :], 0.0)
        nc.vector.memset(vsb[:, :, :, DH:DH + 1], 1.0)

        # ---- LayerNorm (normalize only; affine folded into next weights).
        # dst = src * rsqrt(var+eps); mr row = mean*rstd (for the rank-1
        # -mean correction in consuming matmuls). Stats land replicated on
        # all 128 partitions via the onesq matmul, so everything downstream
        # is a cheap elementwise op.
        def ln_stats(src, bs, bl):
            sl = ds(bs, bl)
            ps_m = psum.tile([128, 512], F32, tag="pa", bufs=2)
            ps_s = psum.tile([128, 512], F32, tag="pj", bufs=2)
            for o in range(KO):
                sq = trans.tile([128, 512], BF16, tag="sq", bufs=3)
                nc.vector.tensor_mul(sq[:, :bl], src[:, o, sl],
                                     src[:, o, sl])
                nc.tensor.matmul(ps_m[:, :bl], onesq, src[:, o, sl],
                                 start=(o == 0), stop=(o == KO - 1))
                nc.tensor.matmul(ps_s[:, :bl], onesq, sq[:, :bl],
                                 start=(o == 0), stop=(o == KO - 1))
            return ps_m, ps_s

        def ln_chain(src, dst, mr_b16, bs, bl, ps_m, ps_s, dst_scale=None):
            sl = ds(bs, bl)
            # the mean row leaves PSUM first so ps_m recycles early
            mrow = trans.tile([1, 512], F32, tag="mrow", bufs=2)
            nc.scalar.activation(mrow[:, :bl], ps_m[0:1, :bl], AF.Identity,
                                 scale=1.0 / D)
            m2 = trans.tile([128, 512], F32, tag="m2", bufs=2)
            nc.scalar.activation(m2[:, :bl], ps_m[:, :bl], AF.Square,
                                 scale=1.0 / D)
            var = trans.tile([128, 512], F32, tag="var", bufs=2)
            nc.vector.scalar_tensor_tensor(
                out=var[:, :bl], in0=ps_s[:, :bl], scalar=1.0 / D,
                in1=m2[:, :bl], op0=ALU.mult, op1=ALU.subtract)
            sd = trans.tile([128, 512], F32, tag="sd", bufs=2)
            nc.scalar.activation(sd[:, :bl], var[:, :bl], AF.Sqrt,
                                 bias=eps_c)
            rstd = trans.tile([128, 512], BF16, tag="rstd", bufs=2)
            with nc.allow_low_precision(reason="bf16 rstd: ~0.4% scale"):
                nc.vector.reciprocal(rstd[:, :bl], sd[:, :bl])
            nc.vector.tensor_mul(mr_b16[:, sl], mrow[:, :bl],
                                 rstd[0:1, :bl])
            for o in range(KO):
                if dst_scale is None:
                    nc.vector.tensor_mul(dst[:, o, sl], src[:, o, sl],
                                         rstd[:, :bl])
                else:
                    nc.vector.scalar_tensor_tensor(
                        out=dst[:, o, sl], in0=src[:, o, sl],
                        scalar=dst_scale, in1=rstd[:, :bl], op0=ALU.mult,
                        op1=ALU.mult)

        # stats b0, b1 fill the 2-deep psum pools; each chain is emitted
        # right before the allocation that recycles its stats buffers
        hl = pers.tile([128, KO, TOKP], F8, tag="hl8")
        s0 = ln_stats(h_main, *BLKS[0])
        s1 = ln_stats(h_main, *BLKS[1])
        ln_chain(h_main, hl, mr1h, *BLKS[0], *s0, dst_scale=8.0)
        s2 = ln_stats(h_main, *BLKS[2])
        ln_chain(h_main, hl, mr1h, *BLKS[1], *s1, dst_scale=8.0)
        ln_chain(h_main, hl, mr1h, *BLKS[2], *s2, dst_scale=8.0)

        # ---- QKV projections (feature-major q/k; token-major v) ----
        def qk_proj(wcol, b01, b2, dst01, dst2, dst_off, src, src_blks,
                    corr, mr):
            # dst[mi] = w[:, :, mi].T @ src + bias, written at dst col offset.
            # fp8 DoubleRow: K=256 per pass (sub-tile pairs), psum carries
            # 128x the real value (weights x16, activations x8); the ACT
            # move rescales by 1/128. The rank-1 -mean correction rides the
            # same psum in the x128 domain (crow pre-scaled on host).
            for dst_t, bias_t, m0, ml in [(dst01, b01, 0, 128),
                                          (dst2, b2, 128, 64)]:
                for bs, bl in src_blks:
                    ps = psum.tile([128, 512], F32, tag="pa", bufs=2)
                    for o in range(0, KO, 2):
                        nc.tensor.matmul(
                            ps[:ml, :bl],
                            wqkv[:, o:o + 2, ds(wcol + m0, ml)],
                            src[:, o:o + 2, ds(bs, bl)],
                            start=(o == 0),
                            stop=(corr is None and o == KO - 2),
                            perf_mode=DR,
                        )
                    if corr is not None:
                        nc.tensor.matmul(
                            ps[:ml, :bl], crow[0:1, ds(corr + m0, ml)],
                            mr[0:1, ds(bs, bl)], start=False, stop=True,
                        )
                    nc.scalar.activation(
                        dst_t[:ml, ds(dst_off + bs, bl)], ps[:ml, :bl],
                        AF.Identity, bias=bias_t, scale=1.0 / 128)

        def v_proj(wcol, b_row, src, n_tiles, kt_base, corr):
            # V[token, feat] = src.T @ w + 1 (x) bias  (token-major output,
            # fp8 DoubleRow in the x128 domain; bias row pre-scaled x128)
            for tt in range(n_tiles):
                tw = 1 if kt_base + tt == NK - 1 else 128  # real tokens
                tsl = ds(tt * 128, tw)
                ps = psum.tile([128, 512], F32, tag="pj", bufs=2)
                for o in range(0, KO, 2):
                    nc.tensor.matmul(
                        ps[:tw, :FG], src[:, o:o + 2, tsl],
                        wqkv[:, o:o + 2, ds(wcol, FG)],
                        start=(o == 0), stop=False, perf_mode=DR,
                    )
                if corr is not None:
                    nc.tensor.matmul(ps[:tw, :FG], mr1h[0:1, tsl],
                                     crow[0:1, ds(corr, FG)],
                                     start=False, stop=False)
                nc.tensor.matmul(ps[:tw, :FG], oner16[0:1, 0:tw], b_row,
                                 start=False, stop=True)
                nc.vector.tensor_scalar_mul(
                    vsb[:tw, kt_base + tt, :, 0:DH],
                    ps[:tw, 0:FG].rearrange("p (h d) -> p h d", h=HG),
                    1.0 / 128)

        # q: all tokens; k(self): keys 256..1280; kc: keys 0..255
        qk_proj(0, bq01, bq2, q01, q2, 0, hl, BLKS, 0, mr1h)
        qk_proj(FG, bk01, bk2, kf01, kf2, SC, hl, BLKS, FG, mr1h)
        qk_proj(3 * FG, brk01, brk2, kf01, kf2, 0, ctx, [(0, 256)],
                None, None)
        v_proj(4 * FG, brv_r, ctx, 2, 0, None)     # context V -> key tiles 0,1
        v_proj(2 * FG, bv_r, hl, NK - 2, 2, 2 * FG)

        # ---- attention + per-block proj/AllReduce pipeline ----
        kf_of = [(kf01, 0), (kf01, 64), (kf2, 0)]
        q_of = [(q01, 0), (q01, 64), (q2, 0)]
        dar_in, dar_out = [], []
        for bi, (bs, bl) in enumerate(BLKS):
            dar_in.append(dram.tile([128, KO, bl], BF16,
                                    name=f"dar_in{bi}"))
            dar_out.append(dram.tile([128, KO, bl], BF16,
                                     name=f"dar_out{bi}"))

        for bi, (qs, ql) in enumerate(BLKS):
            qb0 = qs // 128                      # first 128-query tile
            last_kt = min((qs + ql - 1) // 128 + 2, NK - 1)
            # per key tile: column offset of the first not-fully-masked query
            # (query qs+c attends key j iff j <= 256 + qs + c, so tile kt is
            # all-masked for c < 128*(kt-qb0-2) and triangular for the next
            # 128 columns -- those columns are simply not computed, which is
            # exactly equivalent to multiplying by 0)
            def col0(kt):
                return min(max(0, 128 * (kt - qb0 - 2)), ql)
            # greedy-pack key-tile strips into [128,1024] PSUM groups
            strips = [(kt, col0(kt), ql - col0(kt))
                      for kt in range(last_kt + 1) if ql - col0(kt) > 0]
            groups, off = [], 0
            for kt, co, w in strips:
                if not groups or off + w > 1024:
                    groups.append([])
                    off = 0
                groups[-1].append((kt, co, w, off))
                off += w

            def head_scores(h, ps_a):
                kf_t, kf_o = kf_of[h]
                q_t, q_o = q_of[h]
                for g in groups:
                    ps_s = psum.tile([128, 1024], F32, tag="sc", bufs=2,
                                     name="ps_s")
                    for kt, co, w, off in g:
                        nc.tensor.matmul(
                            ps_s[:, ds(off, w)],
                            kf_t[kf_o:kf_o + DH, ts(kt, 128)],
                            q_t[q_o:q_o + DH, ds(qs + co, w)],
                            start=True, stop=True,
                        )
                    expt = trans.tile([128, 1024], BF16, tag="expt", bufs=3)
                    nw = g[-1][3] + g[-1][2]  # last strip's off + w
                    nc.scalar.activation(expt[:, :nw], ps_s[:, :nw], AF.Exp,
                                         scale=0.125)
                    for kt, co, w, off in g:
                        if kt >= qb0 + 2:
                            # triangle starts at strip col 128*dlt - co
                            dlt = kt - qb0 - 2
                            t0 = 128 * dlt - co
                            tw = min(128, w - t0)
                            nc.vector.tensor_mul(
                                expt[:, ds(off + t0, tw)],
                                expt[:, ds(off + t0, tw)],
                                mask_sb[:, ds(384, tw)])
                            if t0 > 0:  # fully-masked cols below the triangle
                                nc.vector.tensor_mul(
                                    expt[:, ds(off, t0)],
                                    expt[:, ds(off, t0)],
                                    mask_sb[:, ds(0, t0)])
                        nc.tensor.matmul(
                            ps_a[:, ds(co, w)], vsb[:, kt, h, :],
                            expt[:, ds(off, w)],
                            start=(kt == 0), stop=(kt == last_kt),
                        )

            def head_norm(h, ps_a):
                # a = num * (1/den); den sits at partition 64, broadcast to
                # partitions 0..63 via a K=1 PE outer product
                r16 = trans.tile([128, 512], BF16, tag="r16", bufs=2)
                with nc.allow_low_precision(reason="bf16 1/den: ~0.4%"):
                    nc.vector.reciprocal(r16[DH:DH + 1, :ql],
                                         ps_a[DH:DH + 1, :ql])
                ps_r = psum.tile([128, 512], F32, tag="pj", bufs=2,
                                 name="ps_r")[0:DH]
                nc.tensor.matmul(ps_r[:, :ql], ones64,
                                 r16[DH:DH + 1, :ql], start=True, stop=True)
                recs = trans.tile([128, 512], BF16, tag="recs", bufs=2)
                nc.scalar.activation(recs[0:DH, :ql], ps_r[:, :ql],
                                     AF.Identity)
                nc.vector.scalar_tensor_tensor(
                    out=a_all[:, h, ds(qs, ql)], in0=ps_a[0:DH, :ql],
                    scalar=16.0, in1=recs[0:DH, :ql], op0=ALU.mult,
                    op1=ALU.mult)

            if ql > 1:
                for h in range(HG):
                    ps_a = psum.tile([128, 512], F32, tag="pa", bufs=2,
                                     name="ps_a")[0:DH + 1]
                    head_scores(h, ps_a)
                    head_norm(h, ps_a)
            else:
                for h in range(HG):
                    ps_a = psum.tile([128, 512], F32, tag="pa", bufs=2,
                                     name="ps_a")[0:DH + 1]
                    head_scores(h, ps_a)
                    head_norm(h, ps_a)

            # attention output projection for this token block (row-sharded);
            # residual pre-folded as +h/4 so the AllReduce output IS new h.
            # high_priority: this chain feeds the AllReduce that LN2 waits on,
            # so its ACT/DVE ops must not queue behind the next block's exps.
            armo = trans.tile([128, KO, 512], BF16, tag="armo", bufs=2)
            with tc.high_priority():
                for mo in range(KO):
                    ps = psum.tile([128, 512], F32, tag="pj", bufs=2)
                    nc.tensor.matmul(ps[:, :ql], wp_sb[:, 0:2, ts(mo, 128)],
                                     a_all[:, 0:2, ds(qs, ql)],
                                     start=True, stop=False, perf_mode=DR)
                    nc.tensor.matmul(ps[:, :ql], wp_sb[:, 2, ts(mo, 128)],
                                     a_all[:, 2, ds(qs, ql)],
                                     start=False, stop=True)
                    ptmp = trans.tile([128, 512], BF16, tag="ptmp", bufs=2)
                    nc.scalar.activation(ptmp[:, :ql], ps[:, :ql],
                                         AF.Identity, bias=bp4[:, mo:mo + 1],
                                         scale=1.0 / 256)
                    nc.vector.scalar_tensor_tensor(
                        out=armo[:, mo, :ql], in0=h_main[:, mo, ds(qs, ql)],
                        scalar=0.25, in1=ptmp[:, :ql], op0=ALU.mult,
                        op1=ALU.add)
                nc.sync.dma_start(dar_in[bi][:], armo[:, :, :ql])
            if sim_collectives:
                nc.gpsimd.collective_compute(
                    "AllReduce", ALU.add, replica_groups=GROUPS,
                    ins=[dar_in[bi].opt()], outs=[dar_out[bi].opt()],
                )
                # h_main <- allreduced (h + attn out), in place
                nc.gpsimd.dma_start(h_main[:, :, ds(qs, ql)], dar_out[bi][:])
            else:
                # local stand-in: the payload makes the same SBUF->DRAM->SBUF
                # round trip a real AllReduce result would
                nc.gpsimd.dma_start(h_main[:, :, ds(qs, ql)], dar_in[bi][:])

        # ---- LN2 + MLP ----
        z0 = pers.tile([128, KO, TOKP], BF16, tag="ln_out")
        t0 = ln_stats(h_main, *BLKS[0])
        t1 = ln_stats(h_main, *BLKS[1])
        ln_chain(h_main, z0, mr2h, *BLKS[0], *t0)
        t2 = ln_stats(h_main, *BLKS[2])
        ln_chain(h_main, z0, mr2h, *BLKS[1], *t1)
        ln_chain(h_main, z0, mr2h, *BLKS[2], *t2)

        def fc_block(bs, bl):
            for mo in range(KO):
                ps = psum.tile([128, 512], F32, tag="pa", bufs=2)
                for o in range(KO):
                    nc.tensor.matmul(
                        ps[:, :bl], wfc_sb[:, o, ts(mo, 128)],
                        z0[:, o, ds(bs, bl)],
                        start=(o == 0), stop=False,
                    )
                nc.tensor.matmul(
                    ps[:, :bl], crow[0:1, ds(3 * FG + mo * 128, 128)],
                    mr2h[0:1, ds(bs, bl)], start=False, stop=True,
                )
                nc.scalar.activation(gl[:, mo, ds(bs, bl)], ps[:, :bl],
                                     gelu_fn, bias=bfc[:, mo:mo + 1])

        drs_in = [dram.tile([KO, 128, bl], BF16, name=f"drs_in{i}")
                  for i, (_, bl) in enumerate(BLKS)]
        drs_out = [dram.tile([FG, bl], BF16, name=f"drs_out{i}")
                   for i, (_, bl) in enumerate(BLKS)]

        def mp_block(bi):
            bs, bl = BLKS[bi]
            mpart = trans.tile([128, KO, 512], BF16, tag="armo", bufs=2)
            for mo in range(KO):
                ps = psum.tile([128, 512], F32, tag="pj", bufs=2)
                for o in range(KO):
                    nc.tensor.matmul(
                        ps[:, :bl], wmp_sb[:, o, ts(mo, 128)],
                        gl[:, o, ds(bs, bl)],
                        start=(o == 0), stop=(o == KO - 1),
                    )
                mtmp = trans.tile([128, 512], BF16, tag="ptmp", bufs=2)
                with tc.high_priority():
                    nc.scalar.activation(mtmp[:, :bl], ps[:, :bl],
                                         AF.Identity,
                                         bias=bmp4[:, mo:mo + 1])
                    # mpart = h_main/4 + (mproj partial + b_mproj/4)
                    nc.vector.scalar_tensor_tensor(
                        out=mpart[:, mo, :bl], in0=h_main[:, mo, ds(bs, bl)],
                        scalar=0.25, in1=mtmp[:, :bl], op0=ALU.mult,
                        op1=ALU.add)
            if sim_collectives:
                nc.sync.dma_start(
                    drs_in[bi].rearrange("o p n -> p o n"), mpart[:, :, :bl])
                nc.gpsimd.collective_compute(
                    "ReduceScatter", ALU.add, replica_groups=GROUPS,
                    ins=[drs_in[bi].opt()], outs=[drs_out[bi].opt()],
                )
                nc.sync.dma_start(out_d[:, ds(bs, bl)], drs_out[bi][:])
            else:
                # local stand-in for ReduceScatter: this core's output slice
                # comes straight from SBUF (a real RS also lands in out_d)
                nc.gpsimd.dma_start(out_d[0:128, ds(bs, bl)],
                                    mpart[:, 0, :bl])
                nc.gpsimd.dma_start(out_d[128:FG, ds(bs, bl)],
                                    mpart[0:64, 1, :bl])

        for bi in range(3):
            fc_block(*BLKS[bi])
            mp_block(bi)

        if debug_taps:
            for nm, t in [("dbg_hl", hl), ("dbg_q01", q01),
                          ("dbg_kf01", kf01), ("dbg_vsb", vsb),
                          ("dbg_a", a_all), ("dbg_h", h_main),
                          ("dbg_z0", z0), ("dbg_gl", gl),
                          ("dbg_dar_in0", dar_in[0]),
                          ("dbg_dar_out0", dar_out[0])]:
                d = nc.dram_tensor(nm, list(t.shape), t.dtype,
                                   kind="ExternalOutput")
                nc.sync.dma_start(d[:], t[:])

    nc.compile()
    return nc


_NC_CACHE = None


def _get_program():
    global _NC_CACHE
    if _NC_CACHE is None:
        _NC_CACHE = build_program()
    return _NC_CACHE


def make_in_maps(inputs):
    f = lambda a: np.asarray(a, dtype=np.float32)
    bf = ml_dtypes.bfloat16
    x = f(inputs["x"])
    context_seq = f(inputs["context_seq"])
    sos_h = f(inputs["sos_h"])
    g1, b1 = f(inputs["ln1_g"]), f(inputs["ln1_b"])
    W_attn, b_attn = f(inputs["W_attn"]), f(inputs["b_attn"])
    W_ref, b_ref = f(inputs["W_ref"]), f(inputs["b_ref"])
    W_proj, b_proj = f(inputs["W_proj"]), f(inputs["b_proj"])
    g2, b2 = f(inputs["ln2_g"]), f(inputs["ln2_b"])
    W_fc, b_fc = f(inputs["W_fc"]), f(inputs["b_fc"])
    W_mproj, b_mproj = f(inputs["W_mproj"]), f(inputs["b_mproj"])

    # master causal mask: mask[p, c] = 1 iff p <= c - 384
    cix = np.arange(896)[None, :]
    pix = np.arange(128)[:, None]
    mask = (pix <= cix - 384).astype(np.float32)
    maskp = np.ones((128, 1088), np.float32)
    maskp[:, 0:896] = mask

    wfc_g = W_fc * g2[:, None]
    bfc_full = b2 @ W_fc + b_fc

    in_maps = []
    for core in range(N_CORES):
        b, g = core // TP, core % TP
        h = np.concatenate([sos_h[None, :], x[b]], axis=0)  # [1025, 768]
        hT = np.zeros((D, TOKP), bf)
        hT[:, :TOK] = h.T.astype(bf)
        qsl = slice(FG * g, FG * (g + 1))
        ksl = slice(D + FG * g, D + FG * (g + 1))
        vsl = slice(2 * D + FG * g, 2 * D + FG * (g + 1))
        rks = slice(FG * g, FG * (g + 1))
        rvs = slice(D + FG * g, D + FG * (g + 1))
        mcols = slice(D * g, D * (g + 1))    # W_fc column slice (768 per core)
        f8 = ml_dtypes.float8_e4m3fn
        wq = W_attn[:, qsl] * g1[:, None]
        wk = W_attn[:, ksl] * g1[:, None]
        wv = W_attn[:, vsl] * g1[:, None]
        # fp8 weights carry a x16 scale; activations x8; the psum is x128
        wqkv8 = np.concatenate(
            [wq, wk, wv, W_ref[:, rks], W_ref[:, rvs]],
            axis=1).astype(np.float32)
        wqkv8 = (16.0 * wqkv8).astype(f8)
        wp_slab = W_proj[FG * g:FG * (g + 1), :]   # [192, 768]
        wp = np.concatenate([wp_slab[0:64], wp_slab[64:128],
                             wp_slab[128:192]], axis=1)  # [64, 2304]
        wp8 = (16.0 * wp).astype(f8)
        cst = np.zeros((128, 25), np.float32)
        bq = b1 @ W_attn[:, qsl] + b_attn[qsl]
        bk = b1 @ W_attn[:, ksl] + b_attn[ksl]
        cst[:, 0] = bq[0:128]
        cst[0:64, 1] = bq[128:192]
        cst[:, 2] = bk[0:128]
        cst[0:64, 3] = bk[128:192]
        cst[:, 4] = b_ref[rks][0:128]
        cst[0:64, 5] = b_ref[rks][128:192]
        cst[:, 6:12] = (b_proj / TP).reshape(6, 128).T
        cst[:, 12:18] = bfc_full[mcols].reshape(6, 128).T
        cst[:, 18:24] = (b_mproj / TP).reshape(6, 128).T
        cst[:, 24] = EPS
        # colsums of the QUANTIZED weights, in the x128 psum domain:
        # corr = -128*colsum(w8/16)*mr = -8*colsum(w8)*mr
        w8f = wqkv8.astype(np.float32)
        rows16 = np.zeros((1, 1856), np.float32)
        rows16[0, 0:FG] = 128.0 * (b1 @ W_attn[:, vsl] + b_attn[vsl])
        rows16[0, FG:2 * FG] = 128.0 * b_ref[rvs]
        rows16[0, 384:512] = 1.0
        rows16[0, 512:1856] = np.concatenate([
            -8.0 * w8f[:, 0:FG].sum(0), -8.0 * w8f[:, FG:2 * FG].sum(0),
            -8.0 * w8f[:, 2 * FG:3 * FG].sum(0),
            -wfc_g[:, mcols].sum(0)])
        in_maps.append({
            "hT": hT,
            "ctxT": (8.0 * np.ascontiguousarray(context_seq[b].T)
                     ).astype(f8),
            "wqkv": np.ascontiguousarray(wqkv8),
            "wp": np.ascontiguousarray(wp8),
            "wfc": np.ascontiguousarray(wfc_g[:, mcols]).astype(bf),
            "wmp": np.ascontiguousarray(W_mproj[mcols, :]).astype(bf),
            "cst": cst,
            "rows16": rows16.astype(bf),
            "maskp": maskp.astype(bf),
        })
    return in_maps


def assemble_output(results, B=2):
    out = np.empty((B, S, D), np.float32)
    for b in range(B):
        parts = [np.asarray(results[TP * b + g]["out"], np.float32)
                 for g in range(TP)]
        full = np.concatenate(parts, axis=0)  # [768, 1025]
        out[b] = full[:, 1:TOK].T
    return out


def kernel(**inputs):
    nc = _get_program()
    in_maps = make_in_maps(inputs)
    res = run_bass_kernel_spmd(nc, in_maps, list(range(N_CORES)))
    return assemble_output(res.results, B=np.asarray(inputs["x"]).shape[0])


if __name__ == "__main__":
    import reference
    ins = reference.setup_inputs()
    ins = {k: np.asarray(v) for k, v in ins.items()}
    got = kernel(**ins)
    exp = np.asarray(reference.reference(**ins))
    err = np.abs(got - exp).max() / np.abs(exp).max()
    print("max abs err:", np.abs(got - exp).max(), "rel:", err)


# revision 43
# speedup vs baseline: 1.8829x; 1.0405x over previous
"""CoconBlock forward on 8 Trainium2 NeuronCores.

Sharding: core c = (b, g) with b = c // 4 (batch), g = c % 4 (tensor-parallel
rank). Within each batch group of 4 cores:
  - attention QKV / context-KV weights column-sharded by head group (3 heads),
  - W_proj row-sharded, partial outputs AllReduced per token block,
  - MLP W_fc column-sharded / W_mproj row-sharded, partial outputs (with the
    residual and bias pre-folded as +h/4 + b_mproj/4 per core) ReduceScattered
    so each core lands exactly its 192-feature slice of the final output.

All on-device activations are feature-on-partition (f32 has no DMA transpose;
this layout makes every matmul transpose-free). Token axis is processed in
blocks (0,512),(512,512),(1024,1) -- no padded-token compute. LayerNorm
reductions over the feature (partition) axis run on the PE via a [128,128]
ones matmul that leaves the statistics replicated across all partitions (PE
matmul cost depends only on the output free size, so replication is free);
the normalize multiplies are then cheap all-SBUF bf16 DVE ops and no
broadcast matmul is needed. The LN affine (gamma/beta) is folded into the
following weight matrix on the host; the -mean*rstd term is folded into the
consuming matmuls as a rank-1 correction (colsum(W) x mr).

Engine balance: every PSUM->SBUF move that carries a per-partition bias runs
on ACT (Identity with bias AP); exp/gelu/sqrt/square on ACT; masks, rstd
reciprocal, residual scalar_tensor_tensor folds on DVE.

Attention uses the 128-aligned causal structure: with keys (256 context +
1025 self) padded to 1408, a (query-block, key-tile) pair is fully allowed,
partially masked by a shifted-triangle slice of one master mask, or skipped.
Scores for a query block are exp'ed in up-to-2-key-tile groups out of a
[128,1024] PSUM slab; the softmax denominator rides the attend matmul as a
ones-column appended to V (partition 64), and 1/den is broadcast to
partitions 0..63 with a K=1 PE outer product.

The attention output projection and its AllReduce run per token block,
pipelined against the remaining attention blocks; the residual is pre-folded
(h/4 per core) so the AllReduce result IS the new h, DMAed straight back
into h_main.
"""

import sys

sys.path.insert(0, "/opt/trn_rl_repo")

import ml_dtypes
import numpy as np

import concourse.bass as bass
import concourse.bacc as bacc
import concourse.mybir as mybir
import concourse.tile as tile
from concourse.bass_utils import run_bass_kernel_spmd

F32 = mybir.dt.float32
AF = mybir.ActivationFunctionType
ALU = mybir.AluOpType
ts, ds = bass.ts, bass.ds

D = 768
DH = 64
S = 1024
SC = 256
TOK = S + 1            # 1025 (sos + x)
TOKP = 1152            # tile column capacity (only 0..1024 computed)
KEYSP = 1408           # 11 * 128
NK = KEYSP // 128      # 11
KO = D // 128          # 6 feature sub-tiles
TP = 4
FG = 192               # features per core in head-sharded tensors (3 heads)
HG = 3                 # heads per core
EPS = 1e-5
N_CORES = 8
GROUPS = [[0, 1, 2, 3], [4, 5, 6, 7]]
BLKS = [(0, 512), (512, 512), (1024, 1)]  # token blocks (start, len)

BF16 = mybir.dt.bfloat16
F8 = mybir.dt.float8e4
DR = mybir.MatmulPerfMode.DoubleRow


def build_program(sim_collectives=True, gelu_fn=None, debug_taps=False):
    if gelu_fn is None:
        gelu_fn = AF.Gelu_apprx_tanh
    nc = bacc.Bacc(None, num_devices=N_CORES)

    # ---- DRAM I/O ----
    hT_d = nc.dram_tensor("hT", [D, TOKP], BF16, kind="ExternalInput")
    ctxT_d = nc.dram_tensor("ctxT", [D, SC], F8, kind="ExternalInput")
    wqkv_d = nc.dram_tensor("wqkv", [D, 5 * FG], F8, kind="ExternalInput")
    wp_d = nc.dram_tensor("wp", [DH, 3 * D], F8, kind="ExternalInput")
    wfc_d = nc.dram_tensor("wfc", [D, D], BF16, kind="ExternalInput")
    wmp_d = nc.dram_tensor("wmp", [D, D], BF16, kind="ExternalInput")
    cst_d = nc.dram_tensor("cst", [128, 25], F32, kind="ExternalInput")
    rows16_d = nc.dram_tensor("rows16", [1, 1856], BF16, kind="ExternalInput")
    maskp_d = nc.dram_tensor("maskp", [128, 1088], BF16, kind="ExternalInput")
    out_d = nc.dram_tensor("out", [FG, TOK], BF16, kind="ExternalOutput")

    with tile.TileContext(nc) as tc, \
         tc.tile_pool(name="pers", bufs=1) as pers, \
         tc.tile_pool(name="trans", bufs=2) as trans, \
         tc.tile_pool(name="psum", bufs=1, space="PSUM") as psum, \
         tc.tile_pool(name="dram", bufs=1, space="DRAM") as dram:

        # ---- resident SBUF tensors ----
        h_main = pers.tile([128, KO, TOKP], BF16, tag="h_main")
        ctx = pers.tile([128, KO, SC], F8, tag="ctx")
        wqkv = pers.tile([128, KO, 5 * FG], F8, tag="wqkv")
        wp_sb = pers.tile([DH, HG, D], F8, tag="wp")
        wfc_sb = pers.tile([128, KO, D], BF16, tag="wfc")
        wmp_sb = pers.tile([128, KO, D], BF16, tag="wmp")
        kf01 = pers.tile([128, KEYSP], BF16, tag="kf01")
        kf2 = pers.tile([64, KEYSP], BF16, tag="kf2")
        q01 = pers.tile([128, TOKP], BF16, tag="q01")
        q2 = pers.tile([64, TOKP], BF16, tag="q2")
        vsb = pers.tile([128, NK, HG, DH + 1], BF16, tag="vsb")
        a_all = pers.tile([DH, HG, TOKP], F8, tag="a_all")
        gl = pers.tile([128, KO, TOKP], BF16, tag="gl")
        maskp = pers.tile([128, 1088], BF16, tag="maskp")
        mask_sb = maskp[:, 0:896]
        onesq = maskp[:, 896:1024]          # [128,128] ones (LN stats lhsT)
        ones64 = maskp[64:65, 1024:1088]    # [1,64] ones at partition 64
        cst = pers.tile([128, 25], F32, tag="cst")
        bq01, bq2 = cst[:, 0:1], cst[0:64, 1:2]
        bk01, bk2 = cst[:, 2:3], cst[0:64, 3:4]
        brk01, brk2 = cst[:, 4:5], cst[0:64, 5:6]
        bp4, bfc, bmp4 = cst[:, 6:12], cst[:, 12:18], cst[:, 18:24]
        eps_c = cst[:, 24:25]
        rows16 = pers.tile([1, 1856], BF16, tag="rows16")
        bv_r, brv_r = rows16[:, 0:FG], rows16[:, FG:2 * FG]
        oner16 = rows16[:, 384:512]
        crow = rows16[:, 512:1856]
        mr1h = pers.tile([1, TOKP], BF16, tag="mr1h")
        mr2h = pers.tile([1, TOKP], BF16, tag="mr2h")

        # ---- input DMAs (mask consts + hT block 0 first: LN1 starts early) -
        pin = lambda t: t.rearrange("(o p) n -> p o n", p=128)
        nc.sync.dma_start(out=maskp[:], in_=maskp_d[:])
        nc.sync.dma_start(out=h_main[:, 0:3, ds(0, 512)],
                          in_=pin(hT_d)[:, 0:3, ds(0, 512)])
        nc.sync.dma_start(out=h_main[:, 3:6, ds(0, 512)],
                          in_=pin(hT_d)[:, 3:6, ds(0, 512)])
        nc.sync.dma_start(out=cst[:], in_=cst_d[:])
        for bs, bl in BLKS[1:]:
            nc.sync.dma_start(out=h_main[:, :, ds(bs, bl)],
                              in_=pin(hT_d)[:, :, ds(bs, bl)])
        nc.sync.dma_start(out=rows16[:], in_=rows16_d[:])
        nc.sync.dma_start(out=ctx[:], in_=pin(ctxT_d))
        nc.sync.dma_start(out=wqkv[:], in_=pin(wqkv_d))
        nc.sync.dma_start(out=wp_sb[:], in_=wp_d.rearrange("p (h n) -> p h n",
                                                           h=HG))
        nc.sync.dma_start(out=wfc_sb[:], in_=pin(wfc_d))
        nc.sync.dma_start(out=wmp_sb[:], in_=pin(wmp_d))
        nc.vector.memset(kf01[:, ds(SC + TOK, KEYSP - SC - TOK)], 0.0)
        nc.vector.memset(kf2[:, ds(SC + TOK, KEYSP - SC - TOK)], 0.0)
        nc.vector.memset(vsb[:, NK - 1, :, :], 0.0)
        nc.vector.memset(vsb[:, :, :, DH:DH + 1], 1.0)

        # ---- LayerNorm (normalize only; affine folded into next weights).
        # dst = src * rsqrt(var+eps); mr row = mean*rstd (for the rank-1
        # -mean correction in consuming matmuls). Stats land replicated on
        # all 128 partitions via the onesq matmul, so everything downstream
        # is a cheap elementwise op.
        def ln_stats(src, bs, bl):
            sl = ds(bs, bl)
            ps_m = psum.tile([128, 512], F32, tag="pa", bufs=2)
            ps_s = psum.tile([128, 512], F32, tag="pj", bufs=2)
            for o in range(KO):
                sq = trans.tile([128, 512], BF16, tag="sq", bufs=3)
                nc.vector.tensor_mul(sq[:, :bl], src[:, o, sl],
                                     src[:, o, sl])
                nc.tensor.matmul(ps_m[:, :bl], onesq, src[:, o, sl],
                                 start=(o == 0), stop=(o == KO - 1))
                nc.tensor.matmul(ps_s[:, :bl], onesq, sq[:, :bl],
                                 start=(o == 0), stop=(o == KO - 1))
            return ps_m, ps_s

        def ln_chain(src, dst, mr_b16, bs, bl, ps_m, ps_s, dst_scale=None):
            sl = ds(bs, bl)
            # the mean row leaves PSUM first so ps_m recycles early
            mrow = trans.tile([1, 512], F32, tag="mrow", bufs=2)
            nc.scalar.activation(mrow[:, :bl], ps_m[0:1, :bl], AF.Identity,
                                 scale=1.0 / D)
            m2 = trans.tile([128, 512], F32, tag="m2", bufs=2)
            nc.scalar.activation(m2[:, :bl], ps_m[:, :bl], AF.Square,
                                 scale=1.0 / D)
            var = trans.tile([128, 512], F32, tag="var", bufs=2)
            nc.vector.scalar_tensor_tensor(
                out=var[:, :bl], in0=ps_s[:, :bl], scalar=1.0 / D,
                in1=m2[:, :bl], op0=ALU.mult, op1=ALU.subtract)
            sd = trans.tile([128, 512], F32, tag="sd", bufs=2)
            nc.scalar.activation(sd[:, :bl], var[:, :bl], AF.Sqrt,
                                 bias=eps_c)
            rstd = trans.tile([128, 512], BF16, tag="rstd", bufs=2)
            with nc.allow_low_precision(reason="bf16 rstd: ~0.4% scale"):
                nc.vector.reciprocal(rstd[:, :bl], sd[:, :bl])
            nc.vector.tensor_mul(mr_b16[:, sl], mrow[:, :bl],
                                 rstd[0:1, :bl])
            for o in range(KO):
                if dst_scale is None:
                    nc.vector.tensor_mul(dst[:, o, sl], src[:, o, sl],
                                         rstd[:, :bl])
                else:
                    nc.vector.scalar_tensor_tensor(
                        out=dst[:, o, sl], in0=src[:, o, sl],
                        scalar=dst_scale, in1=rstd[:, :bl], op0=ALU.mult,
                        op1=ALU.mult)

        # stats b0, b1 fill the 2-deep psum pools; each chain is emitted
        # right before the allocation that recycles its stats buffers
        hl = pers.tile([128, KO, TOKP], F8, tag="hl8")
        s0 = ln_stats(h_main, *BLKS[0])
        s1 = ln_stats(h_main, *BLKS[1])
        ln_chain(h_main, hl, mr1h, *BLKS[0], *s0, dst_scale=8.0)
        s2 = ln_stats(h_main, *BLKS[2])
        ln_chain(h_main, hl, mr1h, *BLKS[1], *s1, dst_scale=8.0)
        ln_chain(h_main, hl, mr1h, *BLKS[2], *s2, dst_scale=8.0)

        # ---- QKV projections (feature-major q/k; token-major v) ----
        def qk_proj(wcol, b01, b2, dst01, dst2, dst_off, src, src_blks,
                    corr, mr):
            # dst[mi] = w[:, :, mi].T @ src + bias, written at dst col offset.
            # fp8 DoubleRow: K=256 per pass (sub-tile pairs), psum carries
            # 128x the real value (weights x16, activations x8); the ACT
            # move rescales by 1/128. The rank-1 -mean correction rides the
            # same psum in the x128 domain (crow pre-scaled on host).
            for dst_t, bias_t, m0, ml in [(dst01, b01, 0, 128),
                                          (dst2, b2, 128, 64)]:
                for bs, bl in src_blks:
                    ps = psum.tile([128, 512], F32, tag="pa", bufs=2)
                    for o in range(0, KO, 2):
                        nc.tensor.matmul(
                            ps[:ml, :bl],
                            wqkv[:, o:o + 2, ds(wcol + m0, ml)],
                            src[:, o:o + 2, ds(bs, bl)],
                            start=(o == 0),
                            stop=(corr is None and o == KO - 2),
                            perf_mode=DR,
                        )
                    if corr is not None:
                        nc.tensor.matmul(
                            ps[:ml, :bl], crow[0:1, ds(corr + m0, ml)],
                            mr[0:1, ds(bs, bl)], start=False, stop=True,
                        )
                    nc.scalar.activation(
                        dst_t[:ml, ds(dst_off + bs, bl)], ps[:ml, :bl],
                        AF.Identity, bias=bias_t, scale=1.0 / 128)

        def v_proj(wcol, b_row, src, tts, kt_base, corr):
            # V[token, feat] = src.T @ w + 1 (x) bias  (token-major output,
            # fp8 DoubleRow in the x128 domain; bias row pre-scaled x128)
            for tt in tts:
                tw = 1 if kt_base + tt == NK - 1 else 128  # real tokens
                tsl = ds(tt * 128, tw)
                ps = psum.tile([128, 512], F32, tag="pj", bufs=2)
                for o in range(0, KO, 2):
                    nc.tensor.matmul(
                        ps[:tw, :FG], src[:, o:o + 2, tsl],
                        wqkv[:, o:o + 2, ds(wcol, FG)],
                        start=(o == 0), stop=False, perf_mode=DR,
                    )
                if corr is not None:
                    nc.tensor.matmul(ps[:tw, :FG], mr1h[0:1, tsl],
                                     crow[0:1, ds(corr, FG)],
                                     start=False, stop=False)
                nc.tensor.matmul(ps[:tw, :FG], oner16[0:1, 0:tw], b_row,
                                 start=False, stop=True)
                nc.vector.tensor_scalar_mul(
                    vsb[:tw, kt_base + tt, :, 0:DH],
                    ps[:tw, 0:FG].rearrange("p (h d) -> p h d", h=HG),
                    1.0 / 128)

        # context K/V (keys 0..255) -- needed by every query block
        qk_proj(3 * FG, brk01, brk2, kf01, kf2, 0, ctx, [(0, 256)],
                None, None)
        v_proj(4 * FG, brv_r, ctx, range(2), 0, None)

        # ---- attention + per-block proj/AllReduce pipeline ----
        kf_of = [(kf01, 0), (kf01, 64), (kf2, 0)]
        q_of = [(q01, 0), (q01, 64), (q2, 0)]
        dar_in, dar_out = [], []
        for bi, (bs, bl) in enumerate(BLKS):
            dar_in.append(dram.tile([128, KO, bl], BF16,
                                    name=f"dar_in{bi}"))
            dar_out.append(dram.tile([128, KO, bl], BF16,
                                     name=f"dar_out{bi}"))

        def attn_block(bi):
            qs, ql = BLKS[bi]
            qb0 = qs // 128                      # first 128-query tile
            last_kt = min((qs + ql - 1) // 128 + 2, NK - 1)
            # per key tile: column offset of the first not-fully-masked query
            # (query qs+c attends key j iff j <= 256 + qs + c, so tile kt is
            # all-masked for c < 128*(kt-qb0-2) and triangular for the next
            # 128 columns -- those columns are simply not computed, which is
            # exactly equivalent to multiplying by 0)
            def col0(kt):
                return min(max(0, 128 * (kt - qb0 - 2)), ql)
            # greedy-pack key-tile strips into [128,1024] PSUM groups
            strips = [(kt, col0(kt), ql - col0(kt))
                      for kt in range(last_kt + 1) if ql - col0(kt) > 0]
            groups, off = [], 0
            for kt, co, w in strips:
                if not groups or off + w > 1024:
                    groups.append([])
                    off = 0
                groups[-1].append((kt, co, w, off))
                off += w

            def head_scores(h, ps_a):
                kf_t, kf_o = kf_of[h]
                q_t, q_o = q_of[h]
                for g in groups:
                    ps_s = psum.tile([128, 1024], F32, tag="sc", bufs=2,
                                     name="ps_s")
                    for kt, co, w, off in g:
                        nc.tensor.matmul(
                            ps_s[:, ds(off, w)],
                            kf_t[kf_o:kf_o + DH, ts(kt, 128)],
                            q_t[q_o:q_o + DH, ds(qs + co, w)],
                            start=True, stop=True,
                        )
                    expt = trans.tile([128, 1024], BF16, tag="expt", bufs=3)
                    nw = g[-1][3] + g[-1][2]  # last strip's off + w
                    nc.scalar.activation(expt[:, :nw], ps_s[:, :nw], AF.Exp,
                                         scale=0.125)
                    for kt, co, w, off in g:
                        if kt >= qb0 + 2:
                            # triangle starts at strip col 128*dlt - co
                            dlt = kt - qb0 - 2
                            t0 = 128 * dlt - co
                            tw = min(128, w - t0)
                            nc.vector.tensor_mul(
                                expt[:, ds(off + t0, tw)],
                                expt[:, ds(off + t0, tw)],
                                mask_sb[:, ds(384, tw)])
                            if t0 > 0:  # fully-masked cols below the triangle
                                nc.vector.tensor_mul(
                                    expt[:, ds(off, t0)],
                                    expt[:, ds(off, t0)],
                                    mask_sb[:, ds(0, t0)])
                        nc.tensor.matmul(
                            ps_a[:, ds(co, w)], vsb[:, kt, h, :],
                            expt[:, ds(off, w)],
                            start=(kt == 0), stop=(kt == last_kt),
                        )

            def head_norm(h, ps_a):
                # a = num * (1/den); den sits at partition 64, broadcast to
                # partitions 0..63 via a K=1 PE outer product
                r16 = trans.tile([128, 512], BF16, tag="r16", bufs=2)
                with nc.allow_low_precision(reason="bf16 1/den: ~0.4%"):
                    nc.vector.reciprocal(r16[DH:DH + 1, :ql],
                                         ps_a[DH:DH + 1, :ql])
                ps_r = psum.tile([128, 512], F32, tag="pj", bufs=2,
                                 name="ps_r")[0:DH]
                nc.tensor.matmul(ps_r[:, :ql], ones64,
                                 r16[DH:DH + 1, :ql], start=True, stop=True)
                recs = trans.tile([128, 512], BF16, tag="recs", bufs=2)
                nc.scalar.activation(recs[0:DH, :ql], ps_r[:, :ql],
                                     AF.Identity)
                nc.vector.scalar_tensor_tensor(
                    out=a_all[:, h, ds(qs, ql)], in0=ps_a[0:DH, :ql],
                    scalar=16.0, in1=recs[0:DH, :ql], op0=ALU.mult,
                    op1=ALU.mult)

            if ql > 1:
                for h in range(HG):
                    ps_a = psum.tile([128, 512], F32, tag="pa", bufs=2,
                                     name="ps_a")[0:DH + 1]
                    head_scores(h, ps_a)
                    head_norm(h, ps_a)
            else:
                for h in range(HG):
                    ps_a = psum.tile([128, 512], F32, tag="pa", bufs=2,
                                     name="ps_a")[0:DH + 1]
                    head_scores(h, ps_a)
                    head_norm(h, ps_a)

            # attention output projection for this token block (row-sharded);
            # residual pre-folded as +h/4 so the AllReduce output IS new h.
            # high_priority: this chain feeds the AllReduce that LN2 waits on,
            # so its ACT/DVE ops must not queue behind the next block's exps.
            armo = trans.tile([128, KO, 512], BF16, tag="armo", bufs=2)
            with tc.high_priority():
                for mo in range(KO):
                    ps = psum.tile([128, 512], F32, tag="pj", bufs=2)
                    nc.tensor.matmul(ps[:, :ql], wp_sb[:, 0:2, ts(mo, 128)],
                                     a_all[:, 0:2, ds(qs, ql)],
                                     start=True, stop=False, perf_mode=DR)
                    nc.tensor.matmul(ps[:, :ql], wp_sb[:, 2, ts(mo, 128)],
                                     a_all[:, 2, ds(qs, ql)],
                                     start=False, stop=True)
                    ptmp = trans.tile([128, 512], BF16, tag="ptmp", bufs=2)
                    nc.scalar.activation(ptmp[:, :ql], ps[:, :ql],
                                         AF.Identity, bias=bp4[:, mo:mo + 1],
                                         scale=1.0 / 256)
                    nc.vector.scalar_tensor_tensor(
                        out=armo[:, mo, :ql], in0=h_main[:, mo, ds(qs, ql)],
                        scalar=0.25, in1=ptmp[:, :ql], op0=ALU.mult,
                        op1=ALU.add)
                nc.sync.dma_start(dar_in[bi][:], armo[:, :, :ql])
            if sim_collectives:
                nc.gpsimd.collective_compute(
                    "AllReduce", ALU.add, replica_groups=GROUPS,
                    ins=[dar_in[bi].opt()], outs=[dar_out[bi].opt()],
                )
                # h_main <- allreduced (h + attn out), in place
                nc.gpsimd.dma_start(h_main[:, :, ds(qs, ql)], dar_out[bi][:])
            else:
                # local stand-in: the payload makes the same SBUF->DRAM->SBUF
                # round trip a real AllReduce result would
                nc.gpsimd.dma_start(h_main[:, :, ds(qs, ql)], dar_in[bi][:])

        # software pipeline: each query block's attention starts as soon as
        # its own q/k/v slices exist (qb0 needs only block-0 projections +
        # context KV), so attention overlaps the remaining projections
        VT = [range(0, 4), range(4, 8), range(8, 9)]  # v tiles per block
        for bi in (0, 1, 2):
            qk_proj(0, bq01, bq2, q01, q2, 0, hl, [BLKS[bi]], 0, mr1h)
            qk_proj(FG, bk01, bk2, kf01, kf2, SC, hl, [BLKS[bi]], FG, mr1h)
            v_proj(2 * FG, bv_r, hl, VT[bi], 2, 2 * FG)
            attn_block(bi)

        # ---- LN2 + MLP ----
        z0 = pers.tile([128, KO, TOKP], BF16, tag="ln_out")
        t0 = ln_stats(h_main, *BLKS[0])
        t1 = ln_stats(h_main, *BLKS[1])
        ln_chain(h_main, z0, mr2h, *BLKS[0], *t0)
        t2 = ln_stats(h_main, *BLKS[2])
        ln_chain(h_main, z0, mr2h, *BLKS[1], *t1)
        ln_chain(h_main, z0, mr2h, *BLKS[2], *t2)

        def fc_block(bs, bl):
            for mo in range(KO):
                ps = psum.tile([128, 512], F32, tag="pa", bufs=2)
                for o in range(KO):
                    nc.tensor.matmul(
                        ps[:, :bl], wfc_sb[:, o, ts(mo, 128)],
                        z0[:, o, ds(bs, bl)],
                        start=(o == 0), stop=False,
                    )
                nc.tensor.matmul(
                    ps[:, :bl], crow[0:1, ds(3 * FG + mo * 128, 128)],
                    mr2h[0:1, ds(bs, bl)], start=False, stop=True,
                )
                nc.scalar.activation(gl[:, mo, ds(bs, bl)], ps[:, :bl],
                                     gelu_fn, bias=bfc[:, mo:mo + 1])

        drs_in = [dram.tile([KO, 128, bl], BF16, name=f"drs_in{i}")
                  for i, (_, bl) in enumerate(BLKS)]
        drs_out = [dram.tile([FG, bl], BF16, name=f"drs_out{i}")
                   for i, (_, bl) in enumerate(BLKS)]

        def mp_block(bi):
            bs, bl = BLKS[bi]
            mpart = trans.tile([128, KO, 512], BF16, tag="armo", bufs=2)
            for mo in range(KO):
                ps = psum.tile([128, 512], F32, tag="pj", bufs=2)
                for o in range(KO):
                    nc.tensor.matmul(
                        ps[:, :bl], wmp_sb[:, o, ts(mo, 128)],
                        gl[:, o, ds(bs, bl)],
                        start=(o == 0), stop=(o == KO - 1),
                    )
                mtmp = trans.tile([128, 512], BF16, tag="ptmp", bufs=2)
                with tc.high_priority():
                    nc.scalar.activation(mtmp[:, :bl], ps[:, :bl],
                                         AF.Identity,
                                         bias=bmp4[:, mo:mo + 1])
                    # mpart = h_main/4 + (mproj partial + b_mproj/4)
                    nc.vector.scalar_tensor_tensor(
                        out=mpart[:, mo, :bl], in0=h_main[:, mo, ds(bs, bl)],
                        scalar=0.25, in1=mtmp[:, :bl], op0=ALU.mult,
                        op1=ALU.add)
            if sim_collectives:
                nc.sync.dma_start(
                    drs_in[bi].rearrange("o p n -> p o n"), mpart[:, :, :bl])
                nc.gpsimd.collective_compute(
                    "ReduceScatter", ALU.add, replica_groups=GROUPS,
                    ins=[drs_in[bi].opt()], outs=[drs_out[bi].opt()],
                )
                nc.sync.dma_start(out_d[:, ds(bs, bl)], drs_out[bi][:])
            else:
                # local stand-in for ReduceScatter: this core's output slice
                # comes straight from SBUF (a real RS also lands in out_d)
                nc.gpsimd.dma_start(out_d[0:128, ds(bs, bl)],
                                    mpart[:, 0, :bl])
                nc.gpsimd.dma_start(out_d[128:FG, ds(bs, bl)],
                                    mpart[0:64, 1, :bl])

        for bi in range(3):
            fc_block(*BLKS[bi])
            mp_block(bi)

        if debug_taps:
            for nm, t in [("dbg_hl", hl), ("dbg_q01", q01),
                          ("dbg_kf01", kf01), ("dbg_vsb", vsb),
                          ("dbg_a", a_all), ("dbg_h", h_main),
                          ("dbg_z0", z0), ("dbg_gl", gl),
                          ("dbg_dar_in0", dar_in[0]),
                          ("dbg_dar_out0", dar_out[0])]:
                d = nc.dram_tensor(nm, list(t.shape), t.dtype,
                                   kind="ExternalOutput")
                nc.sync.dma_start(d[:], t[:])

    nc.compile()
    return nc


_NC_CACHE = None


def _get_program():
    global _NC_CACHE
    if _NC_CACHE is None:
        _NC_CACHE = build_program()
    return _NC_CACHE


def make_in_maps(inputs):
    f = lambda a: np.asarray(a, dtype=np.float32)
    bf = ml_dtypes.bfloat16
    x = f(inputs["x"])
    context_seq = f(inputs["context_seq"])
    sos_h = f(inputs["sos_h"])
    g1, b1 = f(inputs["ln1_g"]), f(inputs["ln1_b"])
    W_attn, b_attn = f(inputs["W_attn"]), f(inputs["b_attn"])
    W_ref, b_ref = f(inputs["W_ref"]), f(inputs["b_ref"])
    W_proj, b_proj = f(inputs["W_proj"]), f(inputs["b_proj"])
    g2, b2 = f(inputs["ln2_g"]), f(inputs["ln2_b"])
    W_fc, b_fc = f(inputs["W_fc"]), f(inputs["b_fc"])
    W_mproj, b_mproj = f(inputs["W_mproj"]), f(inputs["b_mproj"])

    # master causal mask: mask[p, c] = 1 iff p <= c - 384
    cix = np.arange(896)[None, :]
    pix = np.arange(128)[:, None]
    mask = (pix <= cix - 384).astype(np.float32)
    maskp = np.ones((128, 1088), np.float32)
    maskp[:, 0:896] = mask

    wfc_g = W_fc * g2[:, None]
    bfc_full = b2 @ W_fc + b_fc

    in_maps = []
    for core in range(N_CORES):
        b, g = core // TP, core % TP
        h = np.concatenate([sos_h[None, :], x[b]], axis=0)  # [1025, 768]
        hT = np.zeros((D, TOKP), bf)
        hT[:, :TOK] = h.T.astype(bf)
        qsl = slice(FG * g, FG * (g + 1))
        ksl = slice(D + FG * g, D + FG * (g + 1))
        vsl = slice(2 * D + FG * g, 2 * D + FG * (g + 1))
        rks = slice(FG * g, FG * (g + 1))
        rvs = slice(D + FG * g, D + FG * (g + 1))
        mcols = slice(D * g, D * (g + 1))    # W_fc column slice (768 per core)
        f8 = ml_dtypes.float8_e4m3fn
        wq = W_attn[:, qsl] * g1[:, None]
        wk = W_attn[:, ksl] * g1[:, None]
        wv = W_attn[:, vsl] * g1[:, None]
        # fp8 weights carry a x16 scale; activations x8; the psum is x128
        wqkv8 = np.concatenate(
            [wq, wk, wv, W_ref[:, rks], W_ref[:, rvs]],
            axis=1).astype(np.float32)
        wqkv8 = (16.0 * wqkv8).astype(f8)
        wp_slab = W_proj[FG * g:FG * (g + 1), :]   # [192, 768]
        wp = np.concatenate([wp_slab[0:64], wp_slab[64:128],
                             wp_slab[128:192]], axis=1)  # [64, 2304]
        wp8 = (16.0 * wp).astype(f8)
        cst = np.zeros((128, 25), np.float32)
        bq = b1 @ W_attn[:, qsl] + b_attn[qsl]
        bk = b1 @ W_attn[:, ksl] + b_attn[ksl]
        cst[:, 0] = bq[0:128]
        cst[0:64, 1] = bq[128:192]
        cst[:, 2] = bk[0:128]
        cst[0:64, 3] = bk[128:192]
        cst[:, 4] = b_ref[rks][0:128]
        cst[0:64, 5] = b_ref[rks][128:192]
        cst[:, 6:12] = (b_proj / TP).reshape(6, 128).T
        cst[:, 12:18] = bfc_full[mcols].reshape(6, 128).T
        cst[:, 18:24] = (b_mproj / TP).reshape(6, 128).T
        cst[:, 24] = EPS
        # colsums of the QUANTIZED weights, in the x128 psum domain:
        # corr = -128*colsum(w8/16)*mr = -8*colsum(w8)*mr
        w8f = wqkv8.astype(np.float32)
        rows16 = np.zeros((1, 1856), np.float32)
        rows16[0, 0:FG] = 128.0 * (b1 @ W_attn[:, vsl] + b_attn[vsl])
        rows16[0, FG:2 * FG] = 128.0 * b_ref[rvs]
        rows16[0, 384:512] = 1.0
        rows16[0, 512:1856] = np.concatenate([
            -8.0 * w8f[:, 0:FG].sum(0), -8.0 * w8f[:, FG:2 * FG].sum(0),
            -8.0 * w8f[:, 2 * FG:3 * FG].sum(0),
            -wfc_g[:, mcols].sum(0)])
        in_maps.append({
            "hT": hT,
            "ctxT": (8.0 * np.ascontiguousarray(context_seq[b].T)
                     ).astype(f8),
            "wqkv": np.ascontiguousarray(wqkv8),
            "wp": np.ascontiguousarray(wp8),
            "wfc": np.ascontiguousarray(wfc_g[:, mcols]).astype(bf),
            "wmp": np.ascontiguousarray(W_mproj[mcols, :]).astype(bf),
            "cst": cst,
            "rows16": rows16.astype(bf),
            "maskp": maskp.astype(bf),
        })
    return in_maps


def assemble_output(results, B=2):
    out = np.empty((B, S, D), np.float32)
    for b in range(B):
        parts = [np.asarray(results[TP * b + g]["out"], np.float32)
                 for g in range(TP)]
        full = np.concatenate(parts, axis=0)  # [768, 1025]
        out[b] = full[:, 1:TOK].T
    return out


def kernel(**inputs):
    nc = _get_program()
    in_maps = make_in_maps(inputs)
    res = run_bass_kernel_spmd(nc, in_maps, list(range(N_CORES)))
    return assemble_output(res.results, B=np.asarray(inputs["x"]).shape[0])


if __name__ == "__main__":
    import reference
    ins = reference.setup_inputs()
    ins = {k: np.asarray(v) for k, v in ins.items()}
    got = kernel(**ins)
    exp = np.asarray(reference.reference(**ins))
    err = np.abs(got - exp).max() / np.abs(exp).max()
    print("max abs err:", np.abs(got - exp).max(), "rel:", err)


# revision 49
# speedup vs baseline: 1.8919x; 1.0048x over previous
"""CoconBlock forward on 8 Trainium2 NeuronCores.

Sharding: core c = (b, g) with b = c // 4 (batch), g = c % 4 (tensor-parallel
rank). Within each batch group of 4 cores:
  - attention QKV / context-KV weights column-sharded by head group (3 heads),
  - W_proj row-sharded, partial outputs AllReduced per token block,
  - MLP W_fc column-sharded / W_mproj row-sharded, partial outputs (with the
    residual and bias pre-folded as +h/4 + b_mproj/4 per core) ReduceScattered
    so each core lands exactly its 192-feature slice of the final output.

All on-device activations are feature-on-partition (f32 has no DMA transpose;
this layout makes every matmul transpose-free). Token axis is processed in
blocks (0,512),(512,512),(1024,1) -- no padded-token compute. LayerNorm
reductions over the feature (partition) axis run on the PE via a [128,128]
ones matmul that leaves the statistics replicated across all partitions (PE
matmul cost depends only on the output free size, so replication is free);
the normalize multiplies are then cheap all-SBUF bf16 DVE ops and no
broadcast matmul is needed. The LN affine (gamma/beta) is folded into the
following weight matrix on the host; the -mean*rstd term is folded into the
consuming matmuls as a rank-1 correction (colsum(W) x mr).

Engine balance: every PSUM->SBUF move that carries a per-partition bias runs
on ACT (Identity with bias AP); exp/gelu/sqrt/square on ACT; masks, rstd
reciprocal, residual scalar_tensor_tensor folds on DVE.

Attention uses the 128-aligned causal structure: with keys (256 context +
1025 self) padded to 1408, a (query-block, key-tile) pair is fully allowed,
partially masked by a shifted-triangle slice of one master mask, or skipped.
Scores for a query block are exp'ed in up-to-2-key-tile groups out of a
[128,1024] PSUM slab; the softmax denominator rides the attend matmul as a
ones-column appended to V (partition 64), and 1/den is broadcast to
partitions 0..63 with a K=1 PE outer product.

The attention output projection and its AllReduce run per token block,
pipelined against the remaining attention blocks; the residual is pre-folded
(h/4 per core) so the AllReduce result IS the new h, DMAed straight back
into h_main.
"""

import sys

sys.path.insert(0, "/opt/trn_rl_repo")

import ml_dtypes
import numpy as np

import concourse.bass as bass
import concourse.bacc as bacc
import concourse.mybir as mybir
import concourse.tile as tile
from concourse.bass_utils import run_bass_kernel_spmd

F32 = mybir.dt.float32
AF = mybir.ActivationFunctionType
ALU = mybir.AluOpType
ts, ds = bass.ts, bass.ds

D = 768
DH = 64
S = 1024
SC = 256
TOK = S + 1            # 1025 (sos + x)
TOKP = 1152            # tile column capacity (only 0..1024 computed)
KEYSP = 1408           # 11 * 128
NK = KEYSP // 128      # 11
KO = D // 128          # 6 feature sub-tiles
TP = 4
FG = 192               # features per core in head-sharded tensors (3 heads)
HG = 3                 # heads per core
EPS = 1e-5
N_CORES = 8
GROUPS = [[0, 1, 2, 3], [4, 5, 6, 7]]
BLKS = [(0, 512), (512, 512), (1024, 1)]  # token blocks (start, len)

BF16 = mybir.dt.bfloat16
F8 = mybir.dt.float8e4
DR = mybir.MatmulPerfMode.DoubleRow


def build_program(sim_collectives=True, gelu_fn=None, debug_taps=False):
    if gelu_fn is None:
        gelu_fn = AF.Gelu_apprx_tanh
    nc = bacc.Bacc(None, num_devices=N_CORES)

    # ---- DRAM I/O ----
    hT_d = nc.dram_tensor("hT", [D, TOKP], BF16, kind="ExternalInput")
    ctxT_d = nc.dram_tensor("ctxT", [D, SC], F8, kind="ExternalInput")
    wqkv_d = nc.dram_tensor("wqkv", [D, 5 * FG], F8, kind="ExternalInput")
    wp_d = nc.dram_tensor("wp", [DH, 3 * D], F8, kind="ExternalInput")
    wfc_d = nc.dram_tensor("wfc", [D, D], BF16, kind="ExternalInput")
    wmp_d = nc.dram_tensor("wmp", [D, D], BF16, kind="ExternalInput")
    cst_d = nc.dram_tensor("cst", [128, 25], F32, kind="ExternalInput")
    rows16_d = nc.dram_tensor("rows16", [1, 1856], BF16, kind="ExternalInput")
    maskp_d = nc.dram_tensor("maskp", [128, 1088], BF16, kind="ExternalInput")
    out_d = nc.dram_tensor("out", [FG, TOK], BF16, kind="ExternalOutput")

    with tile.TileContext(nc) as tc, \
         tc.tile_pool(name="pers", bufs=1) as pers, \
         tc.tile_pool(name="trans", bufs=2) as trans, \
         tc.tile_pool(name="psum", bufs=1, space="PSUM") as psum, \
         tc.tile_pool(name="dram", bufs=1, space="DRAM") as dram:

        # ---- resident SBUF tensors ----
        h_main = pers.tile([128, KO, TOKP], BF16, tag="h_main")
        ctx = pers.tile([128, KO, SC], F8, tag="ctx")
        wqkv = pers.tile([128, KO, 5 * FG], F8, tag="wqkv")
        wp_sb = pers.tile([DH, HG, D], F8, tag="wp")
        wfc_sb = pers.tile([128, KO, D], BF16, tag="wfc")
        wmp_sb = pers.tile([128, KO, D], BF16, tag="wmp")
        kf01 = pers.tile([128, KEYSP], BF16, tag="kf01")
        kf2 = pers.tile([64, KEYSP], BF16, tag="kf2")
        q01 = pers.tile([128, TOKP], BF16, tag="q01")
        q2 = pers.tile([64, TOKP], BF16, tag="q2")
        vsb = pers.tile([128, NK, HG, DH + 1], BF16, tag="vsb")
        a_all = pers.tile([DH, HG, TOKP], F8, tag="a_all")
        gl = pers.tile([128, KO, TOKP], BF16, tag="gl")
        maskp = pers.tile([128, 1088], BF16, tag="maskp")
        mask_sb = maskp[:, 0:896]
        onesq = maskp[:, 896:1024]          # [128,128] ones (LN stats lhsT)
        ones64 = maskp[64:65, 1024:1088]    # [1,64] ones at partition 64
        cst = pers.tile([128, 25], F32, tag="cst")
        bq01, bq2 = cst[:, 0:1], cst[0:64, 1:2]
        bk01, bk2 = cst[:, 2:3], cst[0:64, 3:4]
        brk01, brk2 = cst[:, 4:5], cst[0:64, 5:6]
        bp4, bfc, bmp4 = cst[:, 6:12], cst[:, 12:18], cst[:, 18:24]
        eps_c = cst[:, 24:25]
        rows16 = pers.tile([1, 1856], BF16, tag="rows16")
        bv_r, brv_r = rows16[:, 0:FG], rows16[:, FG:2 * FG]
        oner16 = rows16[:, 384:512]
        crow = rows16[:, 512:1856]
        mr1h = pers.tile([1, TOKP], BF16, tag="mr1h")
        mr2h = pers.tile([1, TOKP], BF16, tag="mr2h")

        # ---- input DMAs (mask consts + hT block 0 first: LN1 starts early) -
        pin = lambda t: t.rearrange("(o p) n -> p o n", p=128)
        nc.sync.dma_start(out=maskp[:], in_=maskp_d[:])
        nc.sync.dma_start(out=h_main[:, 0:3, ds(0, 512)],
                          in_=pin(hT_d)[:, 0:3, ds(0, 512)])
        nc.sync.dma_start(out=h_main[:, 3:6, ds(0, 512)],
                          in_=pin(hT_d)[:, 3:6, ds(0, 512)])
        nc.sync.dma_start(out=cst[:], in_=cst_d[:])
        for bs, bl in BLKS[1:]:
            nc.sync.dma_start(out=h_main[:, :, ds(bs, bl)],
                              in_=pin(hT_d)[:, :, ds(bs, bl)])
        nc.sync.dma_start(out=rows16[:], in_=rows16_d[:])
        nc.sync.dma_start(out=ctx[:], in_=pin(ctxT_d))
        nc.sync.dma_start(out=wqkv[:], in_=pin(wqkv_d))
        nc.sync.dma_start(out=wp_sb[:], in_=wp_d.rearrange("p (h n) -> p h n",
                                                           h=HG))
        nc.sync.dma_start(out=wfc_sb[:], in_=pin(wfc_d))
        nc.sync.dma_start(out=wmp_sb[:], in_=pin(wmp_d))
        nc.vector.memset(kf01[:, ds(SC + TOK, KEYSP - SC - TOK)], 0.0)
        nc.vector.memset(kf2[:, ds(SC + TOK, KEYSP - SC - TOK)], 0.0)
        nc.vector.memset(vsb[:, NK - 1, :, :], 0.0)
        nc.vector.memset(vsb[:, :, :, DH:DH + 1], 1.0)

        # ---- LayerNorm (normalize only; affine folded into next weights).
        # dst = src * rsqrt(var+eps); mr row = mean*rstd (for the rank-1
        # -mean correction in consuming matmuls). Stats land replicated on
        # all 128 partitions via the onesq matmul, so everything downstream
        # is a cheap elementwise op.
        def ln_stats(src, bs, bl):
            sl = ds(bs, bl)
            ps_m = psum.tile([128, 512], F32, tag="pa", bufs=2)
            ps_s = psum.tile([128, 512], F32, tag="pj", bufs=2)
            for o in range(KO):
                sq = trans.tile([128, 512], BF16, tag="sq", bufs=3)
                nc.vector.tensor_mul(sq[:, :bl], src[:, o, sl],
                                     src[:, o, sl])
                nc.tensor.matmul(ps_m[:, :bl], onesq, src[:, o, sl],
                                 start=(o == 0), stop=(o == KO - 1))
                nc.tensor.matmul(ps_s[:, :bl], onesq, sq[:, :bl],
                                 start=(o == 0), stop=(o == KO - 1))
            return ps_m, ps_s

        def ln_chain(src, dst, mr_b16, bs, bl, ps_m, ps_s, dst_scale=None):
            sl = ds(bs, bl)
            # the mean row leaves PSUM first so ps_m recycles early
            mrow = trans.tile([1, 512], F32, tag="mrow", bufs=2)
            nc.scalar.activation(mrow[:, :bl], ps_m[0:1, :bl], AF.Identity,
                                 scale=1.0 / D)
            m2 = trans.tile([128, 512], F32, tag="m2", bufs=2)
            nc.scalar.activation(m2[:, :bl], ps_m[:, :bl], AF.Square,
                                 scale=1.0 / D)
            var = trans.tile([128, 512], F32, tag="var", bufs=2)
            nc.vector.scalar_tensor_tensor(
                out=var[:, :bl], in0=ps_s[:, :bl], scalar=1.0 / D,
                in1=m2[:, :bl], op0=ALU.mult, op1=ALU.subtract)
            sd = trans.tile([128, 512], F32, tag="sd", bufs=2)
            nc.scalar.activation(sd[:, :bl], var[:, :bl], AF.Sqrt,
                                 bias=eps_c)
            rstd = trans.tile([128, 512], BF16, tag="rstd", bufs=2)
            with nc.allow_low_precision(reason="bf16 rstd: ~0.4% scale"):
                nc.vector.reciprocal(rstd[:, :bl], sd[:, :bl])
            nc.vector.tensor_mul(mr_b16[:, sl], mrow[:, :bl],
                                 rstd[0:1, :bl])
            for o in range(KO):
                if dst_scale is None:
                    nc.vector.tensor_mul(dst[:, o, sl], src[:, o, sl],
                                         rstd[:, :bl])
                else:
                    nc.vector.scalar_tensor_tensor(
                        out=dst[:, o, sl], in0=src[:, o, sl],
                        scalar=dst_scale, in1=rstd[:, :bl], op0=ALU.mult,
                        op1=ALU.mult)

        # stats b0, b1 fill the 2-deep psum pools; each chain is emitted
        # right before the allocation that recycles its stats buffers
        hl = pers.tile([128, KO, TOKP], F8, tag="hl8")
        s0 = ln_stats(h_main, *BLKS[0])
        s1 = ln_stats(h_main, *BLKS[1])
        ln_chain(h_main, hl, mr1h, *BLKS[0], *s0, dst_scale=8.0)
        s2 = ln_stats(h_main, *BLKS[2])
        ln_chain(h_main, hl, mr1h, *BLKS[1], *s1, dst_scale=8.0)
        ln_chain(h_main, hl, mr1h, *BLKS[2], *s2, dst_scale=8.0)

        # ---- QKV projections (feature-major q/k; token-major v) ----
        def qk_proj(wcol, b01, b2, dst01, dst2, dst_off, src, src_blks,
                    corr, mr):
            # dst[mi] = w[:, :, mi].T @ src + bias, written at dst col offset.
            # fp8 DoubleRow: K=256 per pass (sub-tile pairs), psum carries
            # 128x the real value (weights x16, activations x8); the ACT
            # move rescales by 1/128. The rank-1 -mean correction rides the
            # same psum in the x128 domain (crow pre-scaled on host).
            for dst_t, bias_t, m0, ml in [(dst01, b01, 0, 128),
                                          (dst2, b2, 128, 64)]:
                for bs, bl in src_blks:
                    ps = psum.tile([128, 512], F32, tag="pa", bufs=2)
                    for o in range(0, KO, 2):
                        nc.tensor.matmul(
                            ps[:ml, :bl],
                            wqkv[:, o:o + 2, ds(wcol + m0, ml)],
                            src[:, o:o + 2, ds(bs, bl)],
                            start=(o == 0),
                            stop=(corr is None and o == KO - 2),
                            perf_mode=DR,
                        )
                    if corr is not None:
                        nc.tensor.matmul(
                            ps[:ml, :bl], crow[0:1, ds(corr + m0, ml)],
                            mr[0:1, ds(bs, bl)], start=False, stop=True,
                        )
                    nc.scalar.activation(
                        dst_t[:ml, ds(dst_off + bs, bl)], ps[:ml, :bl],
                        AF.Identity, bias=bias_t, scale=1.0 / 128)

        def v_proj(wcol, b_row, src, tts, kt_base, corr):
            # V[token, feat] = src.T @ w + 1 (x) bias  (token-major output,
            # fp8 DoubleRow in the x128 domain; bias row pre-scaled x128)
            for tt in tts:
                tw = 1 if kt_base + tt == NK - 1 else 128  # real tokens
                tsl = ds(tt * 128, tw)
                ps = psum.tile([128, 512], F32, tag="pj", bufs=2)
                for o in range(0, KO, 2):
                    nc.tensor.matmul(
                        ps[:tw, :FG], src[:, o:o + 2, tsl],
                        wqkv[:, o:o + 2, ds(wcol, FG)],
                        start=(o == 0), stop=False, perf_mode=DR,
                    )
                if corr is not None:
                    nc.tensor.matmul(ps[:tw, :FG], mr1h[0:1, tsl],
                                     crow[0:1, ds(corr, FG)],
                                     start=False, stop=False)
                nc.tensor.matmul(ps[:tw, :FG], oner16[0:1, 0:tw], b_row,
                                 start=False, stop=True)
                nc.vector.tensor_scalar_mul(
                    vsb[:tw, kt_base + tt, :, 0:DH],
                    ps[:tw, 0:FG].rearrange("p (h d) -> p h d", h=HG),
                    1.0 / 128)

        # context K/V (keys 0..255) -- needed by every query block
        qk_proj(3 * FG, brk01, brk2, kf01, kf2, 0, ctx, [(0, 256)],
                None, None)
        v_proj(4 * FG, brv_r, ctx, range(2), 0, None)

        # ---- attention + per-block proj/AllReduce pipeline ----
        kf_of = [(kf01, 0), (kf01, 64), (kf2, 0)]
        q_of = [(q01, 0), (q01, 64), (q2, 0)]
        dar_in, dar_out = [], []
        for bi, (bs, bl) in enumerate(BLKS):
            dar_in.append(dram.tile([128, KO, bl], BF16,
                                    name=f"dar_in{bi}"))
            dar_out.append(dram.tile([128, KO, bl], BF16,
                                     name=f"dar_out{bi}"))

        def attn_block(bi):
            qs, ql = BLKS[bi]
            qb0 = qs // 128                      # first 128-query tile
            last_kt = min((qs + ql - 1) // 128 + 2, NK - 1)
            # per key tile: column offset of the first not-fully-masked query
            # (query qs+c attends key j iff j <= 256 + qs + c, so tile kt is
            # all-masked for c < 128*(kt-qb0-2) and triangular for the next
            # 128 columns -- those columns are simply not computed, which is
            # exactly equivalent to multiplying by 0)
            def col0(kt):
                return min(max(0, 128 * (kt - qb0 - 2)), ql)
            # greedy-pack key-tile strips into [128,1024] PSUM groups
            strips = [(kt, col0(kt), ql - col0(kt))
                      for kt in range(last_kt + 1) if ql - col0(kt) > 0]
            groups, off = [], 0
            for kt, co, w in strips:
                if not groups or off + w > 1024:
                    groups.append([])
                    off = 0
                groups[-1].append((kt, co, w, off))
                off += w

            def head_scores(h, ps_a):
                kf_t, kf_o = kf_of[h]
                q_t, q_o = q_of[h]
                for g in groups:
                    ps_s = psum.tile([128, 1024], F32, tag="sc", bufs=2,
                                     name="ps_s")
                    for kt, co, w, off in g:
                        nc.tensor.matmul(
                            ps_s[:, ds(off, w)],
                            kf_t[kf_o:kf_o + DH, ts(kt, 128)],
                            q_t[q_o:q_o + DH, ds(qs + co, w)],
                            start=True, stop=True,
                        )
                    expt = trans.tile([128, 1024], BF16, tag="expt", bufs=3)
                    nw = g[-1][3] + g[-1][2]  # last strip's off + w
                    nc.scalar.activation(expt[:, :nw], ps_s[:, :nw], AF.Exp,
                                         scale=0.125)
                    for kt, co, w, off in g:
                        if kt >= qb0 + 2:
                            # triangle starts at strip col 128*dlt - co
                            dlt = kt - qb0 - 2
                            t0 = 128 * dlt - co
                            tw = min(128, w - t0)
                            nc.vector.tensor_mul(
                                expt[:, ds(off + t0, tw)],
                                expt[:, ds(off + t0, tw)],
                                mask_sb[:, ds(384, tw)])
                            if t0 > 0:  # fully-masked cols below the triangle
                                nc.vector.tensor_mul(
                                    expt[:, ds(off, t0)],
                                    expt[:, ds(off, t0)],
                                    mask_sb[:, ds(0, t0)])
                        nc.tensor.matmul(
                            ps_a[:, ds(co, w)], vsb[:, kt, h, :],
                            expt[:, ds(off, w)],
                            start=(kt == 0), stop=(kt == last_kt),
                        )

            def head_norm(h, ps_a):
                # a = num * (1/den); den sits at partition 64, broadcast to
                # partitions 0..63 via a K=1 PE outer product
                r16 = trans.tile([128, 512], BF16, tag="r16", bufs=2)
                with nc.allow_low_precision(reason="bf16 1/den: ~0.4%"):
                    nc.vector.reciprocal(r16[DH:DH + 1, :ql],
                                         ps_a[DH:DH + 1, :ql])
                ps_r = psum.tile([128, 512], F32, tag="pj", bufs=2,
                                 name="ps_r")[0:DH]
                nc.tensor.matmul(ps_r[:, :ql], ones64,
                                 r16[DH:DH + 1, :ql], start=True, stop=True)
                recs = trans.tile([128, 512], BF16, tag="recs", bufs=2)
                nc.vector.tensor_copy(recs[0:DH, :ql], ps_r[:, :ql])
                nc.vector.scalar_tensor_tensor(
                    out=a_all[:, h, ds(qs, ql)], in0=ps_a[0:DH, :ql],
                    scalar=16.0, in1=recs[0:DH, :ql], op0=ALU.mult,
                    op1=ALU.mult)

            if ql > 1:
                for h in range(HG):
                    ps_a = psum.tile([128, 512], F32, tag="pa", bufs=2,
                                     name="ps_a")[0:DH + 1]
                    head_scores(h, ps_a)
                    head_norm(h, ps_a)
            else:
                for h in range(HG):
                    ps_a = psum.tile([128, 512], F32, tag="pa", bufs=2,
                                     name="ps_a")[0:DH + 1]
                    head_scores(h, ps_a)
                    head_norm(h, ps_a)

            # attention output projection for this token block (row-sharded);
            # residual pre-folded as +h/4 so the AllReduce output IS new h.
            # high_priority: this chain feeds the AllReduce that LN2 waits on,
            # so its ACT/DVE ops must not queue behind the next block's exps.
            armo = trans.tile([128, KO, 512], BF16, tag="armo", bufs=2)
            with tc.high_priority():
                for mo in range(KO):
                    ps = psum.tile([128, 512], F32, tag="pj", bufs=2)
                    nc.tensor.matmul(ps[:, :ql], wp_sb[:, 0:2, ts(mo, 128)],
                                     a_all[:, 0:2, ds(qs, ql)],
                                     start=True, stop=False, perf_mode=DR)
                    nc.tensor.matmul(ps[:, :ql], wp_sb[:, 2, ts(mo, 128)],
                                     a_all[:, 2, ds(qs, ql)],
                                     start=False, stop=True)
                    ptmp = trans.tile([128, 512], BF16, tag="ptmp", bufs=2)
                    nc.scalar.activation(ptmp[:, :ql], ps[:, :ql],
                                         AF.Identity, bias=bp4[:, mo:mo + 1],
                                         scale=1.0 / 256)
                    nc.vector.scalar_tensor_tensor(
                        out=armo[:, mo, :ql], in0=h_main[:, mo, ds(qs, ql)],
                        scalar=0.25, in1=ptmp[:, :ql], op0=ALU.mult,
                        op1=ALU.add)
                nc.sync.dma_start(dar_in[bi][:], armo[:, :, :ql])
            if sim_collectives:
                nc.gpsimd.collective_compute(
                    "AllReduce", ALU.add, replica_groups=GROUPS,
                    ins=[dar_in[bi].opt()], outs=[dar_out[bi].opt()],
                )
                # h_main <- allreduced (h + attn out), in place
                nc.gpsimd.dma_start(h_main[:, :, ds(qs, ql)], dar_out[bi][:])
            else:
                # local stand-in: the payload makes the same SBUF->DRAM->SBUF
                # round trip a real AllReduce result would
                nc.gpsimd.dma_start(h_main[:, :, ds(qs, ql)], dar_in[bi][:])

        # software pipeline: each query block's attention starts as soon as
        # its own q/k/v slices exist (qb0 needs only block-0 projections +
        # context KV), so attention overlaps the remaining projections
        VT = [range(0, 4), range(4, 8), range(8, 9)]  # v tiles per block
        for bi in (0, 1, 2):
            qk_proj(0, bq01, bq2, q01, q2, 0, hl, [BLKS[bi]], 0, mr1h)
            qk_proj(FG, bk01, bk2, kf01, kf2, SC, hl, [BLKS[bi]], FG, mr1h)
            v_proj(2 * FG, bv_r, hl, VT[bi], 2, 2 * FG)
            attn_block(bi)

        # ---- LN2 + MLP ----
        z0 = pers.tile([128, KO, TOKP], BF16, tag="ln_out")
        t0 = ln_stats(h_main, *BLKS[0])
        t1 = ln_stats(h_main, *BLKS[1])
        ln_chain(h_main, z0, mr2h, *BLKS[0], *t0)
        t2 = ln_stats(h_main, *BLKS[2])
        ln_chain(h_main, z0, mr2h, *BLKS[1], *t1)
        ln_chain(h_main, z0, mr2h, *BLKS[2], *t2)

        def fc_block(bs, bl):
            for mo in range(KO):
                ps = psum.tile([128, 512], F32, tag="pa", bufs=2)
                for o in range(KO):
                    nc.tensor.matmul(
                        ps[:, :bl], wfc_sb[:, o, ts(mo, 128)],
                        z0[:, o, ds(bs, bl)],
                        start=(o == 0), stop=False,
                    )
                nc.tensor.matmul(
                    ps[:, :bl], crow[0:1, ds(3 * FG + mo * 128, 128)],
                    mr2h[0:1, ds(bs, bl)], start=False, stop=True,
                )
                nc.scalar.activation(gl[:, mo, ds(bs, bl)], ps[:, :bl],
                                     gelu_fn, bias=bfc[:, mo:mo + 1])

        drs_in = [dram.tile([KO, 128, bl], BF16, name=f"drs_in{i}")
                  for i, (_, bl) in enumerate(BLKS)]
        drs_out = [dram.tile([FG, bl], BF16, name=f"drs_out{i}")
                   for i, (_, bl) in enumerate(BLKS)]

        def mp_block(bi):
            bs, bl = BLKS[bi]
            mpart = trans.tile([128, KO, 512], BF16, tag="armo", bufs=2)
            for mo in range(KO):
                ps = psum.tile([128, 512], F32, tag="pj", bufs=2)
                for o in range(KO):
                    nc.tensor.matmul(
                        ps[:, :bl], wmp_sb[:, o, ts(mo, 128)],
                        gl[:, o, ds(bs, bl)],
                        start=(o == 0), stop=(o == KO - 1),
                    )
                mtmp = trans.tile([128, 512], BF16, tag="ptmp", bufs=2)
                with tc.high_priority():
                    nc.scalar.activation(mtmp[:, :bl], ps[:, :bl],
                                         AF.Identity,
                                         bias=bmp4[:, mo:mo + 1])
                    # mpart = h_main/4 + (mproj partial + b_mproj/4)
                    nc.vector.scalar_tensor_tensor(
                        out=mpart[:, mo, :bl], in0=h_main[:, mo, ds(bs, bl)],
                        scalar=0.25, in1=mtmp[:, :bl], op0=ALU.mult,
                        op1=ALU.add)
            if sim_collectives:
                nc.sync.dma_start(
                    drs_in[bi].rearrange("o p n -> p o n"), mpart[:, :, :bl])
                nc.gpsimd.collective_compute(
                    "ReduceScatter", ALU.add, replica_groups=GROUPS,
                    ins=[drs_in[bi].opt()], outs=[drs_out[bi].opt()],
                )
                nc.sync.dma_start(out_d[:, ds(bs, bl)], drs_out[bi][:])
            else:
                # local stand-in for ReduceScatter: this core's output slice
                # comes straight from SBUF (a real RS also lands in out_d)
                nc.gpsimd.dma_start(out_d[0:128, ds(bs, bl)],
                                    mpart[:, 0, :bl])
                nc.gpsimd.dma_start(out_d[128:FG, ds(bs, bl)],
                                    mpart[0:64, 1, :bl])

        for bi in range(3):
            fc_block(*BLKS[bi])
            mp_block(bi)

        if debug_taps:
            for nm, t in [("dbg_hl", hl), ("dbg_q01", q01),
                          ("dbg_kf01", kf01), ("dbg_vsb", vsb),
                          ("dbg_a", a_all), ("dbg_h", h_main),
                          ("dbg_z0", z0), ("dbg_gl", gl),
                          ("dbg_dar_in0", dar_in[0]),
                          ("dbg_dar_out0", dar_out[0])]:
                d = nc.dram_tensor(nm, list(t.shape), t.dtype,
                                   kind="ExternalOutput")
                nc.sync.dma_start(d[:], t[:])

    nc.compile()
    return nc


_NC_CACHE = None


def _get_program():
    global _NC_CACHE
    if _NC_CACHE is None:
        _NC_CACHE = build_program()
    return _NC_CACHE


def make_in_maps(inputs):
    f = lambda a: np.asarray(a, dtype=np.float32)
    bf = ml_dtypes.bfloat16
    x = f(inputs["x"])
    context_seq = f(inputs["context_seq"])
    sos_h = f(inputs["sos_h"])
    g1, b1 = f(inputs["ln1_g"]), f(inputs["ln1_b"])
    W_attn, b_attn = f(inputs["W_attn"]), f(inputs["b_attn"])
    W_ref, b_ref = f(inputs["W_ref"]), f(inputs["b_ref"])
    W_proj, b_proj = f(inputs["W_proj"]), f(inputs["b_proj"])
    g2, b2 = f(inputs["ln2_g"]), f(inputs["ln2_b"])
    W_fc, b_fc = f(inputs["W_fc"]), f(inputs["b_fc"])
    W_mproj, b_mproj = f(inputs["W_mproj"]), f(inputs["b_mproj"])

    # master causal mask: mask[p, c] = 1 iff p <= c - 384
    cix = np.arange(896)[None, :]
    pix = np.arange(128)[:, None]
    mask = (pix <= cix - 384).astype(np.float32)
    maskp = np.ones((128, 1088), np.float32)
    maskp[:, 0:896] = mask

    wfc_g = W_fc * g2[:, None]
    bfc_full = b2 @ W_fc + b_fc

    in_maps = []
    for core in range(N_CORES):
        b, g = core // TP, core % TP
        h = np.concatenate([sos_h[None, :], x[b]], axis=0)  # [1025, 768]
        hT = np.zeros((D, TOKP), bf)
        hT[:, :TOK] = h.T.astype(bf)
        qsl = slice(FG * g, FG * (g + 1))
        ksl = slice(D + FG * g, D + FG * (g + 1))
        vsl = slice(2 * D + FG * g, 2 * D + FG * (g + 1))
        rks = slice(FG * g, FG * (g + 1))
        rvs = slice(D + FG * g, D + FG * (g + 1))
        mcols = slice(D * g, D * (g + 1))    # W_fc column slice (768 per core)
        f8 = ml_dtypes.float8_e4m3fn
        wq = W_attn[:, qsl] * g1[:, None]
        wk = W_attn[:, ksl] * g1[:, None]
        wv = W_attn[:, vsl] * g1[:, None]
        # fp8 weights carry a x16 scale; activations x8; the psum is x128
        wqkv8 = np.concatenate(
            [wq, wk, wv, W_ref[:, rks], W_ref[:, rvs]],
            axis=1).astype(np.float32)
        wqkv8 = (16.0 * wqkv8).astype(f8)
        wp_slab = W_proj[FG * g:FG * (g + 1), :]   # [192, 768]
        wp = np.concatenate([wp_slab[0:64], wp_slab[64:128],
                             wp_slab[128:192]], axis=1)  # [64, 2304]
        wp8 = (16.0 * wp).astype(f8)
        cst = np.zeros((128, 25), np.float32)
        bq = b1 @ W_attn[:, qsl] + b_attn[qsl]
        bk = b1 @ W_attn[:, ksl] + b_attn[ksl]
        cst[:, 0] = bq[0:128]
        cst[0:64, 1] = bq[128:192]
        cst[:, 2] = bk[0:128]
        cst[0:64, 3] = bk[128:192]
        cst[:, 4] = b_ref[rks][0:128]
        cst[0:64, 5] = b_ref[rks][128:192]
        cst[:, 6:12] = (b_proj / TP).reshape(6, 128).T
        cst[:, 12:18] = bfc_full[mcols].reshape(6, 128).T
        cst[:, 18:24] = (b_mproj / TP).reshape(6, 128).T
        cst[:, 24] = EPS
        # colsums of the QUANTIZED weights, in the x128 psum domain:
        # corr = -128*colsum(w8/16)*mr = -8*colsum(w8)*mr
        w8f = wqkv8.astype(np.float32)
        rows16 = np.zeros((1, 1856), np.float32)
        rows16[0, 0:FG] = 128.0 * (b1 @ W_attn[:, vsl] + b_attn[vsl])
        rows16[0, FG:2 * FG] = 128.0 * b_ref[rvs]
        rows16[0, 384:512] = 1.0
        rows16[0, 512:1856] = np.concatenate([
            -8.0 * w8f[:, 0:FG].sum(0), -8.0 * w8f[:, FG:2 * FG].sum(0),
            -8.0 * w8f[:, 2 * FG:3 * FG].sum(0),
            -wfc_g[:, mcols].sum(0)])
        in_maps.append({
            "hT": hT,
            "ctxT": (8.0 * np.ascontiguousarray(context_seq[b].T)
                     ).astype(f8),
            "wqkv": np.ascontiguousarray(wqkv8),
            "wp": np.ascontiguousarray(wp8),
            "wfc": np.ascontiguousarray(wfc_g[:, mcols]).astype(bf),
            "wmp": np.ascontiguousarray(W_mproj[mcols, :]).astype(bf),
            "cst": cst,
            "rows16": rows16.astype(bf),
            "maskp": maskp.astype(bf),
        })
    return in_maps


def assemble_output(results, B=2):
    out = np.empty((B, S, D), np.float32)
    for b in range(B):
        parts = [np.asarray(results[TP * b + g]["out"], np.float32)
                 for g in range(TP)]
        full = np.concatenate(parts, axis=0)  # [768, 1025]
        out[b] = full[:, 1:TOK].T
    return out


def kernel(**inputs):
    nc = _get_program()
    in_maps = make_in_maps(inputs)
    res = run_bass_kernel_spmd(nc, in_maps, list(range(N_CORES)))
    return assemble_output(res.results, B=np.asarray(inputs["x"]).shape[0])


if __name__ == "__main__":
    import reference
    ins = reference.setup_inputs()
    ins = {k: np.asarray(v) for k, v in ins.items()}
    got = kernel(**ins)
    exp = np.asarray(reference.reference(**ins))
    err = np.abs(got - exp).max() / np.abs(exp).max()
    print("max abs err:", np.abs(got - exp).max(), "rel:", err)


# revision 51
# speedup vs baseline: 2.0006x; 1.0575x over previous
"""CoconBlock forward on 8 Trainium2 NeuronCores.

Sharding: core c = (b, g) with b = c // 4 (batch), g = c % 4 (tensor-parallel
rank). Within each batch group of 4 cores:
  - attention QKV / context-KV weights column-sharded by head group (3 heads),
  - W_proj row-sharded, partial outputs AllReduced per token block,
  - MLP W_fc column-sharded / W_mproj row-sharded, partial outputs (with the
    residual and bias pre-folded as +h/4 + b_mproj/4 per core) ReduceScattered
    so each core lands exactly its 192-feature slice of the final output.

All on-device activations are feature-on-partition (f32 has no DMA transpose;
this layout makes every matmul transpose-free). Token axis is processed in
blocks (0,512),(512,512),(1024,1) -- no padded-token compute. LayerNorm
reductions over the feature (partition) axis run on the PE via a [128,128]
ones matmul that leaves the statistics replicated across all partitions (PE
matmul cost depends only on the output free size, so replication is free);
the normalize multiplies are then cheap all-SBUF bf16 DVE ops and no
broadcast matmul is needed. The LN affine (gamma/beta) is folded into the
following weight matrix on the host; the -mean*rstd term is folded into the
consuming matmuls as a rank-1 correction (colsum(W) x mr).

Engine balance: every PSUM->SBUF move that carries a per-partition bias runs
on ACT (Identity with bias AP); exp/gelu/sqrt/square on ACT; masks, rstd
reciprocal, residual scalar_tensor_tensor folds on DVE.

Attention uses the 128-aligned causal structure: with keys (256 context +
1025 self) padded to 1408, a (query-block, key-tile) pair is fully allowed,
partially masked by a shifted-triangle slice of one master mask, or skipped.
Scores for a query block are exp'ed in up-to-2-key-tile groups out of a
[128,1024] PSUM slab; the softmax denominator rides the attend matmul as a
ones-column appended to V (partition 64), and 1/den is broadcast to
partitions 0..63 with a K=1 PE outer product.

The attention output projection and its AllReduce run per token block,
pipelined against the remaining attention blocks; the residual is pre-folded
(h/4 per core) so the AllReduce result IS the new h, DMAed straight back
into h_main.
"""

import sys

sys.path.insert(0, "/opt/trn_rl_repo")

import ml_dtypes
import numpy as np

import concourse.bass as bass
import concourse.bacc as bacc
import concourse.mybir as mybir
import concourse.tile as tile
from concourse.bass_utils import run_bass_kernel_spmd

F32 = mybir.dt.float32
AF = mybir.ActivationFunctionType
ALU = mybir.AluOpType
ts, ds = bass.ts, bass.ds

D = 768
DH = 64
S = 1024
SC = 256
TOK = S + 1            # 1025 (sos + x)
TOKP = 1152            # tile column capacity (only 0..1024 computed)
KEYSP = 1408           # 11 * 128
NK = KEYSP // 128      # 11
KO = D // 128          # 6 feature sub-tiles
TP = 4
FG = 192               # features per core in head-sharded tensors (3 heads)
HG = 3                 # heads per core
EPS = 1e-5
N_CORES = 8
GROUPS = [[0, 1, 2, 3], [4, 5, 6, 7]]
BLKS = [(0, 512), (512, 512), (1024, 1)]  # token blocks (start, len)

BF16 = mybir.dt.bfloat16
F8 = mybir.dt.float8e4
DR = mybir.MatmulPerfMode.DoubleRow


def build_program(sim_collectives=True, gelu_fn=None, debug_taps=False):
    if gelu_fn is None:
        gelu_fn = AF.Gelu_apprx_tanh
    nc = bacc.Bacc(None, num_devices=N_CORES)

    # ---- DRAM I/O ----
    hT_d = nc.dram_tensor("hT", [D, TOKP], BF16, kind="ExternalInput")
    ctxT_d = nc.dram_tensor("ctxT", [D, SC], F8, kind="ExternalInput")
    wqkv_d = nc.dram_tensor("wqkv", [D, 5 * FG], F8, kind="ExternalInput")
    wp_d = nc.dram_tensor("wp", [DH, 3 * D], F8, kind="ExternalInput")
    wfc_d = nc.dram_tensor("wfc", [D, D], BF16, kind="ExternalInput")
    wmp_d = nc.dram_tensor("wmp", [D, D], BF16, kind="ExternalInput")
    cst_d = nc.dram_tensor("cst", [128, 25], F32, kind="ExternalInput")
    rows16_d = nc.dram_tensor("rows16", [1, 1856], BF16, kind="ExternalInput")
    maskp_d = nc.dram_tensor("maskp", [128, 1088], BF16, kind="ExternalInput")
    out_d = nc.dram_tensor("out", [FG, TOK], BF16, kind="ExternalOutput")

    with tile.TileContext(nc) as tc, \
         tc.tile_pool(name="pers", bufs=1) as pers, \
         tc.tile_pool(name="trans", bufs=2) as trans, \
         tc.tile_pool(name="psum", bufs=1, space="PSUM") as psum, \
         tc.tile_pool(name="dram", bufs=1, space="DRAM") as dram:

        # ---- resident SBUF tensors ----
        h_main = pers.tile([128, KO, TOKP], BF16, tag="h_main")
        ctx = pers.tile([128, KO, SC], F8, tag="ctx")
        wqkv = pers.tile([128, KO, 5 * FG], F8, tag="wqkv")
        wp_sb = pers.tile([DH, HG, D], F8, tag="wp")
        wfc_sb = pers.tile([128, KO, D], BF16, tag="wfc")
        wmp_sb = pers.tile([128, KO, D], BF16, tag="wmp")
        kf01 = pers.tile([128, KEYSP], BF16, tag="kf01")
        kf2 = pers.tile([64, KEYSP], BF16, tag="kf2")
        q01 = pers.tile([128, TOKP], BF16, tag="q01")
        q2 = pers.tile([64, TOKP], BF16, tag="q2")
        vsb = pers.tile([128, NK, HG, DH + 1], BF16, tag="vsb")
        a_all = pers.tile([DH, HG, TOKP], F8, tag="a_all")
        gl = pers.tile([128, KO, TOKP], BF16, tag="gl")
        maskp = pers.tile([128, 1088], BF16, tag="maskp")
        mask_sb = maskp[:, 0:896]
        onesq = maskp[:, 896:1024]          # [128,128] ones (LN stats lhsT)
        ones64 = maskp[64:65, 1024:1088]    # [1,64] ones at partition 64
        cst = pers.tile([128, 25], F32, tag="cst")
        bq01, bq2 = cst[:, 0:1], cst[0:64, 1:2]
        bk01, bk2 = cst[:, 2:3], cst[0:64, 3:4]
        brk01, brk2 = cst[:, 4:5], cst[0:64, 5:6]
        bp4, bfc, bmp4 = cst[:, 6:12], cst[:, 12:18], cst[:, 18:24]
        eps_c = cst[:, 24:25]
        rows16 = pers.tile([1, 1856], BF16, tag="rows16")
        bv_r, brv_r = rows16[:, 0:FG], rows16[:, FG:2 * FG]
        oner16 = rows16[:, 384:512]
        crow = rows16[:, 512:1856]
        mr1h = pers.tile([1, TOKP], BF16, tag="mr1h")
        mr2h = pers.tile([1, TOKP], BF16, tag="mr2h")

        # ---- input DMAs (mask consts + hT block 0 first: LN1 starts early) -
        pin = lambda t: t.rearrange("(o p) n -> p o n", p=128)
        nc.sync.dma_start(out=maskp[:], in_=maskp_d[:])
        nc.sync.dma_start(out=h_main[:, 0:3, ds(0, 512)],
                          in_=pin(hT_d)[:, 0:3, ds(0, 512)])
        nc.sync.dma_start(out=h_main[:, 3:6, ds(0, 512)],
                          in_=pin(hT_d)[:, 3:6, ds(0, 512)])
        nc.sync.dma_start(out=cst[:], in_=cst_d[:])
        for bs, bl in BLKS[1:]:
            nc.sync.dma_start(out=h_main[:, :, ds(bs, bl)],
                              in_=pin(hT_d)[:, :, ds(bs, bl)])
        nc.sync.dma_start(out=rows16[:], in_=rows16_d[:])
        nc.sync.dma_start(out=ctx[:], in_=pin(ctxT_d))
        nc.sync.dma_start(out=wqkv[:], in_=pin(wqkv_d))
        nc.sync.dma_start(out=wp_sb[:], in_=wp_d.rearrange("p (h n) -> p h n",
                                                           h=HG))
        nc.sync.dma_start(out=wfc_sb[:], in_=pin(wfc_d))
        nc.sync.dma_start(out=wmp_sb[:], in_=pin(wmp_d))
        nc.vector.memset(kf01[:, ds(SC + TOK, KEYSP - SC - TOK)], 0.0)
        nc.vector.memset(kf2[:, ds(SC + TOK, KEYSP - SC - TOK)], 0.0)
        nc.vector.memset(vsb[:, NK - 1, :, :], 0.0)
        nc.vector.memset(vsb[:, :, :, DH:DH + 1], 1.0)

        # ---- LayerNorm (normalize only; affine folded into next weights).
        # dst = src * rsqrt(var+eps); mr row = mean*rstd (for the rank-1
        # -mean correction in consuming matmuls). Stats land replicated on
        # all 128 partitions via the onesq matmul, so everything downstream
        # is a cheap elementwise op.
        def ln_stats(src, bs, bl):
            sl = ds(bs, bl)
            ps_b = psum.tile([128, 1024], F32, tag="sc", bufs=2)
            ps_m, ps_s = ps_b[:, 0:512], ps_b[:, 512:1024]
            for o in range(KO):
                sq = trans.tile([128, 512], BF16, tag="sq", bufs=3)
                nc.vector.tensor_mul(sq[:, :bl], src[:, o, sl],
                                     src[:, o, sl])
                nc.tensor.matmul(ps_m[:, :bl], onesq, src[:, o, sl],
                                 start=(o == 0), stop=(o == KO - 1))
                nc.tensor.matmul(ps_s[:, :bl], onesq, sq[:, :bl],
                                 start=(o == 0), stop=(o == KO - 1))
            return ps_m, ps_s

        def ln_chain(src, dst, mr_b16, bs, bl, ps_m, ps_s, dst_scale=None):
            sl = ds(bs, bl)
            # the mean row leaves PSUM first so ps_m recycles early
            mrow = trans.tile([1, 512], F32, tag="mrow", bufs=2)
            nc.scalar.activation(mrow[:, :bl], ps_m[0:1, :bl], AF.Identity,
                                 scale=1.0 / D)
            m2 = trans.tile([128, 512], F32, tag="m2", bufs=2)
            nc.scalar.activation(m2[:, :bl], ps_m[:, :bl], AF.Square,
                                 scale=1.0 / D)
            var = trans.tile([128, 512], F32, tag="var", bufs=2)
            nc.vector.scalar_tensor_tensor(
                out=var[:, :bl], in0=ps_s[:, :bl], scalar=1.0 / D,
                in1=m2[:, :bl], op0=ALU.mult, op1=ALU.subtract)
            sd = trans.tile([128, 512], F32, tag="sd", bufs=2)
            nc.scalar.activation(sd[:, :bl], var[:, :bl], AF.Sqrt,
                                 bias=eps_c)
            rstd = trans.tile([128, 512], BF16, tag="rstd", bufs=2)
            with nc.allow_low_precision(reason="bf16 rstd: ~0.4% scale"):
                nc.vector.reciprocal(rstd[:, :bl], sd[:, :bl])
            nc.vector.tensor_mul(mr_b16[:, sl], mrow[:, :bl],
                                 rstd[0:1, :bl])
            for o in range(KO):
                if dst_scale is None:
                    nc.vector.tensor_mul(dst[:, o, sl], src[:, o, sl],
                                         rstd[:, :bl])
                else:
                    nc.vector.scalar_tensor_tensor(
                        out=dst[:, o, sl], in0=src[:, o, sl],
                        scalar=dst_scale, in1=rstd[:, :bl], op0=ALU.mult,
                        op1=ALU.mult)

        # stats b0, b1 fill the 2-deep psum pools; each chain is emitted
        # right before the allocation that recycles its stats buffers
        hl = pers.tile([128, KO, TOKP], F8, tag="hl8")
        s0 = ln_stats(h_main, *BLKS[0])
        s1 = ln_stats(h_main, *BLKS[1])
        ln_chain(h_main, hl, mr1h, *BLKS[0], *s0, dst_scale=8.0)
        s2 = ln_stats(h_main, *BLKS[2])
        ln_chain(h_main, hl, mr1h, *BLKS[1], *s1, dst_scale=8.0)
        ln_chain(h_main, hl, mr1h, *BLKS[2], *s2, dst_scale=8.0)

        # ---- QKV projections (feature-major q/k; token-major v) ----
        def qk_proj(wcol, b01, b2, dst01, dst2, dst_off, src, src_blks,
                    corr, mr):
            # dst[mi] = w[:, :, mi].T @ src + bias, written at dst col offset.
            # fp8 DoubleRow: K=256 per pass (sub-tile pairs), psum carries
            # 128x the real value (weights x16, activations x8); the ACT
            # move rescales by 1/128. The rank-1 -mean correction rides the
            # same psum in the x128 domain (crow pre-scaled on host).
            for dst_t, bias_t, m0, ml in [(dst01, b01, 0, 128),
                                          (dst2, b2, 128, 64)]:
                for bs, bl in src_blks:
                    ps = psum.tile([128, 512], F32, tag="pa", bufs=2)
                    for o in range(0, KO, 2):
                        nc.tensor.matmul(
                            ps[:ml, :bl],
                            wqkv[:, o:o + 2, ds(wcol + m0, ml)],
                            src[:, o:o + 2, ds(bs, bl)],
                            start=(o == 0),
                            stop=(corr is None and o == KO - 2),
                            perf_mode=DR,
                        )
                    if corr is not None:
                        nc.tensor.matmul(
                            ps[:ml, :bl], crow[0:1, ds(corr + m0, ml)],
                            mr[0:1, ds(bs, bl)], start=False, stop=True,
                        )
                    nc.scalar.activation(
                        dst_t[:ml, ds(dst_off + bs, bl)], ps[:ml, :bl],
                        AF.Identity, bias=bias_t, scale=1.0 / 128)

        def v_proj(wcol, b_row, src, tts, kt_base, corr):
            # V[token, feat] = src.T @ w + 1 (x) bias  (token-major output,
            # fp8 DoubleRow in the x128 domain; bias row pre-scaled x128)
            for tt in tts:
                tw = 1 if kt_base + tt == NK - 1 else 128  # real tokens
                tsl = ds(tt * 128, tw)
                ps = psum.tile([128, 512], F32, tag="pj", bufs=2)
                for o in range(0, KO, 2):
                    nc.tensor.matmul(
                        ps[:tw, :FG], src[:, o:o + 2, tsl],
                        wqkv[:, o:o + 2, ds(wcol, FG)],
                        start=(o == 0), stop=False, perf_mode=DR,
                    )
                if corr is not None:
                    nc.tensor.matmul(ps[:tw, :FG], mr1h[0:1, tsl],
                                     crow[0:1, ds(corr, FG)],
                                     start=False, stop=False)
                nc.tensor.matmul(ps[:tw, :FG], oner16[0:1, 0:tw], b_row,
                                 start=False, stop=True)
                nc.vector.tensor_scalar_mul(
                    vsb[:tw, kt_base + tt, :, 0:DH],
                    ps[:tw, 0:FG].rearrange("p (h d) -> p h d", h=HG),
                    1.0 / 128)

        # context K/V (keys 0..255) -- needed by every query block
        qk_proj(3 * FG, brk01, brk2, kf01, kf2, 0, ctx, [(0, 256)],
                None, None)
        v_proj(4 * FG, brv_r, ctx, range(2), 0, None)

        # ---- attention + per-block proj/AllReduce pipeline ----
        kf_of = [(kf01, 0), (kf01, 64), (kf2, 0)]
        q_of = [(q01, 0), (q01, 64), (q2, 0)]
        dar_in, dar_out = [], []
        for bi, (bs, bl) in enumerate(BLKS):
            dar_in.append(dram.tile([128, KO, bl], BF16,
                                    name=f"dar_in{bi}"))
            dar_out.append(dram.tile([128, KO, bl], BF16,
                                     name=f"dar_out{bi}"))

        def attn_block(bi):
            qs, ql = BLKS[bi]
            qb0 = qs // 128                      # first 128-query tile
            last_kt = min((qs + ql - 1) // 128 + 2, NK - 1)
            # per key tile: column offset of the first not-fully-masked query
            # (query qs+c attends key j iff j <= 256 + qs + c, so tile kt is
            # all-masked for c < 128*(kt-qb0-2) and triangular for the next
            # 128 columns -- those columns are simply not computed, which is
            # exactly equivalent to multiplying by 0)
            def col0(kt):
                return min(max(0, 128 * (kt - qb0 - 2)), ql)
            # greedy-pack key-tile strips into [128,1024] PSUM groups
            strips = [(kt, col0(kt), ql - col0(kt))
                      for kt in range(last_kt + 1) if ql - col0(kt) > 0]
            groups, off = [], 0
            for kt, co, w in strips:
                if not groups or off + w > 1024:
                    groups.append([])
                    off = 0
                groups[-1].append((kt, co, w, off))
                off += w

            def head_scores(h, ps_a):
                kf_t, kf_o = kf_of[h]
                q_t, q_o = q_of[h]
                for g in groups:
                    ps_s = psum.tile([128, 1024], F32, tag="sc", bufs=2,
                                     name="ps_s")
                    for kt, co, w, off in g:
                        nc.tensor.matmul(
                            ps_s[:, ds(off, w)],
                            kf_t[kf_o:kf_o + DH, ts(kt, 128)],
                            q_t[q_o:q_o + DH, ds(qs + co, w)],
                            start=True, stop=True,
                        )
                    expt = trans.tile([128, 1024], BF16, tag="expt", bufs=3)
                    nw = g[-1][3] + g[-1][2]  # last strip's off + w
                    nc.scalar.activation(expt[:, :nw], ps_s[:, :nw], AF.Exp,
                                         scale=0.125)
                    for kt, co, w, off in g:
                        if kt >= qb0 + 2:
                            # triangle starts at strip col 128*dlt - co
                            dlt = kt - qb0 - 2
                            t0 = 128 * dlt - co
                            tw = min(128, w - t0)
                            nc.vector.tensor_mul(
                                expt[:, ds(off + t0, tw)],
                                expt[:, ds(off + t0, tw)],
                                mask_sb[:, ds(384, tw)])
                            if t0 > 0:  # fully-masked cols below the triangle
                                nc.vector.tensor_mul(
                                    expt[:, ds(off, t0)],
                                    expt[:, ds(off, t0)],
                                    mask_sb[:, ds(0, t0)])
                        nc.tensor.matmul(
                            ps_a[:, ds(co, w)], vsb[:, kt, h, :],
                            expt[:, ds(off, w)],
                            start=(kt == 0), stop=(kt == last_kt),
                        )

            def head_norm(h, ps_a):
                # a = num * (1/den); den sits at partition 64, broadcast to
                # partitions 0..63 via a K=1 PE outer product
                r16 = trans.tile([128, 512], BF16, tag="r16", bufs=2)
                with nc.allow_low_precision(reason="bf16 1/den: ~0.4%"):
                    nc.vector.reciprocal(r16[DH:DH + 1, :ql],
                                         ps_a[DH:DH + 1, :ql])
                ps_r = psum.tile([128, 512], F32, tag="pj", bufs=2,
                                 name="ps_r")[0:DH]
                nc.tensor.matmul(ps_r[:, :ql], ones64,
                                 r16[DH:DH + 1, :ql], start=True, stop=True)
                recs = trans.tile([128, 512], BF16, tag="recs", bufs=2)
                nc.vector.tensor_copy(recs[0:DH, :ql], ps_r[:, :ql])
                nc.vector.scalar_tensor_tensor(
                    out=a_all[:, h, ds(qs, ql)], in0=ps_a[0:DH, :ql],
                    scalar=16.0, in1=recs[0:DH, :ql], op0=ALU.mult,
                    op1=ALU.mult)

            if ql > 1:
                for h in range(HG):
                    ps_a = psum.tile([128, 512], F32, tag="pa", bufs=2,
                                     name="ps_a")[0:DH + 1]
                    head_scores(h, ps_a)
                    head_norm(h, ps_a)
            else:
                for h in range(HG):
                    ps_a = psum.tile([128, 512], F32, tag="pa", bufs=2,
                                     name="ps_a")[0:DH + 1]
                    head_scores(h, ps_a)
                    head_norm(h, ps_a)

            # attention output projection for this token block (row-sharded);
            # residual pre-folded as +h/4 so the AllReduce output IS new h.
            # high_priority: this chain feeds the AllReduce that LN2 waits on,
            # so its ACT/DVE ops must not queue behind the next block's exps.
            armo = trans.tile([128, KO, 512], BF16, tag="armo", bufs=2)
            with tc.high_priority():
                for mo in range(KO):
                    ps = psum.tile([128, 512], F32, tag="pj", bufs=2)
                    nc.tensor.matmul(ps[:, :ql], wp_sb[:, 0:2, ts(mo, 128)],
                                     a_all[:, 0:2, ds(qs, ql)],
                                     start=True, stop=False, perf_mode=DR)
                    nc.tensor.matmul(ps[:, :ql], wp_sb[:, 2, ts(mo, 128)],
                                     a_all[:, 2, ds(qs, ql)],
                                     start=False, stop=True)
                    ptmp = trans.tile([128, 512], BF16, tag="ptmp", bufs=2)
                    nc.scalar.activation(ptmp[:, :ql], ps[:, :ql],
                                         AF.Identity, bias=bp4[:, mo:mo + 1],
                                         scale=1.0 / 256)
                    nc.vector.scalar_tensor_tensor(
                        out=armo[:, mo, :ql], in0=h_main[:, mo, ds(qs, ql)],
                        scalar=0.25, in1=ptmp[:, :ql], op0=ALU.mult,
                        op1=ALU.add)
                nc.sync.dma_start(dar_in[bi][:], armo[:, :, :ql])
            if sim_collectives:
                nc.gpsimd.collective_compute(
                    "AllReduce", ALU.add, replica_groups=GROUPS,
                    ins=[dar_in[bi].opt()], outs=[dar_out[bi].opt()],
                )
                # h_main <- allreduced (h + attn out), in place
                nc.gpsimd.dma_start(h_main[:, :, ds(qs, ql)], dar_out[bi][:])
            else:
                # local stand-in: the payload makes the same SBUF->DRAM->SBUF
                # round trip a real AllReduce result would
                nc.gpsimd.dma_start(h_main[:, :, ds(qs, ql)], dar_in[bi][:])

        # software pipeline: each query block's attention starts as soon as
        # its own q/k/v slices exist (qb0 needs only block-0 projections +
        # context KV), so attention overlaps the remaining projections
        VT = [range(0, 4), range(4, 8), range(8, 9)]  # v tiles per block
        for bi in (0, 1, 2):
            qk_proj(0, bq01, bq2, q01, q2, 0, hl, [BLKS[bi]], 0, mr1h)
            qk_proj(FG, bk01, bk2, kf01, kf2, SC, hl, [BLKS[bi]], FG, mr1h)
            v_proj(2 * FG, bv_r, hl, VT[bi], 2, 2 * FG)
            attn_block(bi)

        # ---- LN2 + MLP (per token block: stats, chain, fc, mp) ----
        z0 = pers.tile([128, KO, TOKP], BF16, tag="ln_out")

        def fc_block(bs, bl):
            for mo in range(KO):
                ps = psum.tile([128, 512], F32, tag="pa", bufs=2)
                for o in range(KO):
                    nc.tensor.matmul(
                        ps[:, :bl], wfc_sb[:, o, ts(mo, 128)],
                        z0[:, o, ds(bs, bl)],
                        start=(o == 0), stop=False,
                    )
                nc.tensor.matmul(
                    ps[:, :bl], crow[0:1, ds(3 * FG + mo * 128, 128)],
                    mr2h[0:1, ds(bs, bl)], start=False, stop=True,
                )
                nc.scalar.activation(gl[:, mo, ds(bs, bl)], ps[:, :bl],
                                     gelu_fn, bias=bfc[:, mo:mo + 1])

        drs_in = [dram.tile([KO, 128, bl], BF16, name=f"drs_in{i}")
                  for i, (_, bl) in enumerate(BLKS)]
        drs_out = [dram.tile([FG, bl], BF16, name=f"drs_out{i}")
                   for i, (_, bl) in enumerate(BLKS)]

        def mp_block(bi):
            bs, bl = BLKS[bi]
            mpart = trans.tile([128, KO, 512], BF16, tag="armo", bufs=2)
            for mo in range(KO):
                ps = psum.tile([128, 512], F32, tag="pj", bufs=2)
                for o in range(KO):
                    nc.tensor.matmul(
                        ps[:, :bl], wmp_sb[:, o, ts(mo, 128)],
                        gl[:, o, ds(bs, bl)],
                        start=(o == 0), stop=(o == KO - 1),
                    )
                mtmp = trans.tile([128, 512], BF16, tag="ptmp", bufs=2)
                with tc.high_priority():
                    nc.scalar.activation(mtmp[:, :bl], ps[:, :bl],
                                         AF.Identity,
                                         bias=bmp4[:, mo:mo + 1])
                    # mpart = h_main/4 + (mproj partial + b_mproj/4)
                    nc.vector.scalar_tensor_tensor(
                        out=mpart[:, mo, :bl], in0=h_main[:, mo, ds(bs, bl)],
                        scalar=0.25, in1=mtmp[:, :bl], op0=ALU.mult,
                        op1=ALU.add)
            if sim_collectives:
                nc.sync.dma_start(
                    drs_in[bi].rearrange("o p n -> p o n"), mpart[:, :, :bl])
                nc.gpsimd.collective_compute(
                    "ReduceScatter", ALU.add, replica_groups=GROUPS,
                    ins=[drs_in[bi].opt()], outs=[drs_out[bi].opt()],
                )
                nc.sync.dma_start(out_d[:, ds(bs, bl)], drs_out[bi][:])
            else:
                # local stand-in for ReduceScatter: this core's output slice
                # comes straight from SBUF (a real RS also lands in out_d)
                nc.gpsimd.dma_start(out_d[0:128, ds(bs, bl)],
                                    mpart[:, 0, :bl])
                nc.gpsimd.dma_start(out_d[128:FG, ds(bs, bl)],
                                    mpart[0:64, 1, :bl])

        t0 = ln_stats(h_main, *BLKS[0])
        t1 = ln_stats(h_main, *BLKS[1])
        ln_chain(h_main, z0, mr2h, *BLKS[0], *t0)
        t2 = ln_stats(h_main, *BLKS[2])
        ln_chain(h_main, z0, mr2h, *BLKS[1], *t1)
        ln_chain(h_main, z0, mr2h, *BLKS[2], *t2)
        for bi in range(3):
            fc_block(*BLKS[bi])
            mp_block(bi)

        if debug_taps:
            for nm, t in [("dbg_hl", hl), ("dbg_q01", q01),
                          ("dbg_kf01", kf01), ("dbg_vsb", vsb),
                          ("dbg_a", a_all), ("dbg_h", h_main),
                          ("dbg_z0", z0), ("dbg_gl", gl),
                          ("dbg_dar_in0", dar_in[0]),
                          ("dbg_dar_out0", dar_out[0])]:
                d = nc.dram_tensor(nm, list(t.shape), t.dtype,
                                   kind="ExternalOutput")
                nc.sync.dma_start(d[:], t[:])

    nc.compile()
    return nc


_NC_CACHE = None


def _get_program():
    global _NC_CACHE
    if _NC_CACHE is None:
        _NC_CACHE = build_program()
    return _NC_CACHE


def make_in_maps(inputs):
    f = lambda a: np.asarray(a, dtype=np.float32)
    bf = ml_dtypes.bfloat16
    x = f(inputs["x"])
    context_seq = f(inputs["context_seq"])
    sos_h = f(inputs["sos_h"])
    g1, b1 = f(inputs["ln1_g"]), f(inputs["ln1_b"])
    W_attn, b_attn = f(inputs["W_attn"]), f(inputs["b_attn"])
    W_ref, b_ref = f(inputs["W_ref"]), f(inputs["b_ref"])
    W_proj, b_proj = f(inputs["W_proj"]), f(inputs["b_proj"])
    g2, b2 = f(inputs["ln2_g"]), f(inputs["ln2_b"])
    W_fc, b_fc = f(inputs["W_fc"]), f(inputs["b_fc"])
    W_mproj, b_mproj = f(inputs["W_mproj"]), f(inputs["b_mproj"])

    # master causal mask: mask[p, c] = 1 iff p <= c - 384
    cix = np.arange(896)[None, :]
    pix = np.arange(128)[:, None]
    mask = (pix <= cix - 384).astype(np.float32)
    maskp = np.ones((128, 1088), np.float32)
    maskp[:, 0:896] = mask

    wfc_g = W_fc * g2[:, None]
    bfc_full = b2 @ W_fc + b_fc

    in_maps = []
    for core in range(N_CORES):
        b, g = core // TP, core % TP
        h = np.concatenate([sos_h[None, :], x[b]], axis=0)  # [1025, 768]
        hT = np.zeros((D, TOKP), bf)
        hT[:, :TOK] = h.T.astype(bf)
        qsl = slice(FG * g, FG * (g + 1))
        ksl = slice(D + FG * g, D + FG * (g + 1))
        vsl = slice(2 * D + FG * g, 2 * D + FG * (g + 1))
        rks = slice(FG * g, FG * (g + 1))
        rvs = slice(D + FG * g, D + FG * (g + 1))
        mcols = slice(D * g, D * (g + 1))    # W_fc column slice (768 per core)
        f8 = ml_dtypes.float8_e4m3fn
        wq = W_attn[:, qsl] * g1[:, None]
        wk = W_attn[:, ksl] * g1[:, None]
        wv = W_attn[:, vsl] * g1[:, None]
        # fp8 weights carry a x16 scale; activations x8; the psum is x128
        wqkv8 = np.concatenate(
            [wq, wk, wv, W_ref[:, rks], W_ref[:, rvs]],
            axis=1).astype(np.float32)
        wqkv8 = (16.0 * wqkv8).astype(f8)
        wp_slab = W_proj[FG * g:FG * (g + 1), :]   # [192, 768]
        wp = np.concatenate([wp_slab[0:64], wp_slab[64:128],
                             wp_slab[128:192]], axis=1)  # [64, 2304]
        wp8 = (16.0 * wp).astype(f8)
        cst = np.zeros((128, 25), np.float32)
        bq = b1 @ W_attn[:, qsl] + b_attn[qsl]
        bk = b1 @ W_attn[:, ksl] + b_attn[ksl]
        cst[:, 0] = bq[0:128]
        cst[0:64, 1] = bq[128:192]
        cst[:, 2] = bk[0:128]
        cst[0:64, 3] = bk[128:192]
        cst[:, 4] = b_ref[rks][0:128]
        cst[0:64, 5] = b_ref[rks][128:192]
        cst[:, 6:12] = (b_proj / TP).reshape(6, 128).T
        cst[:, 12:18] = bfc_full[mcols].reshape(6, 128).T
        cst[:, 18:24] = (b_mproj / TP).reshape(6, 128).T
        cst[:, 24] = EPS
        # colsums of the QUANTIZED weights, in the x128 psum domain:
        # corr = -128*colsum(w8/16)*mr = -8*colsum(w8)*mr
        w8f = wqkv8.astype(np.float32)
        rows16 = np.zeros((1, 1856), np.float32)
        rows16[0, 0:FG] = 128.0 * (b1 @ W_attn[:, vsl] + b_attn[vsl])
        rows16[0, FG:2 * FG] = 128.0 * b_ref[rvs]
        rows16[0, 384:512] = 1.0
        rows16[0, 512:1856] = np.concatenate([
            -8.0 * w8f[:, 0:FG].sum(0), -8.0 * w8f[:, FG:2 * FG].sum(0),
            -8.0 * w8f[:, 2 * FG:3 * FG].sum(0),
            -wfc_g[:, mcols].sum(0)])
        in_maps.append({
            "hT": hT,
            "ctxT": (8.0 * np.ascontiguousarray(context_seq[b].T)
                     ).astype(f8),
            "wqkv": np.ascontiguousarray(wqkv8),
            "wp": np.ascontiguousarray(wp8),
            "wfc": np.ascontiguousarray(wfc_g[:, mcols]).astype(bf),
            "wmp": np.ascontiguousarray(W_mproj[mcols, :]).astype(bf),
            "cst": cst,
            "rows16": rows16.astype(bf),
            "maskp": maskp.astype(bf),
        })
    return in_maps


def assemble_output(results, B=2):
    out = np.empty((B, S, D), np.float32)
    for b in range(B):
        parts = [np.asarray(results[TP * b + g]["out"], np.float32)
                 for g in range(TP)]
        full = np.concatenate(parts, axis=0)  # [768, 1025]
        out[b] = full[:, 1:TOK].T
    return out


def kernel(**inputs):
    nc = _get_program()
    in_maps = make_in_maps(inputs)
    res = run_bass_kernel_spmd(nc, in_maps, list(range(N_CORES)))
    return assemble_output(res.results, B=np.asarray(inputs["x"]).shape[0])


if __name__ == "__main__":
    import reference
    ins = reference.setup_inputs()
    ins = {k: np.asarray(v) for k, v in ins.items()}
    got = kernel(**ins)
    exp = np.asarray(reference.reference(**ins))
    err = np.abs(got - exp).max() / np.abs(exp).max()
    print("max abs err:", np.abs(got - exp).max(), "rel:", err)
